# revision 1
# baseline (speedup 1.0000x reference)
"""GAT (2-layer, single-head) Trainium2 Bass kernel, 8-core SPMD.

Strategy (edge/graph parallelism per the sharding hint):
  - Destination nodes are 1D-sharded: core c owns nodes [c*12500, (c+1)*12500).
  - Edges are routed to the core owning their destination (host bucketing by
    dst), grouped into 128-node dst blocks; per block, edges are bucketed by
    source segment (32k node ranges) so gathers can use int16 indices.
  - Each core computes its slice of the per-node feature table
    feat = [1 | x@W | x@W@al | x@W@ar | pad] ([12544, 64] f32, 256B rows)
    and the full table is AllGathered so every core can gather src rows.
  - Edge phase per dst block: one dma_gather per (block-group, segment) pulls
    feat[src] rows; one dma_gather per block-group pulls er[dst] (from the
    core-local slice); attention logits e = leaky_relu(el[src] + er[dst]),
    ex = exp(e) (softmax shift-invariance: max-subtraction dropped; logits
    are O(1) so fp32 exp is safe); a one-hot segment-reduction matmul per
    128-edge chunk: onehot_ex = (iota == dst_local) * ex built in a single
    two-op tensor_scalar, then PSUM-accumulated out = onehot_ex^T @ [1|feat],
    giving softmax denominator (col 0) and numerator in one chain.
  - out_block = numerator / denom + bias (+ relu between layers); layer-2
    table is produced inline per block (PE transpose + matmul), AllGathered,
    and the edge phase repeats; each core writes its [12500, 32] out slice.
"""

import numpy as np

N = 100000
E = 1600000
F = 128
H = 32
NCORES = 8
NPC = N // NCORES          # nodes per core
P = 128
NB = (NPC + P - 1) // P    # dst blocks per core (98; last block 84 rows)
LB = NPC - (NB - 1) * P    # rows in last block
NPCP = NB * P              # padded nodes per core (table rows per core)
TROWS = NCORES * NPCP      # full table rows
TW = 64                    # table row: [1, feat(32), el, er, pad...] = 256B
ELCOL = 1 + H              # 33
ERCOL = 2 + H              # 34
SEG = 32768                # src segment size (int16 gather indices)
NSEG = (TROWS + SEG - 1) // SEG
GB = 1                     # dst blocks per gather group
NGB = (NB + GB - 1) // GB
CAPC = 8                   # max chunks (1024 indices) per dma_gather op

_cache = {}


def _plan(R):
    """Chunk-column layout shared by host prep and program build.

    R: [NB, NSEG] chunks per (block, segment).
    Returns (CH, groups) where groups[g] = (cg0, Rg, feat_ops, blocks);
    feat_ops[s] = (c0, R_gs); blocks[b] = list of (s, c0, Rbs).
    """
    groups = []
    c = 0
    for g in range(NGB):
        bs = list(range(g * GB, min((g + 1) * GB, NB)))
        cg0 = c
        feat_ops = []
        blocks = {b: [] for b in bs}
        for s in range(NSEG):
            c0 = c
            for b in bs:
                blocks[b].append((s, c, int(R[b, s])))
                c += int(R[b, s])
            feat_ops.append((c0, c - c0))
        groups.append((cg0, c - cg0, feat_ops, blocks))
    return c, groups


def _wrap16(i_op):
    """dma_gather index layout: op-local index i -> (row, col16)."""
    return i_op % 16, i_op // 16


def _host_prep(x, src, dst, W1, al1, ar1, b1, W2, al2, ar2, b2):
    f32, i16 = np.float32, np.int16
    src = np.asarray(src).astype(np.int64)
    dst = np.asarray(dst).astype(np.int64)

    core = dst // NPC
    r = dst % NPC
    b = r // P
    dl = (r % P).astype(f32)
    trow_src = (src // NPC) * NPCP + (src % NPC)
    seg = trow_src // SEG
    bgid = core * NB + b

    order = np.lexsort((seg, bgid))
    s_src_trow = trow_src[order]
    s_seg = seg[order]
    s_bgid = bgid[order]
    s_dl = dl[order]
    s_b = b[order]

    key = s_bgid * NSEG + s_seg
    counts = np.bincount(key, minlength=NCORES * NB * NSEG)
    counts3 = counts.reshape(NCORES, NB, NSEG)
    R = -(-counts3.max(axis=0) // P)          # [NB, NSEG] chunks (may be 0)
    R = np.maximum(R, 1)
    CH, groups = _plan(R)

    # per-edge rank within its (core, block, seg) run
    seg_start = np.concatenate([[0], np.cumsum(counts)])[:-1]
    rank = np.arange(len(order), dtype=np.int64) - seg_start[key]

    # chunk column of each (block, seg): c0 table
    c0_tab = np.zeros((NB, NSEG), np.int64)
    for g, (cg0, Rg, feat_ops, blocks) in enumerate(groups):
        for bb, lst in blocks.items():
            for (s, c0, _Rbs) in lst:
                c0_tab[bb, s] = c0
    cg0_of_b = np.zeros(NB, np.int64)
    for g, (cg0, Rg, feat_ops, blocks) in enumerate(groups):
        for bb in blocks:
            cg0_of_b[bb] = cg0

    col = c0_tab[s_b, s_seg] + rank // P
    p = rank % P

    dstl = np.full((NCORES, P, CH), 200.0, f32)
    flat = s_bgid // NB * (P * CH) + p * CH + col
    dstl.reshape(-1)[flat] = s_dl

    # feat gather indices: [16, CH*8] wrapped, relative to segment base
    fidx = np.zeros((NCORES, 16, CH * 8), i16)
    i_op = (col - c0_tab[s_b, s_seg]) * P + p
    row16, col16 = _wrap16(i_op)
    abscol = c0_tab[s_b, s_seg] * 8 + col16
    fflat = (s_bgid // NB) * (16 * CH * 8) + row16 * (CH * 8) + abscol
    fidx.reshape(-1)[fflat] = (s_src_trow - s_seg * SEG).astype(i16)

    # er gather indices: relative to group window (GB*128 rows)
    eidx = np.zeros((NCORES, 16, CH * 8), i16)
    i_op = (col - cg0_of_b[s_b]) * P + p
    row16, col16 = _wrap16(i_op)
    abscol = cg0_of_b[s_b] * 8 + col16
    eflat = (s_bgid // NB) * (16 * CH * 8) + row16 * (CH * 8) + abscol
    er_local = (s_b - (s_b // GB) * GB) * P + s_dl.astype(np.int64)
    eidx.reshape(-1)[eflat] = er_local.astype(i16)

    fidx = np.tile(fidx, (1, 8, 1))
    eidx = np.tile(eidx, (1, 8, 1))

    def aug(W, al, ar):
        Wa = np.zeros((W.shape[0], TW), f32)
        Wa[:, 1:1 + H] = W
        Wa[:, ELCOL] = W @ al
        Wa[:, ERCOL] = W @ ar
        return Wa

    W1a = aug(np.asarray(W1, f32), np.asarray(al1, f32), np.asarray(ar1, f32))
    W2a = aug(np.asarray(W2, f32), np.asarray(al2, f32), np.asarray(ar2, f32))
    b1r = np.tile(np.asarray(b1, f32)[None, :], (P, 1))
    b2r = np.tile(np.asarray(b2, f32)[None, :], (P, 1))
    iota = np.tile(np.arange(P, dtype=f32)[None, :], (P, 1))

    x = np.asarray(x, f32)
    xsT = np.zeros((NCORES, F, NPCP), f32)
    for cc in range(NCORES):
        xsT[cc, :, :NPC] = x[cc * NPC:(cc + 1) * NPC].T

    in_maps = []
    for cc in range(NCORES):
        in_maps.append({
            "xsT": xsT[cc],
            "W1a": W1a, "W2a": W2a, "b1r": b1r, "b2r": b2r, "iota": iota,
            "fidx": fidx[cc], "eidx": eidx[cc], "dstl": dstl[cc],
        })
    return in_maps, tuple(int(v) for v in R.reshape(-1))


def _build_program(R_key, single=False):
    import concourse.bacc as bacc
    import concourse.mybir as mybir
    import concourse.tile as tile
    from concourse.masks import make_identity

    dt = mybir.dt
    R = np.asarray(R_key, np.int64).reshape(NB, NSEG)
    CH, groups = _plan(R)
    ncores = 1 if single else NCORES

    nc = bacc.Bacc("TRN2", target_bir_lowering=False, debug=False,
                   num_devices=ncores, num_swdge_queues=4)

    xsT = nc.dram_tensor("xsT", [F, NPCP], dt.float32, kind="ExternalInput")
    W1a = nc.dram_tensor("W1a", [F, TW], dt.float32, kind="ExternalInput")
    W2a = nc.dram_tensor("W2a", [H, TW], dt.float32, kind="ExternalInput")
    b1r = nc.dram_tensor("b1r", [P, H], dt.float32, kind="ExternalInput")
    b2r = nc.dram_tensor("b2r", [P, H], dt.float32, kind="ExternalInput")
    iota = nc.dram_tensor("iota", [P, P], dt.float32, kind="ExternalInput")
    fidx = nc.dram_tensor("fidx", [P, CH * 8], dt.int16, kind="ExternalInput")
    eidx = nc.dram_tensor("eidx", [P, CH * 8], dt.int16, kind="ExternalInput")
    dstl = nc.dram_tensor("dstl", [P, CH], dt.float32, kind="ExternalInput")
    out_ext = nc.dram_tensor("out", [NPC, H], dt.float32, kind="ExternalOutput")

    qn_state = [0]

    def qn():
        qn_state[0] = (qn_state[0] + 1) % 4
        return qn_state[0]

    with tile.TileContext(nc) as tc:
        with (
            tc.tile_pool(name="const", bufs=1) as const,
            tc.tile_pool(name="prod", bufs=4) as prod,
            tc.tile_pool(name="gath", bufs=16) as gpool,
            tc.tile_pool(name="erg", bufs=8) as erpool,
            tc.tile_pool(name="edge", bufs=6) as epool,
            tc.tile_pool(name="oh", bufs=12) as ohpool,
            tc.tile_pool(name="epi", bufs=4) as epipool,
            tc.tile_pool(name="ps", bufs=4, space="PSUM") as psum,
            tc.tile_pool(name="pst", bufs=2, space="PSUM") as psumt,
            tc.tile_pool(name="dram", bufs=1, space="DRAM") as dram,
        ):
            iota_sb = const.tile([P, P], dt.float32)
            nc.sync.dma_start(out=iota_sb[:], in_=iota[:])
            W1a_sb = const.tile([F, TW], dt.float32)
            nc.sync.dma_start(out=W1a_sb[:], in_=W1a[:])
            W2a_sb = const.tile([H, TW], dt.float32)
            nc.sync.dma_start(out=W2a_sb[:], in_=W2a[:])
            b1r_sb = const.tile([P, H], dt.float32)
            nc.sync.dma_start(out=b1r_sb[:], in_=b1r[:])
            b2r_sb = const.tile([P, H], dt.float32)
            nc.sync.dma_start(out=b2r_sb[:], in_=b2r[:])
            fidx_sb = const.tile([P, CH * 8], dt.int16)
            nc.sync.dma_start(out=fidx_sb[:], in_=fidx[:])
            eidx_sb = const.tile([P, CH * 8], dt.int16)
            nc.sync.dma_start(out=eidx_sb[:], in_=eidx[:])
            dstl_sb = const.tile([P, CH], dt.float32)
            nc.sync.dma_start(out=dstl_sb[:], in_=dstl[:])
            ident = const.tile([P, P], dt.float32)
            make_identity(nc, ident[:])

            feat1_s = dram.tile([NPCP, TW], dt.float32)
            feat1_f = dram.tile([TROWS, TW], dt.float32, addr_space="Shared")
            feat2_s = dram.tile([NPCP, TW], dt.float32)
            feat2_f = dram.tile([TROWS, TW], dt.float32, addr_space="Shared")

            # ---- layer-1 table production ----
            for b in range(NB):
                xt = prod.tile([F, P], dt.float32, tag="xt")
                nc.sync.dma_start(out=xt[:], in_=xsT[:, b * P:(b + 1) * P])
                pmm = psumt.tile([P, TW], dt.float32, tag="pmm")
                nc.tensor.matmul(out=pmm[:], lhsT=xt[:], rhs=W1a_sb[:],
                                 start=True, stop=True)
                fsb = prod.tile([P, TW], dt.float32, tag="fsb")
                nc.vector.tensor_copy(out=fsb[:, 1:], in_=pmm[:, 1:])
                nc.vector.memset(fsb[:, 0:1], 1.0)
                nc.sync.dma_start(out=feat1_s[b * P:(b + 1) * P, :],
                                  in_=fsb[:])

            def allgather(src_t, dst_t):
                if single:
                    nc.sync.dma_start(out=dst_t[0:NPCP, :], in_=src_t[:])
                else:
                    nc.gpsimd.collective_compute(
                        "AllGather", mybir.AluOpType.bypass,
                        replica_groups=[list(range(NCORES))],
                        ins=[src_t[:]], outs=[dst_t[:]],
                    )

            allgather(feat1_s, feat1_f)

            # ---- edge phase ----
            def edge_phase(feat_f, feat_s, bias_sb, relu, out_writer):
                def emit_gather(tt, in_ap, idx_sb, c0, R):
                    # ucode caps one dma_gather at 1024 indices (8 chunks)
                    tv = tt[:].rearrange("p (r e) -> p r e", e=TW)
                    for off in range(0, R, CAPC):
                        take = min(CAPC, R - off)
                        nc.gpsimd.dma_gather(
                            out_ap=tv[:, off:off + take, :],
                            in_ap=in_ap,
                            idxs_ap=idx_sb[:, (c0 + off) * 8:
                                           (c0 + off + take) * 8],
                            num_idxs=take * P, num_idxs_reg=take * P,
                            elem_size=TW, queue_num=qn(),
                        )

                for g, (cg0, Rg, feat_ops, blocks) in enumerate(groups):
                    erg = erpool.tile([P, Rg * TW], dt.float32, tag="erg")
                    emit_gather(
                        erg,
                        feat_s[g * GB * P:(g * GB + len(blocks)) * P, :],
                        eidx_sb, cg0, Rg)
                    tts = []
                    for s, (c0, Rgs) in enumerate(feat_ops):
                        tt = gpool.tile([P, Rgs * TW], dt.float32,
                                        tag=f"T{s}")
                        seg_lo = s * SEG
                        seg_hi = min(seg_lo + SEG, TROWS)
                        emit_gather(tt, feat_f[seg_lo:seg_hi, :],
                                    fidx_sb, c0, Rgs)
                        tts.append((tt, c0))
                    for b in sorted(blocks):
                        chunks = blocks[b]   # [(s, c0, Rbs)]
                        nch = sum(rr for (_s, _c, rr) in chunks)
                        pacc = psum.tile([P, 1 + H], dt.float32, tag="pacc")
                        done = 0
                        for (s, c0b, Rbs) in chunks:
                            tt, c0op = tts[s]
                            tv = tt[:].rearrange("p (r e) -> p r e", e=TW)
                            ev = erg[:].rearrange("p (r e) -> p r e", e=TW)
                            rb0 = c0b - c0op
                            re0 = c0b - cg0
                            ee = epool.tile([P, Rbs], dt.float32, tag="ee")
                            nc.vector.tensor_tensor(
                                out=ee[:], in0=tv[:, rb0:rb0 + Rbs, ELCOL],
                                in1=ev[:, re0:re0 + Rbs, ERCOL],
                                op=mybir.AluOpType.add)
                            et = epool.tile([P, Rbs], dt.float32, tag="et")
                            nc.vector.tensor_scalar_mul(out=et[:], in0=ee[:],
                                                        scalar1=0.2)
                            nc.vector.tensor_tensor(
                                out=ee[:], in0=ee[:], in1=et[:],
                                op=mybir.AluOpType.max)
                            ex = epool.tile([P, Rbs], dt.float32, tag="ex")
                            nc.scalar.activation(
                                out=ex[:], in_=ee[:],
                                func=mybir.ActivationFunctionType.Exp)
                            for rr in range(Rbs):
                                oh = ohpool.tile([P, P], dt.float32, tag="oh")
                                nc.vector.tensor_scalar(
                                    out=oh[:], in0=iota_sb[:],
                                    scalar1=dstl_sb[:, c0b + rr:c0b + rr + 1],
                                    scalar2=ex[:, rr:rr + 1],
                                    op0=mybir.AluOpType.is_equal,
                                    op1=mybir.AluOpType.mult,
                                )
                                nc.tensor.matmul(
                                    out=pacc[:], lhsT=oh[:],
                                    rhs=tv[:, rb0 + rr, 0:1 + H],
                                    start=(done == 0),
                                    stop=(done == nch - 1),
                                )
                                done += 1
                        den = epipool.tile([P, 1], dt.float32, tag="den")
                        nc.vector.tensor_scalar_add(out=den[:],
                                                    in0=pacc[:, 0:1],
                                                    scalar1=1e-30)
                        rec = epipool.tile([P, 1], dt.float32, tag="rec")
                        nc.vector.reciprocal(out=rec[:], in_=den[:])
                        h = epipool.tile([P, H], dt.float32, tag="h")
                        nc.vector.tensor_scalar_mul(out=h[:], in0=pacc[:, 1:],
                                                    scalar1=rec[:])
                        nc.vector.tensor_tensor(out=h[:], in0=h[:],
                                                in1=bias_sb[:],
                                                op=mybir.AluOpType.add)
                        if relu:
                            nc.scalar.activation(
                                out=h[:], in_=h[:],
                                func=mybir.ActivationFunctionType.Relu)
                        out_writer(b, h)

            def l1_writer(b, h):
                pt = psumt.tile([H, P], dt.float32, tag="pt")
                nc.tensor.transpose(out=pt[:], in_=h[:], identity=ident[:])
                hT = prod.tile([H, P], dt.float32, tag="hT")
                nc.vector.tensor_copy(out=hT[:], in_=pt[:])
                pmm2 = psumt.tile([P, TW], dt.float32, tag="pmm")
                nc.tensor.matmul(out=pmm2[:], lhsT=hT[:], rhs=W2a_sb[:],
                                 start=True, stop=True)
                f2 = prod.tile([P, TW], dt.float32, tag="fsb")
                nc.vector.tensor_copy(out=f2[:, 1:], in_=pmm2[:, 1:])
                nc.vector.memset(f2[:, 0:1], 1.0)
                nc.sync.dma_start(out=feat2_s[b * P:(b + 1) * P, :],
                                  in_=f2[:])

            edge_phase(feat1_f, feat1_s, b1r_sb, True, l1_writer)
            allgather(feat2_s, feat2_f)

            def l2_writer(b, h):
                rows = LB if b == NB - 1 else P
                nc.sync.dma_start(out=out_ext[b * P:b * P + rows, :],
                                  in_=h[:rows, :])

            edge_phase(feat2_f, feat2_s, b2r_sb, False, l2_writer)

    nc.compile()
    return nc


def _get_program(R_key, single=False):
    key = ("prog", R_key, single)
    if key not in _cache:
        _cache[key] = _build_program(R_key, single=single)
    return _cache[key]


def kernel(x, src, dst, W1, al1, ar1, b1, W2, al2, ar2, b2):
    from concourse.bass_utils import run_bass_kernel_spmd

    in_maps, R_key = _host_prep(x, src, dst, W1, al1, ar1, b1,
                                W2, al2, ar2, b2)
    nc = _get_program(R_key)
    res = run_bass_kernel_spmd(nc, in_maps, list(range(NCORES)))
    out = np.concatenate([res.results[c]["out"] for c in range(NCORES)],
                         axis=0)
    return out.astype(np.float32)



# revision 2
# speedup vs baseline: 1.0807x; 1.0807x over previous
"""GAT (2-layer, single-head) Trainium2 Bass kernel, 8-core SPMD. v2.

Design (vs v1 baseline):
  - dst nodes 1D-sharded (12500/core); edges routed to dst core, bucketed
    by (dst block of 128, src segment of 32768 table rows), packed into
    128-edge chunk columns, s-major within groups of G blocks so one
    dma_gather op (<=1024 idxs, ucode cap) spans buckets.
  - Table row = 128 bf16 (256B, the dma_gather minimum elem):
    [el as f32 (2 slots) | 1.0 | feat(32) | pad]. el kept f32 for softmax
    accuracy; feat bf16.
  - NO per-edge er gather (v1 spent ~50% of its descriptors on it).
    Instead er[dst] is selected on-chip: host ships, per chunk, the
    TRANSPOSED one-hot M[k,i] = (k == dl[i]) as an fp8 constant streamed
    from DRAM; er_slot column = matmul(lhsT=M_c, rhs=er_col fp16).
  - ex applied to the matmul RHS: pacc += OH_c^T @ (ex * [1|feat]).
    OH (slot->node one-hot) is then STATIC 0/1: built per (block, s-run)
    in ONE batched DVE is_equal with stride-0 broadcast APs (bf16, 2x).
  - Epilogue: out = pacc[:,1:]/pacc[:,0] + bias (+relu), layer-2 table
    built inline; AllGather between layers (excluded from the metric, a
    local copy in the single-core cost program).
"""

import numpy as np
import ml_dtypes

N = 100000
E = 1600000
F = 128
H = 32
NCORES = 8
NPC = N // NCORES          # nodes per core
P = 128
NB = (NPC + P - 1) // P    # dst blocks per core (98; last block 84 rows)
LB = NPC - (NB - 1) * P    # rows in last block
NPCP = NB * P              # padded nodes per core (table rows per core)
TROWS = NCORES * NPCP      # full table rows
TW = 128                   # table row: 128 bf16 = 256B
ELC = 0                    # cols 0-1: el as f32
ONEC = 2                   # col 2: 1.0
FEATC = 3                  # cols 3..34: feat
WFW = 1 + H                # aggregation rhs width: [1 | feat]
SEG = 32768                # src segment size (int16 gather indices)
NSEG = (TROWS + SEG - 1) // SEG
G = 8                      # dst blocks per group tile
NG = (NB + G - 1) // G
CAPC = 8                   # max chunks (1024 indices) per dma_gather op

_cache = {}


def _plan(R):
    """Column layout. R: [NB, NSEG] chunks per bucket.

    Group g covers blocks [gB, gB+G). Columns are s-major within the
    group: for s in segs, for b in group, R[b,s] columns.
    Returns (CH, groups); groups[g] = dict with:
      g0: first global column of the group
      ops: list of (s, c0, nch) gather ops (global col, chunks <= CAPC)
      blocks: {b: [(s, gcol0, Rbs, bcol0)]} runs per block; bcol0 is the
        block-local column offset (block columns are the concat of its
        s-runs, in s order).
    """
    groups = []
    c = 0
    for g in range(NG):
        bs = list(range(g * G, min((g + 1) * G, NB)))
        g0 = c
        ops = []
        blocks = {b: [] for b in bs}
        bcol = {b: 0 for b in bs}
        for s in range(NSEG):
            s0 = c
            for b in bs:
                r = int(R[b, s])
                blocks[b].append((s, c, r, bcol[b]))
                bcol[b] += r
                c += r
            nch = c - s0
            off = 0
            while off < nch:
                take = min(CAPC, nch - off)
                ops.append((s, s0 + off, take))
                off += take
        groups.append({"g0": c - (c - g0), "ops": ops, "blocks": blocks,
                       "nch": c - g0})
        groups[-1]["g0"] = g0
    return c, groups


def _host_prep(x, src, dst, W1, al1, ar1, b1, W2, al2, ar2, b2):
    f32, bf16, i16 = np.float32, ml_dtypes.bfloat16, np.int16
    src = np.asarray(src).astype(np.int64)
    dst = np.asarray(dst).astype(np.int64)

    core = dst // NPC
    r = dst % NPC
    b = r // P
    dl = r % P
    trow_src = (src // NPC) * NPCP + (src % NPC)
    seg = trow_src // SEG
    bgid = core * NB + b

    key = bgid * NSEG + seg
    counts = np.bincount(key, minlength=NCORES * NB * NSEG)
    counts3 = counts.reshape(NCORES, NB, NSEG)
    R = -(-counts3.max(axis=0) // P)          # [NB, NSEG]
    R = np.maximum(R, 1)
    CH, groups = _plan(R)

    order = np.argsort(key, kind="stable")
    s_trow = trow_src[order]
    s_seg = seg[order]
    s_b = b[order]
    s_bgid = bgid[order]
    s_dl = dl[order]

    seg_start = np.concatenate([[0], np.cumsum(counts)])[:-1]
    rank = np.arange(len(order), dtype=np.int64) - seg_start[key[order]]

    c0_tab = np.zeros((NB, NSEG), np.int64)   # global col of bucket start
    for gr in groups:
        for bb, runs in gr["blocks"].items():
            for (s, gcol0, _r, _bc) in runs:
                c0_tab[bb, s] = gcol0

    col = c0_tab[s_b, s_seg] + rank // P      # global column
    p = rank % P

    # dstl: [128, CH] destination-local row per slot (200 = pad)
    dstl = np.full((NCORES, P, CH), 200.0, f32)
    flat = (s_bgid // NB) * (P * CH) + p * CH + col
    dstl.reshape(-1)[flat] = s_dl

    # gather indices: [16 wrap, CH*8], segment-relative
    fidx = np.zeros((NCORES, 16, CH * 8), i16)
    i_op = (col - c0_tab[s_b, s_seg]) * P + p
    # op-local index: ops start at 8-column boundaries within each s-run
    # of a group: recompute relative to the op start column.
    # ops cover [c0, c0+take) chunks; op base = s-run start + 8k.
    srun0 = np.zeros((NB, NSEG), np.int64)    # s-run start col in group
    for gr in groups:
        for (s, c0, take) in gr["ops"]:
            pass
    # op base for column col in s-run starting at sc0: sc0 + ((col-sc0)//8)*8
    sc0_tab = np.zeros((NB, NSEG), np.int64)
    for gr in groups:
        for s in range(NSEG):
            cols = [(c0, r) for bb2, runs in gr["blocks"].items()
                    for (ss, c0, r, _bc) in runs if ss == s]
            if cols:
                sc0 = min(c0 for c0, _ in cols)
                for bb2, runs in gr["blocks"].items():
                    for (ss, c02, r2, _bc) in runs:
                        if ss == s:
                            sc0_tab[bb2, s] = sc0
    opbase = sc0_tab[s_b, s_seg] + ((col - sc0_tab[s_b, s_seg]) // CAPC) * CAPC
    i_op = (col - opbase) * P + p
    row16 = i_op % 16
    col16 = i_op // 16
    abscol = opbase * 8 + col16
    fflat = (s_bgid // NB) * (16 * CH * 8) + row16 * (CH * 8) + abscol
    fidx.reshape(-1)[fflat] = (s_trow - s_seg * SEG).astype(i16)
    fidx = np.tile(fidx, (1, 8, 1))

    # M fp8: [128, CH*128], M[:, c*128+i][k] = (k == dstl[i, c])
    k_iota = np.arange(P, dtype=np.int64)
    Mall = np.zeros((NCORES, P, CH * P), np.uint8)
    one8 = np.float32(1.0).astype(ml_dtypes.float8_e4m3).view(np.uint8)
    for cc in range(NCORES):
        d = dstl[cc]                          # [P(slots) , CH]
        # M[k, c*128+i] = (d[i, c] == k)
        eq = (d.astype(np.int64).T[:, :, None] == k_iota[None, None, :])
        # eq: [CH, i, k] -> M[k, c*128+i]
        M = np.transpose(eq, (2, 0, 1)).reshape(P, CH * P)
        Mall[cc][M] = one8
    Mall = Mall.view(ml_dtypes.float8_e4m3)

    def aug(W, al, ar):
        W = np.asarray(W, f32)
        Wa = np.zeros((W.shape[0], 2 + H), f32)
        Wa[:, 0] = W @ np.asarray(al, f32)
        Wa[:, 1:1 + H] = W
        Wa[:, 1 + H] = W @ np.asarray(ar, f32)
        return Wa

    W1a = aug(W1, al1, ar1)
    W2a = aug(W2, al2, ar2)
    b1r = np.tile(np.asarray(b1, f32)[None, :], (P, 1))
    b2r = np.tile(np.asarray(b2, f32)[None, :], (P, 1))
    iota = np.tile(np.arange(P, dtype=bf16)[None, :], (P, 1))

    x = np.asarray(x, f32)
    xsT = np.zeros((NCORES, F, NPCP), f32)
    for cc in range(NCORES):
        xsT[cc, :, :NPC] = x[cc * NPC:(cc + 1) * NPC].T

    in_maps = []
    for cc in range(NCORES):
        in_maps.append({
            "xsT": xsT[cc],
            "W1a": W1a, "W2a": W2a, "b1r": b1r, "b2r": b2r, "iota": iota,
            "fidx": fidx[cc], "dstl": dstl[cc],
            "m8": Mall[cc],
        })
    return in_maps, tuple(int(v) for v in R.reshape(-1))


def _build_program(R_key, single=False):
    import concourse.bacc as bacc
    import concourse.mybir as mybir
    import concourse.tile as tile

    dt = mybir.dt
    R = np.asarray(R_key, np.int64).reshape(NB, NSEG)
    CH, groups = _plan(R)
    ncores = 1 if single else NCORES

    nc = bacc.Bacc("TRN2", target_bir_lowering=False, debug=False,
                   num_devices=ncores, num_swdge_queues=4)

    xsT = nc.dram_tensor("xsT", [F, NPCP], dt.float32, kind="ExternalInput")
    W1a = nc.dram_tensor("W1a", [F, 2 + H], dt.float32, kind="ExternalInput")
    W2a = nc.dram_tensor("W2a", [H, 2 + H], dt.float32, kind="ExternalInput")
    b1r = nc.dram_tensor("b1r", [P, H], dt.float32, kind="ExternalInput")
    b2r = nc.dram_tensor("b2r", [P, H], dt.float32, kind="ExternalInput")
    iota = nc.dram_tensor("iota", [P, P], dt.bfloat16, kind="ExternalInput")
    fidx = nc.dram_tensor("fidx", [P, CH * 8], dt.int16, kind="ExternalInput")
    dstl = nc.dram_tensor("dstl", [P, CH], dt.float32, kind="ExternalInput")
    m8 = nc.dram_tensor("m8", [P, CH * P], dt.float8e4, kind="ExternalInput")
    out_ext = nc.dram_tensor("out", [NPC, H], dt.float32, kind="ExternalOutput")

    qn_state = [0]

    def qn():
        qn_state[0] = (qn_state[0] + 1) % 4
        return qn_state[0]

    with tile.TileContext(nc) as tc:
        with (
            tc.tile_pool(name="const", bufs=1) as const,
            tc.tile_pool(name="prod", bufs=4) as prod,
            tc.tile_pool(name="tv", bufs=2) as tvpool,
            tc.tile_pool(name="mg", bufs=2) as mgpool,
            tc.tile_pool(name="fxg", bufs=2) as fxpool,
            tc.tile_pool(name="oh", bufs=12) as ohpool,
            tc.tile_pool(name="ee", bufs=4) as eepool,
            tc.tile_pool(name="epi", bufs=4) as epipool,
            tc.tile_pool(name="ps", bufs=3, space="PSUM") as psum,
            tc.tile_pool(name="pse", bufs=2, space="PSUM") as psume,
            tc.tile_pool(name="pst", bufs=1, space="PSUM") as psumt,
            tc.tile_pool(name="dram", bufs=1, space="DRAM") as dram,
        ):
            iota_sb = const.tile([P, P], dt.bfloat16)
            nc.sync.dma_start(out=iota_sb[:], in_=iota[:])
            W1a_sb = const.tile([F, 2 + H], dt.float32)
            nc.sync.dma_start(out=W1a_sb[:], in_=W1a[:])
            W2a_sb = const.tile([H, 2 + H], dt.float32)
            nc.sync.dma_start(out=W2a_sb[:], in_=W2a[:])
            b1r_sb = const.tile([P, H], dt.float32)
            nc.sync.dma_start(out=b1r_sb[:], in_=b1r[:])
            b2r_sb = const.tile([P, H], dt.float32)
            nc.sync.dma_start(out=b2r_sb[:], in_=b2r[:])
            dstl_sb = const.tile([P, CH], dt.float32)
            nc.sync.dma_start(out=dstl_sb[:], in_=dstl[:])
            er_all = const.tile([P, NB], dt.float16)
            er_all2 = const.tile([P, NB], dt.float16)

            feat1_s = dram.tile([NPCP, TW], dt.bfloat16)
            feat1_f = dram.tile([TROWS, TW], dt.bfloat16, addr_space="Shared")
            feat2_s = dram.tile([NPCP, TW], dt.bfloat16)
            feat2_f = dram.tile([TROWS, TW], dt.bfloat16, addr_space="Shared")

            def build_row(pmm, er_dst, b, fsb4, k):
                """pmm [P, 2+H] f32 = [el | feat | er] -> bf16 row + er."""
                o = k * TW
                nc.vector.tensor_copy(
                    out=fsb4[:, o + ELC:o + ELC + 2].bitcast(dt.float32),
                    in_=pmm[:, 0:1])
                nc.vector.memset(fsb4[:, o + ONEC:o + ONEC + 1], 1.0)
                nc.vector.tensor_copy(out=fsb4[:, o + FEATC:o + FEATC + H],
                                      in_=pmm[:, 1:1 + H])
                nc.vector.tensor_copy(out=er_dst[:, b:b + 1],
                                      in_=pmm[:, 1 + H:2 + H])

            TB = 4

            def write_rows(feat_s, b0, nb, fsb4):
                nc.sync.dma_start(
                    out=feat_s[b0 * P:(b0 + nb) * P, :]
                        .rearrange("(blk r) w -> r blk w", blk=nb),
                    in_=fsb4[:, :nb * TW].rearrange("p (blk w) -> p blk w",
                                                    w=TW))

            # ---- layer-1 table ----
            for b0 in range(0, NB, TB):
                nb = min(TB, NB - b0)
                xt = prod.tile([F, TB * P], dt.float32, tag="xt")
                nc.sync.dma_start(out=xt[:, :nb * P],
                                  in_=xsT[:, b0 * P:(b0 + nb) * P])
                fsb4 = prod.tile([P, TB * TW], dt.bfloat16, tag="fsb4")
                for k in range(nb):
                    pmm = psumt.tile([P, 2 + H], dt.float32, tag="pmm")
                    nc.tensor.matmul(out=pmm[:], lhsT=xt[:, k * P:(k + 1) * P],
                                     rhs=W1a_sb[:], start=True, stop=True)
                    build_row(pmm, er_all, b0 + k, fsb4, k)
                write_rows(feat1_s, b0, nb, fsb4)

            def allgather(src_t, dst_t):
                if single:
                    nc.sync.dma_start(
                        out=dst_t[0:NPCP, :].rearrange("a b -> (a b)"),
                        in_=src_t[:].rearrange("a b -> (a b)"))
                else:
                    nc.gpsimd.collective_compute(
                        "AllGather", mybir.AluOpType.bypass,
                        replica_groups=[list(range(NCORES))],
                        ins=[src_t[:]], outs=[dst_t[:]],
                    )

            allgather(feat1_s, feat1_f)

            # ---- edge phase ----
            def edge_phase(feat_f, er_sb, bias_sb, relu, out_writer):
                for g, gr in enumerate(groups):
                    g0, nch = gr["g0"], gr["nch"]
                    fxg = fxpool.tile([P, nch * 8], dt.int16, tag="fxg")
                    nc.sync.dma_start(out=fxg[:],
                                      in_=fidx[:, g0 * 8:(g0 + nch) * 8])
                    mg = mgpool.tile([P, nch * P], dt.float8e4, tag="mg")
                    nc.sync.dma_start(out=mg[:],
                                      in_=m8[:, g0 * P:(g0 + nch) * P])
                    tvg = tvpool.tile([P, nch * TW], dt.bfloat16, tag="tvg")
                    tv3 = tvg[:].rearrange("p (r e) -> p r e", e=TW)
                    for (s, c0, take) in gr["ops"]:
                        seg_lo = s * SEG
                        seg_hi = min(seg_lo + SEG, TROWS)
                        nc.gpsimd.dma_gather(
                            out_ap=tv3[:, c0 - g0:c0 - g0 + take, :],
                            in_ap=feat_f[seg_lo:seg_hi, :],
                            idxs_ap=fxg[:, (c0 - g0) * 8:(c0 - g0 + take) * 8],
                            num_idxs=take * P, num_idxs_reg=take * P,
                            elem_size=TW, queue_num=qn(),
                        )
                    for b in sorted(gr["blocks"]):
                        runs = gr["blocks"][b]       # [(s, gcol0, Rbs, bcol0)]
                        Rb = sum(rr for (_s, _c, rr, _bc) in runs)
                        # er selection: per chunk matmul M_c^T @ er_col
                        er_ps = psume.tile([P, Rb], dt.float32, tag="erps")
                        for (s, gcol0, Rbs, bcol0) in runs:
                            for rr in range(Rbs):
                                gc = gcol0 - g0 + rr
                                nc.tensor.matmul(
                                    out=er_ps[:, bcol0 + rr:bcol0 + rr + 1],
                                    lhsT=mg[:, gc * P:(gc + 1) * P],
                                    rhs=er_sb[:, b:b + 1],
                                    start=True, stop=True)
                        # ee = el + er ; lrelu ; ex
                        ee = eepool.tile([P, Rb], dt.float32, tag="ee")
                        for (s, gcol0, Rbs, bcol0) in runs:
                            el = tv3[:, gcol0 - g0:gcol0 - g0 + Rbs,
                                     ELC:ELC + 2].bitcast(dt.float32)
                            nc.vector.tensor_tensor(
                                out=ee[:, bcol0:bcol0 + Rbs]
                                    .rearrange("p (r o) -> p r o", o=1),
                                in0=el,
                                in1=er_ps[:, bcol0:bcol0 + Rbs]
                                    .rearrange("p (r o) -> p r o", o=1),
                                op=mybir.AluOpType.add)
                        nc.vector.scalar_tensor_tensor(
                            out=ee[:], in0=ee[:], scalar=0.2, in1=ee[:],
                            op0=mybir.AluOpType.mult,
                            op1=mybir.AluOpType.max)
                        ex = eepool.tile([P, Rb], dt.float32, tag="ex")
                        nc.scalar.activation(
                            out=ex[:], in_=ee[:],
                            func=mybir.ActivationFunctionType.Exp)
                        # fused one-hot(+ex) per chunk; rhs = table slice
                        pacc = psum.tile([P, WFW], dt.float32, tag="pacc")
                        done = 0
                        for (s, gcol0, Rbs, bcol0) in runs:
                            for rr in range(Rbs):
                                oh = ohpool.tile([P, P], dt.bfloat16,
                                                 tag="oh")
                                nc.vector.tensor_scalar(
                                    out=oh[:], in0=iota_sb[:],
                                    scalar1=dstl_sb[:, gcol0 + rr:
                                                    gcol0 + rr + 1],
                                    scalar2=ex[:, bcol0 + rr:bcol0 + rr + 1],
                                    op0=mybir.AluOpType.is_equal,
                                    op1=mybir.AluOpType.mult,
                                )
                                nc.tensor.matmul(
                                    out=pacc[:],
                                    lhsT=oh[:],
                                    rhs=tv3[:, gcol0 - g0 + rr,
                                            ONEC:ONEC + WFW],
                                    start=(done == 0), stop=(done == Rb - 1))
                                done += 1
                        # epilogue
                        den = epipool.tile([P, 1], dt.float32, tag="den")
                        nc.vector.tensor_scalar_add(out=den[:],
                                                    in0=pacc[:, 0:1],
                                                    scalar1=1e-30)
                        rec = epipool.tile([P, 1], dt.float32, tag="rec")
                        nc.vector.reciprocal(out=rec[:], in_=den[:])
                        h = epipool.tile([P, H], dt.float32, tag="h")
                        nc.vector.tensor_scalar_mul(out=h[:], in0=pacc[:, 1:],
                                                    scalar1=rec[:])
                        nc.vector.tensor_tensor(out=h[:], in0=h[:],
                                                in1=bias_sb[:],
                                                op=mybir.AluOpType.add)
                        if relu:
                            nc.scalar.activation(
                                out=h[:], in_=h[:],
                                func=mybir.ActivationFunctionType.Relu)
                        out_writer(b, h)

            ident = const.tile([P, P], dt.float32)
            from concourse.masks import make_identity
            make_identity(nc, ident[:])

            l1_state = {}

            def l1_writer(b, h):
                pt = psumt.tile([H, P], dt.float32, tag="pt")
                nc.tensor.transpose(out=pt[:], in_=h[:], identity=ident[:])
                hT = prod.tile([H, P], dt.float32, tag="hT")
                nc.vector.tensor_copy(out=hT[:], in_=pt[:])
                pmm2 = psumt.tile([P, 2 + H], dt.float32, tag="pmm")
                nc.tensor.matmul(out=pmm2[:], lhsT=hT[:], rhs=W2a_sb[:],
                                 start=True, stop=True)
                k = b % TB
                if k == 0:
                    fsb4b = prod.tile([P, TB * TW], dt.bfloat16,
                                      tag="fsb4")
                    l1_state["fsb4"] = fsb4b
                build_row(pmm2, er_all2, b, l1_state["fsb4"], k)
                if k == TB - 1 or b == NB - 1:
                    write_rows(feat2_s, b - k, k + 1, l1_state["fsb4"])

            edge_phase(feat1_f, er_all, b1r_sb, True, l1_writer)
            allgather(feat2_s, feat2_f)

            def l2_writer(b, h):
                rows = LB if b == NB - 1 else P
                nc.sync.dma_start(out=out_ext[b * P:b * P + rows, :],
                                  in_=h[:rows, :])

            edge_phase(feat2_f, er_all2, b2r_sb, False, l2_writer)

    nc.compile()
    return nc


def _get_program(R_key, single=False):
    key = ("prog", R_key, single)
    if key not in _cache:
        _cache[key] = _build_program(R_key, single=single)
    return _cache[key]


def kernel(x, src, dst, W1, al1, ar1, b1, W2, al2, ar2, b2):
    from concourse.bass_utils import run_bass_kernel_spmd

    in_maps, R_key = _host_prep(x, src, dst, W1, al1, ar1, b1,
                                W2, al2, ar2, b2)
    nc = _get_program(R_key)
    res = run_bass_kernel_spmd(nc, in_maps, list(range(NCORES)))
    out = np.concatenate([res.results[c]["out"] for c in range(NCORES)],
                         axis=0)
    return out.astype(np.float32)


# revision 4
# speedup vs baseline: 1.0839x; 1.0030x over previous
"""GAT (2-layer, single-head) Trainium2 Bass kernel, 8-core SPMD. v2.

Design (vs v1 baseline):
  - dst nodes 1D-sharded (12500/core); edges routed to dst core, bucketed
    by (dst block of 128, src segment of 32768 table rows), packed into
    128-edge chunk columns, s-major within groups of G blocks so one
    dma_gather op (<=1024 idxs, ucode cap) spans buckets.
  - Table row = 128 bf16 (256B, the dma_gather minimum elem):
    [el as f32 (2 slots) | 1.0 | feat(32) | pad]. el kept f32 for softmax
    accuracy; feat bf16.
  - NO per-edge er gather (v1 spent ~50% of its descriptors on it).
    er[dst] is selected on-chip in two levels (dl = 8*hi + lo): per
    chunk a host-shipped fp8 hi-one-hot MHI [16,128] is matmul'd with
    the block's er table redistributed to [16,8] fp16, giving [128,8]
    candidates in PSUM; a host-shipped bf16 lo-one-hot mask [128,8] and
    an X-axis reduce pick the final er per slot. 16B+128B of mask bytes
    per chunk replace 256B of gathered bytes per EDGE.
  - aggregation per chunk: one-hot(+ex) built in ONE fused tensor_scalar
    (is_equal, mult) on bf16 iota (4x DVE mode, ~92ns) with ex and dl as
    per-partition scalars; matmul rhs reads the gathered table slice
    [1|feat] directly; denominator accumulates via the "1" column.
  - Epilogue: out = pacc[:,1:]/pacc[:,0] + bias (+relu), layer-2 table
    built inline; AllGather between layers (excluded from the metric, a
    local copy in the single-core cost program).
"""

import numpy as np
import ml_dtypes

N = 100000
E = 1600000
F = 128
H = 32
NCORES = 8
NPC = N // NCORES          # nodes per core
P = 128
NB = (NPC + P - 1) // P    # dst blocks per core (98; last block 84 rows)
LB = NPC - (NB - 1) * P    # rows in last block
NPCP = NB * P              # padded nodes per core (table rows per core)
TROWS = NCORES * NPCP      # full table rows
TW = 128                   # table row: 128 bf16 = 256B
ELC = 0                    # cols 0-1: el as f32
ONEC = 2                   # col 2: 1.0
FEATC = 3                  # cols 3..34: feat
WFW = 1 + H                # aggregation rhs width: [1 | feat]
SEG = 32768                # src segment size (int16 gather indices)
NSEG = (TROWS + SEG - 1) // SEG
G = 8                      # dst blocks per group tile
NG = (NB + G - 1) // G
CAPC = 8                   # max chunks (1024 indices) per dma_gather op

_cache = {}


def _plan(R):
    """Column layout. R: [NB, NSEG] chunks per bucket.

    Group g covers blocks [gB, gB+G). Columns are s-major within the
    group: for s in segs, for b in group, R[b,s] columns.
    Returns (CH, groups); groups[g] = dict with:
      g0: first global column of the group
      ops: list of (s, c0, nch) gather ops (global col, chunks <= CAPC)
      blocks: {b: [(s, gcol0, Rbs, bcol0)]} runs per block; bcol0 is the
        block-local column offset (block columns are the concat of its
        s-runs, in s order).
    """
    groups = []
    c = 0
    for g in range(NG):
        bs = list(range(g * G, min((g + 1) * G, NB)))
        g0 = c
        ops = []
        blocks = {b: [] for b in bs}
        bcol = {b: 0 for b in bs}
        for s in range(NSEG):
            s0 = c
            for b in bs:
                r = int(R[b, s])
                blocks[b].append((s, c, r, bcol[b]))
                bcol[b] += r
                c += r
            nch = c - s0
            off = 0
            while off < nch:
                take = min(CAPC, nch - off)
                ops.append((s, s0 + off, take))
                off += take
        groups.append({"g0": c - (c - g0), "ops": ops, "blocks": blocks,
                       "nch": c - g0})
        groups[-1]["g0"] = g0
    return c, groups


def _host_prep(x, src, dst, W1, al1, ar1, b1, W2, al2, ar2, b2):
    f32, bf16, i16 = np.float32, ml_dtypes.bfloat16, np.int16
    src = np.asarray(src).astype(np.int64)
    dst = np.asarray(dst).astype(np.int64)

    core = dst // NPC
    r = dst % NPC
    b = r // P
    dl = r % P
    trow_src = (src // NPC) * NPCP + (src % NPC)
    seg = trow_src // SEG
    bgid = core * NB + b

    key = bgid * NSEG + seg
    counts = np.bincount(key, minlength=NCORES * NB * NSEG)
    counts3 = counts.reshape(NCORES, NB, NSEG)
    R = -(-counts3.max(axis=0) // P)          # [NB, NSEG]
    R = np.maximum(R, 1)
    CH, groups = _plan(R)

    order = np.argsort(key, kind="stable")
    s_trow = trow_src[order]
    s_seg = seg[order]
    s_b = b[order]
    s_bgid = bgid[order]
    s_dl = dl[order]

    seg_start = np.concatenate([[0], np.cumsum(counts)])[:-1]
    rank = np.arange(len(order), dtype=np.int64) - seg_start[key[order]]

    c0_tab = np.zeros((NB, NSEG), np.int64)   # global col of bucket start
    for gr in groups:
        for bb, runs in gr["blocks"].items():
            for (s, gcol0, _r, _bc) in runs:
                c0_tab[bb, s] = gcol0

    col = c0_tab[s_b, s_seg] + rank // P      # global column
    p = rank % P

    # dstl: [128, CH] destination-local row per slot (200 = pad)
    dstl = np.full((NCORES, P, CH), 200.0, f32)
    flat = (s_bgid // NB) * (P * CH) + p * CH + col
    dstl.reshape(-1)[flat] = s_dl

    # gather indices: [16 wrap, CH*8], segment-relative
    fidx = np.zeros((NCORES, 16, CH * 8), i16)
    i_op = (col - c0_tab[s_b, s_seg]) * P + p
    # op-local index: ops start at 8-column boundaries within each s-run
    # of a group: recompute relative to the op start column.
    # ops cover [c0, c0+take) chunks; op base = s-run start + 8k.
    srun0 = np.zeros((NB, NSEG), np.int64)    # s-run start col in group
    for gr in groups:
        for (s, c0, take) in gr["ops"]:
            pass
    # op base for column col in s-run starting at sc0: sc0 + ((col-sc0)//8)*8
    sc0_tab = np.zeros((NB, NSEG), np.int64)
    for gr in groups:
        for s in range(NSEG):
            cols = [(c0, r) for bb2, runs in gr["blocks"].items()
                    for (ss, c0, r, _bc) in runs if ss == s]
            if cols:
                sc0 = min(c0 for c0, _ in cols)
                for bb2, runs in gr["blocks"].items():
                    for (ss, c02, r2, _bc) in runs:
                        if ss == s:
                            sc0_tab[bb2, s] = sc0
    opbase = sc0_tab[s_b, s_seg] + ((col - sc0_tab[s_b, s_seg]) // CAPC) * CAPC
    i_op = (col - opbase) * P + p
    row16 = i_op % 16
    col16 = i_op // 16
    abscol = opbase * 8 + col16
    fflat = (s_bgid // NB) * (16 * CH * 8) + row16 * (CH * 8) + abscol
    fidx.reshape(-1)[fflat] = (s_trow - s_seg * SEG).astype(i16)
    fidx = np.tile(fidx, (1, 8, 1))

    # two-level er-select masks: dl = 8*hi + lo
    #   MHI fp8 [16, CH*128]: MHI[k, c*128+i] = (dstl[i,c]//8 == k)
    #   LO8 bf16 [128, CH*8]: LO8[i, c*8+l] = (dstl[i,c]%8 == l) & valid
    one8 = np.float32(1.0).astype(ml_dtypes.float8_e4m3).view(np.uint8)
    MHI = np.zeros((NCORES, 16, CH * P), np.uint8)
    LO8 = np.zeros((NCORES, P, CH * 8), bf16)
    for cc in range(NCORES):
        d = dstl[cc].astype(np.int64)         # [P(slots), CH]
        hi = (d // 8).T                       # [CH, i]; pad 200//8=25
        eqh = hi[:, :, None] == np.arange(16)[None, None, :]
        MHI[cc][np.transpose(eqh, (2, 0, 1)).reshape(16, CH * P)] = one8
        lo = (d % 8).T
        valid = (d < P).T
        eql = (lo[:, :, None] == np.arange(8)[None, None, :]) \
            & valid[:, :, None]               # [CH, i, 8]
        LO8[cc][np.transpose(eql, (1, 0, 2)).reshape(P, CH * 8)] = bf16(1.0)
    MHI = MHI.view(ml_dtypes.float8_e4m3)

    def aug(W, al, ar):
        W = np.asarray(W, f32)
        Wa = np.zeros((W.shape[0], 2 + H), f32)
        Wa[:, 0] = W @ np.asarray(al, f32)
        Wa[:, 1:1 + H] = W
        Wa[:, 1 + H] = W @ np.asarray(ar, f32)
        return Wa

    W1a = aug(W1, al1, ar1)
    W2a = aug(W2, al2, ar2)
    b1r = np.tile(np.asarray(b1, f32)[None, :], (P, 1))
    b2r = np.tile(np.asarray(b2, f32)[None, :], (P, 1))
    iota = np.tile(np.arange(P, dtype=bf16)[None, :], (P, 1))

    x = np.asarray(x, f32)
    xsT = np.zeros((NCORES, F, NPCP), f32)
    for cc in range(NCORES):
        xsT[cc, :, :NPC] = x[cc * NPC:(cc + 1) * NPC].T

    in_maps = []
    for cc in range(NCORES):
        in_maps.append({
            "xsT": xsT[cc],
            "W1a": W1a, "W2a": W2a, "b1r": b1r, "b2r": b2r, "iota": iota,
            "fidx": fidx[cc], "dstl": dstl[cc],
            "mhi": MHI[cc], "lo8": LO8[cc],
        })
    return in_maps, tuple(int(v) for v in R.reshape(-1))


def _build_program(R_key, single=False):
    import concourse.bacc as bacc
    import concourse.mybir as mybir
    import concourse.tile as tile

    dt = mybir.dt
    R = np.asarray(R_key, np.int64).reshape(NB, NSEG)
    CH, groups = _plan(R)
    ncores = 1 if single else NCORES

    nc = bacc.Bacc("TRN2", target_bir_lowering=False, debug=False,
                   num_devices=ncores, num_swdge_queues=4)

    xsT = nc.dram_tensor("xsT", [F, NPCP], dt.float32, kind="ExternalInput")
    W1a = nc.dram_tensor("W1a", [F, 2 + H], dt.float32, kind="ExternalInput")
    W2a = nc.dram_tensor("W2a", [H, 2 + H], dt.float32, kind="ExternalInput")
    b1r = nc.dram_tensor("b1r", [P, H], dt.float32, kind="ExternalInput")
    b2r = nc.dram_tensor("b2r", [P, H], dt.float32, kind="ExternalInput")
    iota = nc.dram_tensor("iota", [P, P], dt.bfloat16, kind="ExternalInput")
    fidx = nc.dram_tensor("fidx", [P, CH * 8], dt.int16, kind="ExternalInput")
    dstl = nc.dram_tensor("dstl", [P, CH], dt.float32, kind="ExternalInput")
    mhi = nc.dram_tensor("mhi", [16, CH * P], dt.float8e4, kind="ExternalInput")
    lo8 = nc.dram_tensor("lo8", [P, CH * 8], dt.bfloat16, kind="ExternalInput")
    out_ext = nc.dram_tensor("out", [NPC, H], dt.float32, kind="ExternalOutput")

    qn_state = [0]

    def qn():
        qn_state[0] = (qn_state[0] + 1) % 4
        return qn_state[0]

    with tile.TileContext(nc) as tc:
        with (
            tc.tile_pool(name="const", bufs=1) as const,
            tc.tile_pool(name="prod", bufs=4) as prod,
            tc.tile_pool(name="tv", bufs=2) as tvpool,
            tc.tile_pool(name="mg", bufs=2) as mgpool,
            tc.tile_pool(name="fxg", bufs=2) as fxpool,
            tc.tile_pool(name="oh", bufs=12) as ohpool,
            tc.tile_pool(name="ee", bufs=4) as eepool,
            tc.tile_pool(name="epi", bufs=4) as epipool,
            tc.tile_pool(name="ps", bufs=3, space="PSUM") as psum,
            tc.tile_pool(name="pse", bufs=3, space="PSUM") as psume,
            tc.tile_pool(name="pst", bufs=1, space="PSUM") as psumt,
            tc.tile_pool(name="dram", bufs=1, space="DRAM") as dram,
        ):
            iota_sb = const.tile([P, P], dt.bfloat16)
            nc.sync.dma_start(out=iota_sb[:], in_=iota[:])
            W1a_sb = const.tile([F, 2 + H], dt.float32)
            nc.sync.dma_start(out=W1a_sb[:], in_=W1a[:])
            W2a_sb = const.tile([H, 2 + H], dt.float32)
            nc.sync.dma_start(out=W2a_sb[:], in_=W2a[:])
            b1r_sb = const.tile([P, H], dt.float32)
            nc.sync.dma_start(out=b1r_sb[:], in_=b1r[:])
            b2r_sb = const.tile([P, H], dt.float32)
            nc.sync.dma_start(out=b2r_sb[:], in_=b2r[:])
            dstl_sb = const.tile([P, CH], dt.float32)
            nc.sync.dma_start(out=dstl_sb[:], in_=dstl[:])
            er_all = const.tile([P, NB], dt.float16)
            er_all2 = const.tile([P, NB], dt.float16)
            er2d1 = const.tile([16, NB * 8], dt.float16)
            er2d2 = const.tile([16, NB * 8], dt.float16)

            feat1_s = dram.tile([NPCP, TW], dt.bfloat16)
            feat1_f = dram.tile([TROWS, TW], dt.bfloat16, addr_space="Shared")
            feat2_s = dram.tile([NPCP, TW], dt.bfloat16)
            feat2_f = dram.tile([TROWS, TW], dt.bfloat16, addr_space="Shared")

            def build_row(pmm, er_dst, er2d, b, fsb4, k):
                """pmm [P, 2+H] f32 = [el | feat | er] -> bf16 row + er."""
                o = k * TW
                nc.vector.tensor_copy(
                    out=fsb4[:, o + ELC:o + ELC + 2].bitcast(dt.float32),
                    in_=pmm[:, 0:1])
                nc.vector.tensor_copy(out=fsb4[:, o + FEATC:o + FEATC + H],
                                      in_=pmm[:, 1:1 + H])
                nc.vector.tensor_copy(out=er_dst[:, b:b + 1],
                                      in_=pmm[:, 1 + H:2 + H])

            TB = 8

            def emit_er2d(er2d, er_dst, b0, nb):
                for b in range(b0, b0 + nb):
                    nc.sync.dma_start(out=er2d[:, b * 8:(b + 1) * 8],
                                      in_=er_dst[:, b:b + 1])

            def write_rows(feat_s, b0, nb, fsb4):
                nc.sync.dma_start(
                    out=feat_s[b0 * P:(b0 + nb) * P, :]
                        .rearrange("(blk r) w -> r blk w", blk=nb),
                    in_=fsb4[:, :nb * TW].rearrange("p (blk w) -> p blk w",
                                                    w=TW))

            # ---- layer-1 table ----
            for b0 in range(0, NB, TB):
                nb = min(TB, NB - b0)
                xt = prod.tile([F, TB * P], dt.float32, tag="xt")
                nc.sync.dma_start(out=xt[:, :nb * P],
                                  in_=xsT[:, b0 * P:(b0 + nb) * P])
                fsb4 = prod.tile([P, TB * TW], dt.bfloat16, tag="fsb4")
                nc.vector.memset(
                    fsb4[:].rearrange("p (blk w) -> p blk w", w=TW)
                    [:, :, ONEC:ONEC + 1], 1.0)
                for k in range(nb):
                    pmm = psumt.tile([P, 2 + H], dt.float32, tag="pmm")
                    nc.tensor.matmul(out=pmm[:], lhsT=xt[:, k * P:(k + 1) * P],
                                     rhs=W1a_sb[:], start=True, stop=True)
                    build_row(pmm, er_all, er2d1, b0 + k, fsb4, k)
                emit_er2d(er2d1, er_all, b0, nb)
                write_rows(feat1_s, b0, nb, fsb4)

            def allgather(src_t, dst_t):
                if single:
                    nc.sync.dma_start(
                        out=dst_t[0:NPCP, :].rearrange("a b -> (a b)"),
                        in_=src_t[:].rearrange("a b -> (a b)"))
                else:
                    nc.gpsimd.collective_compute(
                        "AllGather", mybir.AluOpType.bypass,
                        replica_groups=[list(range(NCORES))],
                        ins=[src_t[:]], outs=[dst_t[:]],
                    )

            allgather(feat1_s, feat1_f)

            # ---- edge phase ----
            def edge_phase(feat_f, er2d, bias_sb, relu, out_writer):
                for g, gr in enumerate(groups):
                    g0, nch = gr["g0"], gr["nch"]
                    fxg = fxpool.tile([P, nch * 8], dt.int16, tag="fxg")
                    nc.sync.dma_start(out=fxg[:],
                                      in_=fidx[:, g0 * 8:(g0 + nch) * 8])
                    mhg = mgpool.tile([16, nch * P], dt.float8e4, tag="mhg")
                    nc.sync.dma_start(out=mhg[:],
                                      in_=mhi[:, g0 * P:(g0 + nch) * P])
                    log = mgpool.tile([P, nch * 8], dt.bfloat16, tag="log")
                    nc.sync.dma_start(out=log[:],
                                      in_=lo8[:, g0 * 8:(g0 + nch) * 8])
                    tvg = tvpool.tile([P, nch * TW], dt.bfloat16, tag="tvg")
                    tv3 = tvg[:].rearrange("p (r e) -> p r e", e=TW)
                    for (s, c0, take) in gr["ops"]:
                        seg_lo = s * SEG
                        seg_hi = min(seg_lo + SEG, TROWS)
                        nc.gpsimd.dma_gather(
                            out_ap=tv3[:, c0 - g0:c0 - g0 + take, :],
                            in_ap=feat_f[seg_lo:seg_hi, :],
                            idxs_ap=fxg[:, (c0 - g0) * 8:(c0 - g0 + take) * 8],
                            num_idxs=take * P, num_idxs_reg=take * P,
                            elem_size=TW, queue_num=qn(),
                        )
                    def emit_er(b):
                        # er select stage 1: hi via fp8 [16,128] matmuls ->
                        # [128,8] candidate rows per chunk (emitted one block
                        # ahead so PE's in-order queue can't stall DVE).
                        runs = gr["blocks"][b]
                        Rb = sum(rr for (_s, _c, rr, _bc) in runs)
                        er_ps = psume.tile([P, Rb * 8], dt.float32,
                                           tag="erps")
                        for (s, gcol0, Rbs, bcol0) in runs:
                            for rr in range(Rbs):
                                gc = gcol0 - g0 + rr
                                nc.tensor.matmul(
                                    out=er_ps[:, (bcol0 + rr) * 8:
                                              (bcol0 + rr + 1) * 8],
                                    lhsT=mhg[:, gc * P:(gc + 1) * P],
                                    rhs=er2d[:, b * 8:(b + 1) * 8],
                                    start=True, stop=True)
                        return er_ps

                    bl = sorted(gr["blocks"])
                    er_tiles = {bl[0]: emit_er(bl[0])}
                    for bi, b in enumerate(bl):
                        if bi + 1 < len(bl):
                            er_tiles[bl[bi + 1]] = emit_er(bl[bi + 1])
                        runs = gr["blocks"][b]       # [(s, gcol0, Rbs, bcol0)]
                        Rb = sum(rr for (_s, _c, rr, _bc) in runs)
                        er_ps = er_tiles.pop(b)
                        sel8 = eepool.tile([P, Rb * 8], dt.bfloat16,
                                           tag="sel8")
                        for (s, gcol0, Rbs, bcol0) in runs:
                            nc.vector.tensor_tensor(
                                out=sel8[:, bcol0 * 8:(bcol0 + Rbs) * 8],
                                in0=er_ps[:, bcol0 * 8:(bcol0 + Rbs) * 8],
                                in1=log[:, (gcol0 - g0) * 8:
                                        (gcol0 - g0 + Rbs) * 8],
                                op=mybir.AluOpType.mult)
                        ers = eepool.tile([P, Rb], dt.float32, tag="ers")
                        nc.vector.tensor_reduce(
                            out=ers[:],
                            in_=sel8[:].rearrange("p (r e) -> p r e", e=8),
                            axis=mybir.AxisListType.X,
                            op=mybir.AluOpType.add)
                        # ee = el + er ; lrelu ; ex
                        ee = eepool.tile([P, Rb], dt.float32, tag="ee")
                        for (s, gcol0, Rbs, bcol0) in runs:
                            el = tv3[:, gcol0 - g0:gcol0 - g0 + Rbs,
                                     ELC:ELC + 2].bitcast(dt.float32)
                            nc.vector.tensor_tensor(
                                out=ee[:, bcol0:bcol0 + Rbs]
                                    .rearrange("p (r o) -> p r o", o=1),
                                in0=el,
                                in1=ers[:, bcol0:bcol0 + Rbs]
                                    .rearrange("p (r o) -> p r o", o=1),
                                op=mybir.AluOpType.add)
                        nc.vector.scalar_tensor_tensor(
                            out=ee[:], in0=ee[:], scalar=0.2, in1=ee[:],
                            op0=mybir.AluOpType.mult,
                            op1=mybir.AluOpType.max)
                        ex = eepool.tile([P, Rb], dt.float32, tag="ex")
                        nc.scalar.activation(
                            out=ex[:], in_=ee[:],
                            func=mybir.ActivationFunctionType.Exp)
                        # fused one-hot(+ex) per chunk; rhs = table slice
                        pacc = psum.tile([P, WFW], dt.float32, tag="pacc")
                        done = 0
                        for (s, gcol0, Rbs, bcol0) in runs:
                            for rr in range(Rbs):
                                oh = ohpool.tile([P, P], dt.bfloat16,
                                                 tag="oh")
                                nc.vector.tensor_scalar(
                                    out=oh[:], in0=iota_sb[:],
                                    scalar1=dstl_sb[:, gcol0 + rr:
                                                    gcol0 + rr + 1],
                                    scalar2=ex[:, bcol0 + rr:bcol0 + rr + 1],
                                    op0=mybir.AluOpType.is_equal,
                                    op1=mybir.AluOpType.mult,
                                )
                                nc.tensor.matmul(
                                    out=pacc[:],
                                    lhsT=oh[:],
                                    rhs=tv3[:, gcol0 - g0 + rr,
                                            ONEC:ONEC + WFW],
                                    start=(done == 0), stop=(done == Rb - 1))
                                done += 1
                        # epilogue
                        den = epipool.tile([P, 1], dt.float32, tag="den")
                        nc.vector.tensor_scalar_add(out=den[:],
                                                    in0=pacc[:, 0:1],
                                                    scalar1=1e-30)
                        rec = epipool.tile([P, 1], dt.float32, tag="rec")
                        nc.vector.reciprocal(out=rec[:], in_=den[:])
                        h = epipool.tile([P, H], dt.float32, tag="h")
                        nc.vector.tensor_scalar_mul(out=h[:], in0=pacc[:, 1:],
                                                    scalar1=rec[:])
                        nc.vector.tensor_tensor(out=h[:], in0=h[:],
                                                in1=bias_sb[:],
                                                op=mybir.AluOpType.add)
                        if relu:
                            nc.scalar.activation(
                                out=h[:], in_=h[:],
                                func=mybir.ActivationFunctionType.Relu)
                        out_writer(b, h)

            ident = const.tile([P, P], dt.float32)
            from concourse.masks import make_identity
            make_identity(nc, ident[:])

            l1_state = {}

            def l1_writer(b, h):
                pt = psumt.tile([H, P], dt.float32, tag="pt")
                nc.tensor.transpose(out=pt[:], in_=h[:], identity=ident[:])
                hT = prod.tile([H, P], dt.float32, tag="hT")
                nc.vector.tensor_copy(out=hT[:], in_=pt[:])
                pmm2 = psumt.tile([P, 2 + H], dt.float32, tag="pmm")
                nc.tensor.matmul(out=pmm2[:], lhsT=hT[:], rhs=W2a_sb[:],
                                 start=True, stop=True)
                k = b % TB
                if k == 0:
                    fsb4b = prod.tile([P, TB * TW], dt.bfloat16,
                                      tag="fsb4")
                    nc.vector.memset(
                        fsb4b[:].rearrange("p (blk w) -> p blk w", w=TW)
                        [:, :, ONEC:ONEC + 1], 1.0)
                    l1_state["fsb4"] = fsb4b
                build_row(pmm2, er_all2, er2d2, b, l1_state["fsb4"], k)
                if k == TB - 1 or b == NB - 1:
                    emit_er2d(er2d2, er_all2, b - k, k + 1)
                    write_rows(feat2_s, b - k, k + 1, l1_state["fsb4"])

            edge_phase(feat1_f, er2d1, b1r_sb, True, l1_writer)
            allgather(feat2_s, feat2_f)

            def l2_writer(b, h):
                rows = LB if b == NB - 1 else P
                nc.sync.dma_start(out=out_ext[b * P:b * P + rows, :],
                                  in_=h[:rows, :])

            edge_phase(feat2_f, er2d2, b2r_sb, False, l2_writer)

    nc.compile()
    return nc


def _get_program(R_key, single=False):
    key = ("prog", R_key, single)
    if key not in _cache:
        _cache[key] = _build_program(R_key, single=single)
    return _cache[key]


def kernel(x, src, dst, W1, al1, ar1, b1, W2, al2, ar2, b2):
    from concourse.bass_utils import run_bass_kernel_spmd

    in_maps, R_key = _host_prep(x, src, dst, W1, al1, ar1, b1,
                                W2, al2, ar2, b2)
    nc = _get_program(R_key)
    res = run_bass_kernel_spmd(nc, in_maps, list(range(NCORES)))
    out = np.concatenate([res.results[c]["out"] for c in range(NCORES)],
                         axis=0)
    return out.astype(np.float32)


# revision 10
# speedup vs baseline: 1.1642x; 1.0741x over previous
"""GAT (2-layer, single-head) Trainium2 Bass kernel, 8-core SPMD. v2.

Design (vs v1 baseline):
  - dst nodes 1D-sharded (12500/core); edges routed to dst core, bucketed
    by (dst block of 128, src segment of 32768 table rows), packed into
    128-edge chunk columns, s-major within groups of G blocks so one
    dma_gather op (<=1024 idxs, ucode cap) spans buckets.
  - Table row = 128 bf16 (256B, the dma_gather minimum elem):
    [el as f32 (2 slots) | 1.0 | feat(32) | pad]. el kept f32 for softmax
    accuracy; feat bf16.
  - NO per-edge er gather (v1 spent ~50% of its descriptors on it).
    er[dst] is selected on-chip in two levels (dl = 8*hi + lo): per
    chunk a host-shipped fp8 hi-one-hot MHI [16,128] is matmul'd with
    the block's er table redistributed to [16,8] fp16, giving [128,8]
    candidates in PSUM; a host-shipped bf16 lo-one-hot mask [128,8] and
    an X-axis reduce pick the final er per slot. 16B+128B of mask bytes
    per chunk replace 256B of gathered bytes per EDGE.
  - aggregation per chunk: one-hot(+ex) built in ONE fused tensor_scalar
    (is_equal, mult) on bf16 iota (4x DVE mode, ~92ns) with ex and dl as
    per-partition scalars; matmul rhs reads the gathered table slice
    [1|feat] directly; denominator accumulates via the "1" column.
  - Epilogue: out = pacc[:,1:]/pacc[:,0] + bias (+relu), layer-2 table
    built inline; AllGather between layers (excluded from the metric, a
    local copy in the single-core cost program).
"""

import numpy as np
import ml_dtypes

N = 100000
E = 1600000
F = 128
H = 32
NCORES = 8
NPC = N // NCORES          # nodes per core
P = 128
NB = (NPC + P - 1) // P    # dst blocks per core (98; last block 84 rows)
LB = NPC - (NB - 1) * P    # rows in last block
NPCP = NB * P              # padded nodes per core (table rows per core)
TROWS = NCORES * NPCP      # full table rows
TW = 128                   # table row: 128 bf16 = 256B
ELC = 0                    # cols 0-1: el as f32
ONEC = 2                   # col 2: 1.0
FEATC = 3                  # cols 3..34: feat
WFW = 1 + H                # aggregation rhs width: [1 | feat]
SEG = 32768                # src segment size (int16 gather indices)
NSEG = (TROWS + SEG - 1) // SEG
G = 8                      # dst blocks per group tile
NG = (NB + G - 1) // G
CAPC = 8                   # max chunks (1024 indices) per dma_gather op

_cache = {}


def _plan(R):
    """Column layout. R: [NB, NSEG] chunks per bucket.

    Group g covers blocks [gB, gB+G). Columns are s-major within the
    group: for s in segs, for b in group, R[b,s] columns.
    Returns (CH, groups); groups[g] = dict with:
      g0: first global column of the group
      ops: list of (s, c0, nch) gather ops (global col, chunks <= CAPC)
      blocks: {b: [(s, gcol0, Rbs, bcol0)]} runs per block; bcol0 is the
        block-local column offset (block columns are the concat of its
        s-runs, in s order).
    """
    groups = []
    c = 0
    for g in range(NG):
        bs = list(range(g * G, min((g + 1) * G, NB)))
        g0 = c
        ops = []
        blocks = {b: [] for b in bs}
        bcol = {b: 0 for b in bs}
        for s in range(NSEG):
            s0 = c
            for b in bs:
                r = int(R[b, s])
                blocks[b].append((s, c, r, bcol[b]))
                bcol[b] += r
                c += r
            nch = c - s0
            off = 0
            while off < nch:
                take = min(CAPC, nch - off)
                ops.append((s, s0 + off, take))
                off += take
        groups.append({"g0": c - (c - g0), "ops": ops, "blocks": blocks,
                       "nch": c - g0})
        groups[-1]["g0"] = g0
    return c, groups


def _host_prep(x, src, dst, W1, al1, ar1, b1, W2, al2, ar2, b2):
    f32, bf16, i16 = np.float32, ml_dtypes.bfloat16, np.int16
    src = np.asarray(src).astype(np.int64)
    dst = np.asarray(dst).astype(np.int64)

    core = dst // NPC
    r = dst % NPC
    b = r // P
    dl = r % P
    trow_src = (src // NPC) * NPCP + (src % NPC)
    seg = trow_src // SEG
    bgid = core * NB + b

    key = bgid * NSEG + seg
    counts = np.bincount(key, minlength=NCORES * NB * NSEG)
    counts3 = counts.reshape(NCORES, NB, NSEG)
    R = -(-counts3.max(axis=0) // P)          # [NB, NSEG]
    R = np.maximum(R, 1)
    CH, groups = _plan(R)

    order = np.argsort(key, kind="stable")
    s_trow = trow_src[order]
    s_seg = seg[order]
    s_b = b[order]
    s_bgid = bgid[order]
    s_dl = dl[order]

    seg_start = np.concatenate([[0], np.cumsum(counts)])[:-1]
    rank = np.arange(len(order), dtype=np.int64) - seg_start[key[order]]

    c0_tab = np.zeros((NB, NSEG), np.int64)   # global col of bucket start
    for gr in groups:
        for bb, runs in gr["blocks"].items():
            for (s, gcol0, _r, _bc) in runs:
                c0_tab[bb, s] = gcol0

    col = c0_tab[s_b, s_seg] + rank // P      # global column
    p = rank % P

    # dstl: [128, CH] destination-local row per slot (200 = pad)
    dstl = np.full((NCORES, P, CH), 200.0, f32)
    flat = (s_bgid // NB) * (P * CH) + p * CH + col
    dstl.reshape(-1)[flat] = s_dl

    # gather indices: [16 wrap, CH*8], segment-relative
    fidx = np.zeros((NCORES, 16, CH * 8), i16)
    i_op = (col - c0_tab[s_b, s_seg]) * P + p
    # op-local index: ops start at 8-column boundaries within each s-run
    # of a group: recompute relative to the op start column.
    # ops cover [c0, c0+take) chunks; op base = s-run start + 8k.
    srun0 = np.zeros((NB, NSEG), np.int64)    # s-run start col in group
    for gr in groups:
        for (s, c0, take) in gr["ops"]:
            pass
    # op base for column col in s-run starting at sc0: sc0 + ((col-sc0)//8)*8
    sc0_tab = np.zeros((NB, NSEG), np.int64)
    for gr in groups:
        for s in range(NSEG):
            cols = [(c0, r) for bb2, runs in gr["blocks"].items()
                    for (ss, c0, r, _bc) in runs if ss == s]
            if cols:
                sc0 = min(c0 for c0, _ in cols)
                for bb2, runs in gr["blocks"].items():
                    for (ss, c02, r2, _bc) in runs:
                        if ss == s:
                            sc0_tab[bb2, s] = sc0
    opbase = sc0_tab[s_b, s_seg] + ((col - sc0_tab[s_b, s_seg]) // CAPC) * CAPC
    i_op = (col - opbase) * P + p
    row16 = i_op % 16
    col16 = i_op // 16
    abscol = opbase * 8 + col16
    fflat = (s_bgid // NB) * (16 * CH * 8) + row16 * (CH * 8) + abscol
    fidx.reshape(-1)[fflat] = (s_trow - s_seg * SEG).astype(i16)
    fidx = np.tile(fidx, (1, 8, 1))

    # two-level er-select masks: dl = 8*hi + lo
    #   MHI fp8 [16, CH*128]: MHI[k, c*128+i] = (dstl[i,c]//8 == k)
    #   LO8 bf16 [128, CH*8]: LO8[i, c*8+l] = (dstl[i,c]%8 == l) & valid
    one8 = np.float32(1.0).astype(ml_dtypes.float8_e4m3).view(np.uint8)
    MHI = np.zeros((NCORES, 16, CH * P), np.uint8)
    LO8 = np.zeros((NCORES, P, CH * 8), bf16)
    for cc in range(NCORES):
        d = dstl[cc].astype(np.int64)         # [P(slots), CH]
        hi = (d // 8).T                       # [CH, i]; pad 200//8=25
        eqh = hi[:, :, None] == np.arange(16)[None, None, :]
        MHI[cc][np.transpose(eqh, (2, 0, 1)).reshape(16, CH * P)] = one8
        lo = (d % 8).T
        valid = (d < P).T
        eql = (lo[:, :, None] == np.arange(8)[None, None, :]) \
            & valid[:, :, None]               # [CH, i, 8]
        LO8[cc][np.transpose(eql, (1, 0, 2)).reshape(P, CH * 8)] = bf16(1.0)
    MHI = MHI.view(ml_dtypes.float8_e4m3)

    def aug(W, al, ar):
        W = np.asarray(W, f32)
        Wa = np.zeros((W.shape[0], 2 + H), f32)
        Wa[:, 0] = W @ np.asarray(al, f32)
        Wa[:, 1:1 + H] = W
        Wa[:, 1 + H] = W @ np.asarray(ar, f32)
        return Wa

    W1a = aug(W1, al1, ar1)
    W2a = aug(W2, al2, ar2)
    b1r = np.tile(np.asarray(b1, f32)[None, :], (P, 1))
    b2r = np.tile(np.asarray(b2, f32)[None, :], (P, 1))
    iota = np.tile(np.arange(P, dtype=bf16)[None, :], (P, 1))

    x = np.asarray(x, f32)
    xsT = np.zeros((NCORES, F, NPCP), f32)
    for cc in range(NCORES):
        xsT[cc, :, :NPC] = x[cc * NPC:(cc + 1) * NPC].T

    in_maps = []
    for cc in range(NCORES):
        in_maps.append({
            "xsT": xsT[cc],
            "W1a": W1a, "W2a": W2a, "b1r": b1r, "b2r": b2r, "iota": iota,
            "fidx": fidx[cc], "dstl": dstl[cc],
            "mhi": MHI[cc], "lo8": LO8[cc],
        })
    return in_maps, tuple(int(v) for v in R.reshape(-1))


def _build_program(R_key, single=False):
    import concourse.bacc as bacc
    import concourse.mybir as mybir
    import concourse.tile as tile

    dt = mybir.dt
    R = np.asarray(R_key, np.int64).reshape(NB, NSEG)
    CH, groups = _plan(R)
    ncores = 1 if single else NCORES

    nc = bacc.Bacc("TRN2", target_bir_lowering=False, debug=False,
                   num_devices=ncores, num_swdge_queues=4)

    xsT = nc.dram_tensor("xsT", [F, NPCP], dt.float32, kind="ExternalInput")
    W1a = nc.dram_tensor("W1a", [F, 2 + H], dt.float32, kind="ExternalInput")
    W2a = nc.dram_tensor("W2a", [H, 2 + H], dt.float32, kind="ExternalInput")
    b1r = nc.dram_tensor("b1r", [P, H], dt.float32, kind="ExternalInput")
    b2r = nc.dram_tensor("b2r", [P, H], dt.float32, kind="ExternalInput")
    iota = nc.dram_tensor("iota", [P, P], dt.bfloat16, kind="ExternalInput")
    fidx = nc.dram_tensor("fidx", [P, CH * 8], dt.int16, kind="ExternalInput")
    dstl = nc.dram_tensor("dstl", [P, CH], dt.float32, kind="ExternalInput")
    mhi = nc.dram_tensor("mhi", [16, CH * P], dt.float8e4, kind="ExternalInput")
    lo8 = nc.dram_tensor("lo8", [P, CH * 8], dt.bfloat16, kind="ExternalInput")
    out_ext = nc.dram_tensor("out", [NPC, H], dt.float32, kind="ExternalOutput")

    qn_state = [0]

    def qn():
        qn_state[0] = (qn_state[0] + 1) % 4
        return qn_state[0]

    with tile.TileContext(nc) as tc:
        with (
            tc.tile_pool(name="const", bufs=1) as const,
            tc.tile_pool(name="prod", bufs=4) as prod,
            tc.tile_pool(name="tv", bufs=2) as tvpool,
            tc.tile_pool(name="mg", bufs=2) as mgpool,
            tc.tile_pool(name="fxg", bufs=2) as fxpool,
            tc.tile_pool(name="oh", bufs=12) as ohpool,
            tc.tile_pool(name="ee", bufs=4) as eepool,
            tc.tile_pool(name="epi", bufs=4) as epipool,
            tc.tile_pool(name="ps", bufs=3, space="PSUM") as psum,
            tc.tile_pool(name="pse", bufs=2, space="PSUM") as psume,
            tc.tile_pool(name="pst", bufs=2, space="PSUM") as psumt,
            tc.tile_pool(name="ptt", bufs=1, space="PSUM") as psumtt,
            tc.tile_pool(name="dram", bufs=1, space="DRAM") as dram,
        ):
            iota_sb = const.tile([P, P], dt.bfloat16)
            nc.sync.dma_start(out=iota_sb[:], in_=iota[:])
            W1a_sb = const.tile([F, 2 + H], dt.float32)
            nc.sync.dma_start(out=W1a_sb[:], in_=W1a[:])
            W2a_sb = const.tile([H, 2 + H], dt.float32)
            nc.sync.dma_start(out=W2a_sb[:], in_=W2a[:])
            b1r_sb = const.tile([P, H], dt.float32)
            nc.sync.dma_start(out=b1r_sb[:], in_=b1r[:])
            b2r_sb = const.tile([P, H], dt.float32)
            nc.sync.dma_start(out=b2r_sb[:], in_=b2r[:])
            dstl_sb = const.tile([P, CH], dt.float32)
            nc.sync.dma_start(out=dstl_sb[:], in_=dstl[:])
            er_all = const.tile([P, NB], dt.float16)
            er_all2 = const.tile([P, NB], dt.float16)
            er2d1 = const.tile([16, NB * 8], dt.float16)
            er2d2 = const.tile([16, NB * 8], dt.float16)

            feat1_s = dram.tile([NPCP, TW], dt.bfloat16)
            feat1_f = dram.tile([TROWS, TW], dt.bfloat16, addr_space="Shared")
            feat2_s = dram.tile([NPCP, TW], dt.bfloat16)
            feat2_f = dram.tile([TROWS, TW], dt.bfloat16, addr_space="Shared")

            def build_row(pmm, er_dst, er2d, b, fsb4, k):
                """pmm [P, 2+H] f32 = [el | feat | er] -> bf16 row + er."""
                o = k * TW
                nc.vector.tensor_copy(
                    out=fsb4[:, o + ELC:o + ELC + 2].bitcast(dt.float32),
                    in_=pmm[:, 0:1])
                nc.vector.tensor_copy(out=fsb4[:, o + FEATC:o + FEATC + H],
                                      in_=pmm[:, 1:1 + H])
                nc.vector.tensor_copy(out=er_dst[:, b:b + 1],
                                      in_=pmm[:, 1 + H:2 + H])

            TB = 8

            def emit_er2d(er2d, er_dst):
                # l-major: er2d[k, l*NB + b] = er[8k+l of block b]
                nc.scalar.dma_start(
                    out=er2d[:].rearrange("k (l b) -> k l b", b=NB),
                    in_=er_dst[:])

            def write_rows(feat_s, b0, nb, fsb4):
                nc.sync.dma_start(
                    out=feat_s[b0 * P:(b0 + nb) * P, :]
                        .rearrange("(blk r) w -> r blk w", blk=nb),
                    in_=fsb4[:, :nb * TW].rearrange("p (blk w) -> p blk w",
                                                    w=TW))

            # ---- layer-1 table ----
            for b0 in range(0, NB, TB):
                nb = min(TB, NB - b0)
                xt = prod.tile([F, TB * P], dt.float32, tag="xt")
                nc.sync.dma_start(out=xt[:, :nb * P],
                                  in_=xsT[:, b0 * P:(b0 + nb) * P])
                fsb4 = prod.tile([P, TB * TW], dt.bfloat16, tag="fsb4")
                nc.vector.memset(
                    fsb4[:].rearrange("p (blk w) -> p blk w", w=TW)
                    [:, :, ONEC:ONEC + 1], 1.0)
                for k in range(nb):
                    pmm = psumt.tile([P, 2 + H], dt.float32, tag="pmm")
                    nc.tensor.matmul(out=pmm[:], lhsT=xt[:, k * P:(k + 1) * P],
                                     rhs=W1a_sb[:], start=True, stop=True)
                    build_row(pmm, er_all, er2d1, b0 + k, fsb4, k)
                write_rows(feat1_s, b0, nb, fsb4)

            def allgather(src_t, dst_t):
                if single:
                    nc.sync.dma_start(
                        out=dst_t[0:NPCP, :].rearrange("a b -> (a b)"),
                        in_=src_t[:].rearrange("a b -> (a b)"))
                else:
                    nc.gpsimd.collective_compute(
                        "AllGather", mybir.AluOpType.bypass,
                        replica_groups=[list(range(NCORES))],
                        ins=[src_t[:]], outs=[dst_t[:]],
                    )

            allgather(feat1_s, feat1_f)
            emit_er2d(er2d1, er_all)

            # ---- edge phase ----
            def edge_phase(feat_f, er2d, bias_sb, relu, out_writer):
                for g, gr in enumerate(groups):
                    g0, nch = gr["g0"], gr["nch"]
                    fxg = fxpool.tile([P, nch * 8], dt.int16, tag="fxg")
                    nc.sync.dma_start(out=fxg[:],
                                      in_=fidx[:, g0 * 8:(g0 + nch) * 8])
                    mhg = mgpool.tile([16, nch * P], dt.float8e4, tag="mhg")
                    nc.sync.dma_start(out=mhg[:],
                                      in_=mhi[:, g0 * P:(g0 + nch) * P])
                    log = mgpool.tile([P, nch * 8], dt.bfloat16, tag="log")
                    nc.sync.dma_start(out=log[:],
                                      in_=lo8[:, g0 * 8:(g0 + nch) * 8])
                    tvg = tvpool.tile([P, nch * TW], dt.bfloat16, tag="tvg")
                    tv3 = tvg[:].rearrange("p (r e) -> p r e", e=TW)
                    for (s, c0, take) in gr["ops"]:
                        seg_lo = s * SEG
                        seg_hi = min(seg_lo + SEG, TROWS)
                        nc.gpsimd.dma_gather(
                            out_ap=tv3[:, c0 - g0:c0 - g0 + take, :],
                            in_ap=feat_f[seg_lo:seg_hi, :],
                            idxs_ap=fxg[:, (c0 - g0) * 8:(c0 - g0 + take) * 8],
                            num_idxs=take * P, num_idxs_reg=take * P,
                            elem_size=TW, queue_num=qn(),
                        )
                    def emit_er(b):
                        # er select stage 1: hi via fp8 [16,128] matmuls ->
                        # [128,8] candidate rows per chunk (emitted one block
                        # ahead so PE's in-order queue can't stall DVE).
                        runs = gr["blocks"][b]
                        Rb = sum(rr for (_s, _c, rr, _bc) in runs)
                        er_ps = psume.tile([P, Rb * 8], dt.float32,
                                           tag="erps")
                        for (s, gcol0, Rbs, bcol0) in runs:
                            for rr in range(Rbs):
                                gc = gcol0 - g0 + rr
                                nc.tensor.matmul(
                                    out=er_ps[:, (bcol0 + rr) * 8:
                                              (bcol0 + rr + 1) * 8],
                                    lhsT=mhg[:, gc * P:(gc + 1) * P],
                                    rhs=er2d[:].rearrange(
                                        "k (l b2) -> k l b2", b2=NB)
                                        [:, :, b],
                                    start=True, stop=True)
                        return er_ps

                    bl = sorted(gr["blocks"])
                    er_tiles = {bl[0]: emit_er(bl[0])}
                    for bi, b in enumerate(bl):
                        if bi + 1 < len(bl):
                            er_tiles[bl[bi + 1]] = emit_er(bl[bi + 1])
                        runs = gr["blocks"][b]       # [(s, gcol0, Rbs, bcol0)]
                        Rb = sum(rr for (_s, _c, rr, _bc) in runs)
                        er_ps = er_tiles.pop(b)
                        sel8 = eepool.tile([P, Rb * 8], dt.bfloat16,
                                           tag="sel8")
                        for (s, gcol0, Rbs, bcol0) in runs:
                            nc.vector.tensor_tensor(
                                out=sel8[:, bcol0 * 8:(bcol0 + Rbs) * 8],
                                in0=er_ps[:, bcol0 * 8:(bcol0 + Rbs) * 8],
                                in1=log[:, (gcol0 - g0) * 8:
                                        (gcol0 - g0 + Rbs) * 8],
                                op=mybir.AluOpType.mult)
                        ers = eepool.tile([P, Rb], dt.float32, tag="ers")
                        nc.vector.tensor_reduce(
                            out=ers[:],
                            in_=sel8[:].rearrange("p (r e) -> p r e", e=8),
                            axis=mybir.AxisListType.X,
                            op=mybir.AluOpType.add)
                        # ee = el + er ; lrelu ; ex
                        ee = eepool.tile([P, Rb], dt.float32, tag="ee")
                        for (s, gcol0, Rbs, bcol0) in runs:
                            el = tv3[:, gcol0 - g0:gcol0 - g0 + Rbs,
                                     ELC:ELC + 2].bitcast(dt.float32)
                            nc.vector.tensor_tensor(
                                out=ee[:, bcol0:bcol0 + Rbs]
                                    .rearrange("p (r o) -> p r o", o=1),
                                in0=el,
                                in1=ers[:, bcol0:bcol0 + Rbs]
                                    .rearrange("p (r o) -> p r o", o=1),
                                op=mybir.AluOpType.add)
                        nc.vector.scalar_tensor_tensor(
                            out=ee[:], in0=ee[:], scalar=0.2, in1=ee[:],
                            op0=mybir.AluOpType.mult,
                            op1=mybir.AluOpType.max)
                        ex = eepool.tile([P, Rb], dt.float32, tag="ex")
                        nc.scalar.activation(
                            out=ex[:], in_=ee[:],
                            func=mybir.ActivationFunctionType.Exp)
                        # fused one-hot(+ex) per chunk; rhs = table slice
                        pacc = psum.tile([P, WFW], dt.float32, tag="pacc")
                        done = 0
                        for (s, gcol0, Rbs, bcol0) in runs:
                            for rr in range(Rbs):
                                oh = ohpool.tile([P, P], dt.bfloat16,
                                                 tag="oh")
                                nc.vector.tensor_scalar(
                                    out=oh[:], in0=iota_sb[:],
                                    scalar1=dstl_sb[:, gcol0 + rr:
                                                    gcol0 + rr + 1],
                                    scalar2=ex[:, bcol0 + rr:bcol0 + rr + 1],
                                    op0=mybir.AluOpType.is_equal,
                                    op1=mybir.AluOpType.mult,
                                )
                                nc.tensor.matmul(
                                    out=pacc[:],
                                    lhsT=oh[:],
                                    rhs=tv3[:, gcol0 - g0 + rr,
                                            ONEC:ONEC + WFW],
                                    start=(done == 0), stop=(done == Rb - 1))
                                done += 1
                        # epilogue
                        den = epipool.tile([P, 1], dt.float32, tag="den")
                        nc.vector.tensor_scalar_add(out=den[:],
                                                    in0=pacc[:, 0:1],
                                                    scalar1=1e-30)
                        rec = epipool.tile([P, 1], dt.float32, tag="rec")
                        nc.vector.reciprocal(out=rec[:], in_=den[:])
                        h = epipool.tile([P, H], dt.float32, tag="h")
                        nc.vector.scalar_tensor_tensor(
                            out=h[:], in0=pacc[:, 1:], scalar=rec[:],
                            in1=bias_sb[:],
                            op0=mybir.AluOpType.mult,
                            op1=mybir.AluOpType.add)
                        if relu:
                            nc.scalar.activation(
                                out=h[:], in_=h[:],
                                func=mybir.ActivationFunctionType.Relu)
                        out_writer(b, h)

            ident = const.tile([P, P], dt.float32)
            from concourse.masks import make_identity
            make_identity(nc, ident[:])

            l1_state = {}

            def l1_writer(b, h):
                pt = psumtt.tile([H, P], dt.float32, tag="pt")
                nc.tensor.transpose(out=pt[:], in_=h[:], identity=ident[:])
                hT = prod.tile([H, P], dt.float32, tag="hT")
                nc.vector.tensor_copy(out=hT[:], in_=pt[:])
                pmm2 = psumt.tile([P, 2 + H], dt.float32, tag="pmm")
                nc.tensor.matmul(out=pmm2[:], lhsT=hT[:], rhs=W2a_sb[:],
                                 start=True, stop=True)
                k = b % TB
                if k == 0:
                    fsb4b = prod.tile([P, TB * TW], dt.bfloat16,
                                      tag="fsb4")
                    nc.vector.memset(
                        fsb4b[:].rearrange("p (blk w) -> p blk w", w=TW)
                        [:, :, ONEC:ONEC + 1], 1.0)
                    l1_state["fsb4"] = fsb4b
                build_row(pmm2, er_all2, er2d2, b, l1_state["fsb4"], k)
                if k == TB - 1 or b == NB - 1:
                    write_rows(feat2_s, b - k, k + 1, l1_state["fsb4"])
                if b == NB - 1:
                    emit_er2d(er2d2, er_all2)

            edge_phase(feat1_f, er2d1, b1r_sb, True, l1_writer)
            allgather(feat2_s, feat2_f)

            def l2_writer(b, h):
                rows = LB if b == NB - 1 else P
                nc.sync.dma_start(out=out_ext[b * P:b * P + rows, :],
                                  in_=h[:rows, :])

            edge_phase(feat2_f, er2d2, b2r_sb, False, l2_writer)

    nc.compile()
    return nc


def _get_program(R_key, single=False):
    key = ("prog", R_key, single)
    if key not in _cache:
        _cache[key] = _build_program(R_key, single=single)
    return _cache[key]


def kernel(x, src, dst, W1, al1, ar1, b1, W2, al2, ar2, b2):
    from concourse.bass_utils import run_bass_kernel_spmd

    in_maps, R_key = _host_prep(x, src, dst, W1, al1, ar1, b1,
                                W2, al2, ar2, b2)
    nc = _get_program(R_key)
    res = run_bass_kernel_spmd(nc, in_maps, list(range(NCORES)))
    out = np.concatenate([res.results[c]["out"] for c in range(NCORES)],
                         axis=0)
    return out.astype(np.float32)


# revision 15
# speedup vs baseline: 1.1817x; 1.0151x over previous
"""GAT (2-layer, single-head) Trainium2 Bass kernel, 8-core SPMD. v2.

Design (vs v1 baseline):
  - dst nodes 1D-sharded (12500/core); edges routed to dst core, bucketed
    by (dst block of 128, src segment of 32768 table rows). Within each
    (group of G blocks, segment) run, buckets pack CONTIGUOUSLY into
    128-slot columns (no per-bucket rounding); a column shared by two
    blocks is processed once per block through its own VIEW column of the
    dstl/MHI/LO8 masks (foreign slots masked out). One dma_gather op
    (<=1024 idxs, ucode cap) spans buckets within a run.
  - Table row = 128 bf16 (256B, the dma_gather minimum elem):
    [el as f32 (2 slots) | 1.0 | feat(32) | pad]. el kept f32 for softmax
    accuracy; feat bf16.
  - NO per-edge er gather (v1 spent ~50% of its descriptors on it).
    er[dst] is selected on-chip in two levels (dl = 8*hi + lo): per
    chunk a host-shipped fp8 hi-one-hot MHI [16,128] is matmul'd with
    the block's er table redistributed to [16,8] fp16, giving [128,8]
    candidates in PSUM; a host-shipped bf16 lo-one-hot mask [128,8] and
    an X-axis reduce pick the final er per slot. 16B+128B of mask bytes
    per chunk replace 256B of gathered bytes per EDGE.
  - aggregation per chunk: one-hot(+ex) built in ONE fused tensor_scalar
    (is_equal, mult) on bf16 iota (4x DVE mode, ~92ns) with ex and dl as
    per-partition scalars; matmul rhs reads the gathered table slice
    [1|feat] directly; denominator accumulates via the "1" column.
  - Epilogue: out = pacc[:,1:]/pacc[:,0] + bias (+relu), layer-2 table
    built inline; AllGather between layers (excluded from the metric, a
    local copy in the single-core cost program).
"""

import numpy as np
import ml_dtypes

N = 100000
E = 1600000
F = 128
H = 32
NCORES = 8
NPC = N // NCORES          # nodes per core
P = 128
NB = (NPC + P - 1) // P    # dst blocks per core (98; last block 84 rows)
LB = NPC - (NB - 1) * P    # rows in last block
NPCP = NB * P              # padded nodes per core (table rows per core)
TROWS = NCORES * NPCP      # full table rows
TW = 128                   # table row: 128 bf16 = 256B
ELC = 0                    # cols 0-1: el as f32
ONEC = 2                   # col 2: 1.0
FEATC = 3                  # cols 3..34: feat
WFW = 1 + H                # aggregation rhs width: [1 | feat]
SEG = 32768                # src segment size (int16 gather indices)
NSEG = (TROWS + SEG - 1) // SEG
G = 8                      # dst blocks per group tile
NG = (NB + G - 1) // G
CAPC = 8                   # max chunks (1024 indices) per dma_gather op

_cache = {}


def _plan(S):
    """Column layout. S: [NB, NSEG] slot counts per bucket (max over cores,
    NOT rounded to 128). Within each (group, seg) run, buckets pack
    contiguously; physical columns = ceil(run_slots/128); a column shared
    by two blocks is processed once per block through its own VIEW column
    of the dstl/MHI/LO8 masks (foreign slots masked to 200/zero).

    groups[g]: gp0/nchp (physical cols), gv0/nchv (view cols),
      ops: (s, pcol0, take<=8) gather ops,
      runbase: {s: run base pcol},
      blocks: {b: [(s, pcol0, ncols, vcol0, j0)]}, nv: {b: Vb}.
    """
    groups = []
    cp = 0
    cv = 0
    for g in range(NG):
        bs = list(range(g * G, min((g + 1) * G, NB)))
        gp0, gv0 = cp, cv
        ops = []
        runbase = {}
        blocks = {b: [] for b in bs}
        jloc = {b: 0 for b in bs}
        for s in range(NSEG):
            run_slots = sum(int(S[b, s]) for b in bs)
            if run_slots == 0:
                continue
            runbase[s] = cp
            cols = -(-run_slots // 128)
            off = 0
            while off < cols:
                take = min(CAPC, cols - off)
                ops.append((s, cp + off, take))
                off += take
            o = 0
            for b in bs:
                sb = int(S[b, s])
                if sb == 0:
                    continue
                c_lo = o // 128
                c_hi = (o + sb - 1) // 128
                ncols = c_hi - c_lo + 1
                blocks[b].append((s, cp + c_lo, ncols, cv, jloc[b]))
                jloc[b] += ncols
                cv += ncols
                o += sb
            cp += cols
        groups.append({"gp0": gp0, "nchp": cp - gp0,
                       "gv0": gv0, "nchv": cv - gv0,
                       "ops": ops, "runbase": runbase,
                       "blocks": blocks, "nv": dict(jloc)})
    return cp, cv, groups


def _host_prep(x, src, dst, W1, al1, ar1, b1, W2, al2, ar2, b2):
    f32, bf16, i16 = np.float32, ml_dtypes.bfloat16, np.int16
    src = np.asarray(src).astype(np.int64)
    dst = np.asarray(dst).astype(np.int64)

    core = dst // NPC
    r = dst % NPC
    b = r // P
    dl = r % P
    trow_src = (src // NPC) * NPCP + (src % NPC)
    seg = trow_src // SEG
    bgid = core * NB + b

    key = bgid * NSEG + seg
    counts = np.bincount(key, minlength=NCORES * NB * NSEG)
    counts3 = counts.reshape(NCORES, NB, NSEG)
    S = counts3.max(axis=0)                   # [NB, NSEG] slots per bucket
    CHP, CHV, groups = _plan(S)

    order = np.argsort(key, kind="stable")
    s_trow = trow_src[order]
    s_seg = seg[order]
    s_b = b[order]
    s_bgid = bgid[order]
    s_dl = dl[order]

    seg_start = np.concatenate([[0], np.cumsum(counts)])[:-1]
    rank = np.arange(len(order), dtype=np.int64) - seg_start[key[order]]

    runbase_tab = np.zeros((NB, NSEG), np.int64)  # (g,s) run base pcol
    off_tab = np.zeros((NB, NSEG), np.int64)      # bucket slot offset in run
    vcol_tab = np.zeros((NB, NSEG), np.int64)     # bucket first view col
    pcol_tab = np.zeros((NB, NSEG), np.int64)     # bucket first phys col
    for gr in groups:
        for bb, runs in gr["blocks"].items():
            for (s, pcol0, ncols, vcol0, j0) in runs:
                rb = gr["runbase"][s]
                runbase_tab[bb, s] = rb
                pcol_tab[bb, s] = pcol0
                vcol_tab[bb, s] = vcol0
        # recompute bucket slot offsets in run order
        bs = sorted(gr["blocks"])
        for s in range(NSEG):
            o = 0
            for bb in bs:
                off_tab[bb, s] = o
                o += int(S[bb, s])

    slot = off_tab[s_b, s_seg] + rank
    pcol = runbase_tab[s_b, s_seg] + slot // P
    p = slot % P
    vcol = vcol_tab[s_b, s_seg] + (pcol - pcol_tab[s_b, s_seg])

    # dstl: [128, CHV] destination-local row per (slot, VIEW col); 200 = pad
    dstl = np.full((NCORES, P, CHV), 200.0, f32)
    flat = (s_bgid // NB) * (P * CHV) + p * CHV + vcol
    dstl.reshape(-1)[flat] = s_dl

    # gather indices: [16 wrap, CHP*8], segment-relative; ops start at
    # 8-column boundaries from each run base
    fidx = np.zeros((NCORES, 16, CHP * 8), i16)
    rb = runbase_tab[s_b, s_seg]
    opbase = rb + ((pcol - rb) // CAPC) * CAPC
    i_op = (pcol - opbase) * P + p
    row16 = i_op % 16
    col16 = i_op // 16
    abscol = opbase * 8 + col16
    fflat = (s_bgid // NB) * (16 * CHP * 8) + row16 * (CHP * 8) + abscol
    fidx.reshape(-1)[fflat] = (s_trow - s_seg * SEG).astype(i16)
    fidx = np.tile(fidx, (1, 8, 1))

    # two-level er-select masks: dl = 8*hi + lo
    #   MHI fp8 [16, CH*128]: MHI[k, c*128+i] = (dstl[i,c]//8 == k)
    #   LO8 bf16 [128, CH*8]: LO8[i, c*8+l] = (dstl[i,c]%8 == l) & valid
    one8 = np.float32(1.0).astype(ml_dtypes.float8_e4m3).view(np.uint8)
    MHI = np.zeros((NCORES, 16, CHV * P), np.uint8)
    LO8 = np.zeros((NCORES, P, CHV * 8), bf16)
    for cc in range(NCORES):
        d = dstl[cc].astype(np.int64)         # [P(slots), CH]
        hi = (d // 8).T                       # [CH, i]; pad 200//8=25
        eqh = hi[:, :, None] == np.arange(16)[None, None, :]
        MHI[cc][np.transpose(eqh, (2, 0, 1)).reshape(16, CHV * P)] = one8
        lo = (d % 8).T
        valid = (d < P).T
        eql = (lo[:, :, None] == np.arange(8)[None, None, :]) \
            & valid[:, :, None]               # [CH, i, 8]
        LO8[cc][np.transpose(eql, (1, 0, 2)).reshape(P, CHV * 8)] = bf16(1.0)
    MHI = MHI.view(ml_dtypes.float8_e4m3)

    def aug(W, al, ar):
        W = np.asarray(W, f32)
        Wa = np.zeros((W.shape[0], 2 + H), f32)
        Wa[:, 0] = W @ np.asarray(al, f32)
        Wa[:, 1:1 + H] = W
        Wa[:, 1 + H] = W @ np.asarray(ar, f32)
        return Wa

    W1a = aug(W1, al1, ar1)
    W2a = aug(W2, al2, ar2)
    b1r = np.tile(np.asarray(b1, f32)[None, :], (P, 1))
    b2r = np.tile(np.asarray(b2, f32)[None, :], (P, 1))
    iota = np.tile(np.arange(P, dtype=bf16)[None, :], (P, 1))

    x = np.asarray(x, f32)
    xsT = np.zeros((NCORES, F, NPCP), f32)
    for cc in range(NCORES):
        xsT[cc, :, :NPC] = x[cc * NPC:(cc + 1) * NPC].T

    in_maps = []
    for cc in range(NCORES):
        in_maps.append({
            "xsT": xsT[cc],
            "W1a": W1a, "W2a": W2a, "b1r": b1r, "b2r": b2r, "iota": iota,
            "fidx": fidx[cc], "dstl": dstl[cc],
            "mhi": MHI[cc], "lo8": LO8[cc],
        })
    return in_maps, tuple(int(v) for v in S.reshape(-1))


def _build_program(R_key, single=False):
    import concourse.bacc as bacc
    import concourse.mybir as mybir
    import concourse.tile as tile

    dt = mybir.dt
    S = np.asarray(R_key, np.int64).reshape(NB, NSEG)
    CHP, CHV, groups = _plan(S)
    ncores = 1 if single else NCORES

    nc = bacc.Bacc("TRN2", target_bir_lowering=False, debug=False,
                   num_devices=ncores, num_swdge_queues=4)

    xsT = nc.dram_tensor("xsT", [F, NPCP], dt.float32, kind="ExternalInput")
    W1a = nc.dram_tensor("W1a", [F, 2 + H], dt.float32, kind="ExternalInput")
    W2a = nc.dram_tensor("W2a", [H, 2 + H], dt.float32, kind="ExternalInput")
    b1r = nc.dram_tensor("b1r", [P, H], dt.float32, kind="ExternalInput")
    b2r = nc.dram_tensor("b2r", [P, H], dt.float32, kind="ExternalInput")
    iota = nc.dram_tensor("iota", [P, P], dt.bfloat16, kind="ExternalInput")
    fidx = nc.dram_tensor("fidx", [P, CHP * 8], dt.int16, kind="ExternalInput")
    dstl = nc.dram_tensor("dstl", [P, CHV], dt.float32, kind="ExternalInput")
    mhi = nc.dram_tensor("mhi", [16, CHV * P], dt.float8e4, kind="ExternalInput")
    lo8 = nc.dram_tensor("lo8", [P, CHV * 8], dt.bfloat16, kind="ExternalInput")
    out_ext = nc.dram_tensor("out", [NPC, H], dt.float32, kind="ExternalOutput")

    qn_state = [0]

    def qn():
        qn_state[0] = (qn_state[0] + 1) % 4
        return qn_state[0]

    with tile.TileContext(nc) as tc:
        with (
            tc.tile_pool(name="const", bufs=1) as const,
            tc.tile_pool(name="prod", bufs=4) as prod,
            tc.tile_pool(name="tv", bufs=2) as tvpool,
            tc.tile_pool(name="mg", bufs=2) as mgpool,
            tc.tile_pool(name="fxg", bufs=2) as fxpool,
            tc.tile_pool(name="oh", bufs=12) as ohpool,
            tc.tile_pool(name="ee", bufs=4) as eepool,
            tc.tile_pool(name="epi", bufs=4) as epipool,
            tc.tile_pool(name="ps", bufs=3, space="PSUM") as psum,
            tc.tile_pool(name="pse", bufs=2, space="PSUM") as psume,
            tc.tile_pool(name="pst", bufs=2, space="PSUM") as psumt,
            tc.tile_pool(name="ptt", bufs=1, space="PSUM") as psumtt,
            tc.tile_pool(name="dram", bufs=1, space="DRAM") as dram,
        ):
            iota_sb = const.tile([P, P], dt.bfloat16)
            nc.sync.dma_start(out=iota_sb[:], in_=iota[:])
            W1a_sb = const.tile([F, 2 + H], dt.float32)
            nc.sync.dma_start(out=W1a_sb[:], in_=W1a[:])
            W2a_sb = const.tile([H, 2 + H], dt.float32)
            nc.sync.dma_start(out=W2a_sb[:], in_=W2a[:])
            b1r_sb = const.tile([P, H], dt.float32)
            nc.sync.dma_start(out=b1r_sb[:], in_=b1r[:])
            b2r_sb = const.tile([P, H], dt.float32)
            nc.sync.dma_start(out=b2r_sb[:], in_=b2r[:])
            dstl_sb = const.tile([P, CHV], dt.float32)
            nc.sync.dma_start(out=dstl_sb[:], in_=dstl[:])
            er_all = const.tile([P, NB], dt.float16)
            er_all2 = const.tile([P, NB], dt.float16)
            er2d1 = const.tile([16, NB * 8], dt.float16)
            er2d2 = const.tile([16, NB * 8], dt.float16)

            feat1_s = dram.tile([NPCP, TW], dt.bfloat16)
            feat1_f = dram.tile([TROWS, TW], dt.bfloat16, addr_space="Shared")
            feat2_s = dram.tile([NPCP, TW], dt.bfloat16)
            feat2_f = dram.tile([TROWS, TW], dt.bfloat16, addr_space="Shared")

            def build_row(pmm, er_dst, er2d, b, fsb4, k):
                """pmm [P, 2+H] f32 = [el | feat | er] -> bf16 row + er."""
                o = k * TW
                nc.vector.tensor_copy(
                    out=fsb4[:, o + ELC:o + ELC + 2].bitcast(dt.float32),
                    in_=pmm[:, 0:1])
                nc.vector.tensor_copy(out=fsb4[:, o + FEATC:o + FEATC + H],
                                      in_=pmm[:, 1:1 + H])
                nc.vector.tensor_copy(out=er_dst[:, b:b + 1],
                                      in_=pmm[:, 1 + H:2 + H])

            TB = 8

            def emit_er2d(er2d, er_dst):
                # l-major: er2d[k, l*NB + b] = er[8k+l of block b]
                nc.scalar.dma_start(
                    out=er2d[:].rearrange("k (l b) -> k l b", b=NB),
                    in_=er_dst[:])

            def write_rows(feat_s, b0, nb, fsb4):
                nc.sync.dma_start(
                    out=feat_s[b0 * P:(b0 + nb) * P, :]
                        .rearrange("(blk r) w -> r blk w", blk=nb),
                    in_=fsb4[:, :nb * TW].rearrange("p (blk w) -> p blk w",
                                                    w=TW))

            # ---- layer-1 table ----
            for b0 in range(0, NB, TB):
                nb = min(TB, NB - b0)
                xt = prod.tile([F, TB * P], dt.float32, tag="xt")
                nc.sync.dma_start(out=xt[:, :nb * P],
                                  in_=xsT[:, b0 * P:(b0 + nb) * P])
                fsb4 = prod.tile([P, TB * TW], dt.bfloat16, tag="fsb4")
                nc.vector.memset(
                    fsb4[:].rearrange("p (blk w) -> p blk w", w=TW)
                    [:, :, ONEC:ONEC + 1], 1.0)
                for k in range(nb):
                    pmm = psumt.tile([P, 2 + H], dt.float32, tag="pmm")
                    nc.tensor.matmul(out=pmm[:], lhsT=xt[:, k * P:(k + 1) * P],
                                     rhs=W1a_sb[:], start=True, stop=True)
                    build_row(pmm, er_all, er2d1, b0 + k, fsb4, k)
                write_rows(feat1_s, b0, nb, fsb4)

            def allgather(src_t, dst_t):
                if single:
                    nc.sync.dma_start(
                        out=dst_t[0:NPCP, :].rearrange("a b -> (a b)"),
                        in_=src_t[:].rearrange("a b -> (a b)"))
                else:
                    nc.gpsimd.collective_compute(
                        "AllGather", mybir.AluOpType.bypass,
                        replica_groups=[list(range(NCORES))],
                        ins=[src_t[:]], outs=[dst_t[:]],
                    )

            allgather(feat1_s, feat1_f)
            emit_er2d(er2d1, er_all)

            # ---- edge phase ----
            def edge_phase(feat_f, er2d, bias_sb, relu, out_writer):
                for g, gr in enumerate(groups):
                    gp0, nchp = gr["gp0"], gr["nchp"]
                    gv0, nchv = gr["gv0"], gr["nchv"]
                    fxg = fxpool.tile([P, nchp * 8], dt.int16, tag="fxg")
                    nc.sync.dma_start(out=fxg[:],
                                      in_=fidx[:, gp0 * 8:(gp0 + nchp) * 8])
                    mhg = mgpool.tile([16, nchv * P], dt.float8e4, tag="mhg")
                    nc.sync.dma_start(out=mhg[:],
                                      in_=mhi[:, gv0 * P:(gv0 + nchv) * P])
                    log = mgpool.tile([P, nchv * 8], dt.bfloat16, tag="log")
                    nc.sync.dma_start(out=log[:],
                                      in_=lo8[:, gv0 * 8:(gv0 + nchv) * 8])
                    tvg = tvpool.tile([P, nchp * TW], dt.bfloat16, tag="tvg")
                    tv3 = tvg[:].rearrange("p (r e) -> p r e", e=TW)
                    for (s, c0, take) in gr["ops"]:
                        seg_lo = s * SEG
                        seg_hi = min(seg_lo + SEG, TROWS)
                        nc.gpsimd.dma_gather(
                            out_ap=tv3[:, c0 - gp0:c0 - gp0 + take, :],
                            in_ap=feat_f[seg_lo:seg_hi, :],
                            idxs_ap=fxg[:, (c0 - gp0) * 8:
                                        (c0 - gp0 + take) * 8],
                            num_idxs=take * P, num_idxs_reg=take * P,
                            elem_size=TW, queue_num=qn(),
                        )
                    def emit_er(b):
                        # er select stage 1: hi via fp8 [16,128] matmuls ->
                        # [128,8] candidate rows per chunk (emitted one block
                        # ahead so PE's in-order queue can't stall DVE).
                        runs = gr["blocks"][b]
                        Rb = gr["nv"][b]
                        er_ps = psume.tile([P, Rb * 8], dt.float32,
                                           tag="erps")
                        for (s, pcol0, ncols, vcol0, j0) in runs:
                            for rr in range(ncols):
                                gc = vcol0 - gv0 + rr
                                nc.tensor.matmul(
                                    out=er_ps[:, (j0 + rr) * 8:
                                              (j0 + rr + 1) * 8],
                                    lhsT=mhg[:, gc * P:(gc + 1) * P],
                                    rhs=er2d[:].rearrange(
                                        "k (l b2) -> k l b2", b2=NB)
                                        [:, :, b],
                                    start=True, stop=True)
                        return er_ps

                    bl = sorted(gr["blocks"])
                    er_tiles = {bl[0]: emit_er(bl[0])}
                    for bi, b in enumerate(bl):
                        if bi + 1 < len(bl):
                            er_tiles[bl[bi + 1]] = emit_er(bl[bi + 1])
                        runs = gr["blocks"][b]  # (s, pcol0, ncols, vcol0, j0)
                        Rb = gr["nv"][b]
                        er_ps = er_tiles.pop(b)
                        sel8 = eepool.tile([P, Rb * 8], dt.bfloat16,
                                           tag="sel8")
                        for (s, pcol0, ncols, vcol0, j0) in runs:
                            nc.vector.tensor_tensor(
                                out=sel8[:, j0 * 8:(j0 + ncols) * 8],
                                in0=er_ps[:, j0 * 8:(j0 + ncols) * 8],
                                in1=log[:, (vcol0 - gv0) * 8:
                                        (vcol0 - gv0 + ncols) * 8],
                                op=mybir.AluOpType.mult)
                        ers = eepool.tile([P, Rb], dt.float32, tag="ers")
                        nc.vector.tensor_reduce(
                            out=ers[:],
                            in_=sel8[:].rearrange("p (r e) -> p r e", e=8),
                            axis=mybir.AxisListType.X,
                            op=mybir.AluOpType.add)
                        # ee = el + er ; lrelu ; ex
                        ee = eepool.tile([P, Rb], dt.float32, tag="ee")
                        for (s, pcol0, ncols, vcol0, j0) in runs:
                            el = tv3[:, pcol0 - gp0:pcol0 - gp0 + ncols,
                                     ELC:ELC + 2].bitcast(dt.float32)
                            nc.vector.tensor_tensor(
                                out=ee[:, j0:j0 + ncols]
                                    .rearrange("p (r o) -> p r o", o=1),
                                in0=el,
                                in1=ers[:, j0:j0 + ncols]
                                    .rearrange("p (r o) -> p r o", o=1),
                                op=mybir.AluOpType.add)
                        nc.vector.scalar_tensor_tensor(
                            out=ee[:], in0=ee[:], scalar=0.2, in1=ee[:],
                            op0=mybir.AluOpType.mult,
                            op1=mybir.AluOpType.max)
                        ex = eepool.tile([P, Rb], dt.float32, tag="ex")
                        nc.scalar.activation(
                            out=ex[:], in_=ee[:],
                            func=mybir.ActivationFunctionType.Exp)
                        # fused one-hot(+ex) per chunk; rhs = table slice
                        pacc = psum.tile([P, WFW], dt.float32, tag="pacc")
                        done = 0
                        for (s, pcol0, ncols, vcol0, j0) in runs:
                            for rr in range(ncols):
                                oh = ohpool.tile([P, P], dt.bfloat16,
                                                 tag="oh")
                                nc.vector.tensor_scalar(
                                    out=oh[:], in0=iota_sb[:],
                                    scalar1=dstl_sb[:, vcol0 + rr:
                                                    vcol0 + rr + 1],
                                    scalar2=ex[:, j0 + rr:j0 + rr + 1],
                                    op0=mybir.AluOpType.is_equal,
                                    op1=mybir.AluOpType.mult,
                                )
                                nc.tensor.matmul(
                                    out=pacc[:],
                                    lhsT=oh[:],
                                    rhs=tv3[:, pcol0 - gp0 + rr,
                                            ONEC:ONEC + WFW],
                                    start=(done == 0), stop=(done == Rb - 1))
                                done += 1
                        # epilogue
                        den = epipool.tile([P, 1], dt.float32, tag="den")
                        nc.vector.tensor_scalar_add(out=den[:],
                                                    in0=pacc[:, 0:1],
                                                    scalar1=1e-30)
                        rec = epipool.tile([P, 1], dt.float32, tag="rec")
                        nc.vector.reciprocal(out=rec[:], in_=den[:])
                        h = epipool.tile([P, H], dt.float32, tag="h")
                        nc.vector.scalar_tensor_tensor(
                            out=h[:], in0=pacc[:, 1:], scalar=rec[:],
                            in1=bias_sb[:],
                            op0=mybir.AluOpType.mult,
                            op1=mybir.AluOpType.add)
                        if relu:
                            nc.scalar.activation(
                                out=h[:], in_=h[:],
                                func=mybir.ActivationFunctionType.Relu)
                        out_writer(b, h)

            ident = const.tile([P, P], dt.float32)
            from concourse.masks import make_identity
            make_identity(nc, ident[:])

            l1_state = {}

            def l1_writer(b, h):
                pt = psumtt.tile([H, P], dt.float32, tag="pt")
                nc.tensor.transpose(out=pt[:], in_=h[:], identity=ident[:])
                hT = prod.tile([H, P], dt.float32, tag="hT")
                nc.vector.tensor_copy(out=hT[:], in_=pt[:])
                pmm2 = psumt.tile([P, 2 + H], dt.float32, tag="pmm")
                nc.tensor.matmul(out=pmm2[:], lhsT=hT[:], rhs=W2a_sb[:],
                                 start=True, stop=True)
                k = b % TB
                if k == 0:
                    fsb4b = prod.tile([P, TB * TW], dt.bfloat16,
                                      tag="fsb4")
                    nc.vector.memset(
                        fsb4b[:].rearrange("p (blk w) -> p blk w", w=TW)
                        [:, :, ONEC:ONEC + 1], 1.0)
                    l1_state["fsb4"] = fsb4b
                build_row(pmm2, er_all2, er2d2, b, l1_state["fsb4"], k)
                if k == TB - 1 or b == NB - 1:
                    write_rows(feat2_s, b - k, k + 1, l1_state["fsb4"])
                if b == NB - 1:
                    emit_er2d(er2d2, er_all2)

            edge_phase(feat1_f, er2d1, b1r_sb, True, l1_writer)
            allgather(feat2_s, feat2_f)

            def l2_writer(b, h):
                rows = LB if b == NB - 1 else P
                nc.sync.dma_start(out=out_ext[b * P:b * P + rows, :],
                                  in_=h[:rows, :])

            edge_phase(feat2_f, er2d2, b2r_sb, False, l2_writer)

    nc.compile()
    return nc


def _get_program(R_key, single=False):
    key = ("prog", R_key, single)
    if key not in _cache:
        _cache[key] = _build_program(R_key, single=single)
    return _cache[key]


def kernel(x, src, dst, W1, al1, ar1, b1, W2, al2, ar2, b2):
    from concourse.bass_utils import run_bass_kernel_spmd

    in_maps, R_key = _host_prep(x, src, dst, W1, al1, ar1, b1,
                                W2, al2, ar2, b2)
    nc = _get_program(R_key)
    res = run_bass_kernel_spmd(nc, in_maps, list(range(NCORES)))
    out = np.concatenate([res.results[c]["out"] for c in range(NCORES)],
                         axis=0)
    return out.astype(np.float32)


# revision 24
# speedup vs baseline: 1.2156x; 1.0287x over previous
"""GAT (2-layer, single-head) Trainium2 Bass kernel, 8-core SPMD. v2.

Design (vs v1 baseline):
  - dst nodes 1D-sharded (12500/core); edges routed to dst core, bucketed
    by (dst block of 128, src segment of 32768 table rows). Within each
    (group of G blocks, segment) run, buckets pack CONTIGUOUSLY into
    128-slot columns (no per-bucket rounding); a column shared by two
    blocks is processed once per block through its own VIEW column of the
    dstl/MHI/LO8 masks (foreign slots masked out). One dma_gather op
    (<=1024 idxs, ucode cap) spans buckets within a run.
  - Table row = 128 bf16 (256B, the dma_gather minimum elem):
    [el as f32 (2 slots) | 1.0 | feat(32) | pad]. el kept f32 for softmax
    accuracy; feat bf16.
  - NO per-edge er gather (v1 spent ~50% of its descriptors on it).
    er[dst] is selected on-chip in two levels (dl = 8*hi + lo): per
    chunk a host-shipped fp8 hi-one-hot MHI [16,128] is matmul'd with
    the block's er table redistributed to [16,8] fp16, giving [128,8]
    candidates in PSUM; a host-shipped bf16 lo-one-hot mask [128,8] and
    an X-axis reduce pick the final er per slot. 16B+128B of mask bytes
    per chunk replace 256B of gathered bytes per EDGE.
  - aggregation per chunk: one-hot(+ex) built in ONE fused tensor_scalar
    (is_equal, mult) on bf16 iota (4x DVE mode, ~92ns) with ex and dl as
    per-partition scalars; matmul rhs reads the gathered table slice
    [1|feat] directly; denominator accumulates via the "1" column.
  - Epilogue: out = pacc[:,1:]/pacc[:,0] + bias (+relu), layer-2 table
    built inline; AllGather between layers (excluded from the metric, a
    local copy in the single-core cost program).
"""

import numpy as np
import ml_dtypes

N = 100000
E = 1600000
F = 128
H = 32
NCORES = 8
NPC = N // NCORES          # nodes per core
P = 128
NB = (NPC + P - 1) // P    # dst blocks per core (98; last block 84 rows)
LB = NPC - (NB - 1) * P    # rows in last block
NPCP = NB * P              # padded nodes per core (table rows per core)
TROWS = NCORES * NPCP      # full table rows
TW = 128                   # table row: 128 bf16 = 256B
ELC = 0                    # cols 0-1: el as f32
ONEC = 2                   # col 2: 1.0
FEATC = 3                  # cols 3..34: feat
WFW = 1 + H                # aggregation rhs width: [1 | feat]
SEG = 32768                # src segment size (int16 gather indices)
NSEG = (TROWS + SEG - 1) // SEG
G = 8                      # dst blocks per group tile
NG = (NB + G - 1) // G
CAPC = 8                   # max chunks (1024 indices) per dma_gather op
PADT = 48                  # pad bucket tail to column boundary if gap < PADT

_cache = {}


def _plan(S):
    """Column layout. S: [NB, NSEG] slot counts per bucket (max over cores,
    NOT rounded to 128). Within each (group, seg) run, buckets pack
    contiguously; physical columns = ceil(run_slots/128); a column shared
    by two blocks is processed once per block through its own VIEW column
    of the dstl/MHI/LO8 masks (foreign slots masked to 200/zero).

    groups[g]: gp0/nchp (physical cols), gv0/nchv (view cols),
      ops: (s, pcol0, take<=8) gather ops,
      runbase: {s: run base pcol},
      blocks: {b: [(s, pcol0, ncols, vcol0, j0)]}, nv: {b: Vb}.
    """
    groups = []
    cp = 0
    cv = 0
    for g in range(NG):
        bs = list(range(g * G, min((g + 1) * G, NB)))
        gp0, gv0 = cp, cv
        ops = []
        runbase = {}
        blocks = {b: [] for b in bs}
        jloc = {b: 0 for b in bs}
        for s in range(NSEG):
            run_slots = sum(int(S[b, s]) for b in bs)
            if run_slots == 0:
                continue
            runbase[s] = cp
            op_mark = len(ops)
            o = 0
            for b in bs:
                sb = int(S[b, s])
                if sb == 0:
                    continue
                c_lo = o // 128
                c_hi = (o + sb - 1) // 128
                ncols = c_hi - c_lo + 1
                blocks[b].append((s, cp + c_lo, ncols, cv, jloc[b], o))
                jloc[b] += ncols
                cv += ncols
                o += sb
                gap = (-o) % 128
                if 0 < gap < PADT:
                    o += gap
            cols = -(-o // 128)
            # re-emit ops with the padded column count
            del ops[op_mark:]
            off = 0
            while off < cols:
                take = min(CAPC, cols - off)
                ops.append((s, cp + off, take))
                off += take
            cp += cols
        groups.append({"gp0": gp0, "nchp": cp - gp0,
                       "gv0": gv0, "nchv": cv - gv0,
                       "ops": ops, "runbase": runbase,
                       "blocks": blocks, "nv": dict(jloc)})
    return cp, cv, groups


def _host_prep(x, src, dst, W1, al1, ar1, b1, W2, al2, ar2, b2):
    f32, bf16, i16 = np.float32, ml_dtypes.bfloat16, np.int16
    src = np.asarray(src).astype(np.int64)
    dst = np.asarray(dst).astype(np.int64)

    core = dst // NPC
    r = dst % NPC
    b = r // P
    dl = r % P
    trow_src = (src // NPC) * NPCP + (src % NPC)
    seg = trow_src // SEG
    bgid = core * NB + b

    key = bgid * NSEG + seg
    counts = np.bincount(key, minlength=NCORES * NB * NSEG)
    counts3 = counts.reshape(NCORES, NB, NSEG)
    S = counts3.max(axis=0)                   # [NB, NSEG] slots per bucket
    CHP, CHV, groups = _plan(S)

    order = np.argsort(key, kind="stable")
    s_trow = trow_src[order]
    s_seg = seg[order]
    s_b = b[order]
    s_bgid = bgid[order]
    s_dl = dl[order]

    seg_start = np.concatenate([[0], np.cumsum(counts)])[:-1]
    rank = np.arange(len(order), dtype=np.int64) - seg_start[key[order]]

    runbase_tab = np.zeros((NB, NSEG), np.int64)  # (g,s) run base pcol
    off_tab = np.zeros((NB, NSEG), np.int64)      # bucket slot offset in run
    vcol_tab = np.zeros((NB, NSEG), np.int64)     # bucket first view col
    pcol_tab = np.zeros((NB, NSEG), np.int64)     # bucket first phys col
    for gr in groups:
        for bb, runs in gr["blocks"].items():
            for (s, pcol0, ncols, vcol0, j0, _o) in runs:
                runbase_tab[bb, s] = gr["runbase"][s]
                pcol_tab[bb, s] = pcol0
                vcol_tab[bb, s] = vcol0
                off_tab[bb, s] = _o

    slot = off_tab[s_b, s_seg] + rank
    pcol = runbase_tab[s_b, s_seg] + slot // P
    p = slot % P
    vcol = vcol_tab[s_b, s_seg] + (pcol - pcol_tab[s_b, s_seg])

    # dstl: [128, CHV] destination-local row per (slot, VIEW col); 200 = pad
    dstl = np.full((NCORES, P, CHV), 200.0, f32)
    flat = (s_bgid // NB) * (P * CHV) + p * CHV + vcol
    dstl.reshape(-1)[flat] = s_dl

    # gather indices: [16 wrap, CHP*8], segment-relative; ops start at
    # 8-column boundaries from each run base
    fidx = np.zeros((NCORES, 16, CHP * 8), i16)
    rb = runbase_tab[s_b, s_seg]
    opbase = rb + ((pcol - rb) // CAPC) * CAPC
    i_op = (pcol - opbase) * P + p
    row16 = i_op % 16
    col16 = i_op // 16
    abscol = opbase * 8 + col16
    fflat = (s_bgid // NB) * (16 * CHP * 8) + row16 * (CHP * 8) + abscol
    fidx.reshape(-1)[fflat] = (s_trow - s_seg * SEG).astype(i16)
    fidx = np.tile(fidx, (1, 8, 1))

    # two-level er-select masks: dl = 8*hi + lo
    #   MHI fp8 [16, CH*128]: MHI[k, c*128+i] = (dstl[i,c]//8 == k)
    #   LO8 bf16 [128, CH*8]: LO8[i, c*8+l] = (dstl[i,c]%8 == l) & valid
    one8 = np.float32(1.0).astype(ml_dtypes.float8_e4m3).view(np.uint8)
    MHI = np.zeros((NCORES, 16, CHV * P), np.uint8)
    LO8 = np.zeros((NCORES, P, CHV * 8), bf16)
    for cc in range(NCORES):
        d = dstl[cc].astype(np.int64)         # [P(slots), CH]
        hi = (d // 8).T                       # [CH, i]; pad 200//8=25
        eqh = hi[:, :, None] == np.arange(16)[None, None, :]
        MHI[cc][np.transpose(eqh, (2, 0, 1)).reshape(16, CHV * P)] = one8
        lo = (d % 8).T
        valid = (d < P).T
        eql = (lo[:, :, None] == np.arange(8)[None, None, :]) \
            & valid[:, :, None]               # [CH, i, 8]
        LO8[cc][np.transpose(eql, (1, 0, 2)).reshape(P, CHV * 8)] = bf16(1.0)
    MHI = MHI.view(ml_dtypes.float8_e4m3)

    def aug(W, al, ar):
        W = np.asarray(W, f32)
        Wa = np.zeros((W.shape[0], 2 + H), f32)
        Wa[:, 0] = W @ np.asarray(al, f32)
        Wa[:, 1:1 + H] = W
        Wa[:, 1 + H] = W @ np.asarray(ar, f32)
        return Wa

    W1a = aug(W1, al1, ar1)
    W2a = aug(W2, al2, ar2)
    b1r = np.tile(np.asarray(b1, f32)[None, :], (P, 1))
    b2r = np.tile(np.asarray(b2, f32)[None, :], (P, 1))
    iota = np.tile(np.arange(P, dtype=bf16)[None, :], (P, 1))

    x = np.asarray(x, f32)
    xsT = np.zeros((NCORES, F, NPCP), f32)
    for cc in range(NCORES):
        xsT[cc, :, :NPC] = x[cc * NPC:(cc + 1) * NPC].T

    in_maps = []
    for cc in range(NCORES):
        in_maps.append({
            "xsT": xsT[cc],
            "W1a": W1a, "W2a": W2a, "b1r": b1r, "b2r": b2r, "iota": iota,
            "fidx": fidx[cc], "dstl": dstl[cc],
            "mhi": MHI[cc], "lo8": LO8[cc],
        })
    return in_maps, tuple(int(v) for v in S.reshape(-1))


def _build_program(R_key, single=False):
    import concourse.bacc as bacc
    import concourse.mybir as mybir
    import concourse.tile as tile

    dt = mybir.dt
    S = np.asarray(R_key, np.int64).reshape(NB, NSEG)
    CHP, CHV, groups = _plan(S)
    ncores = 1 if single else NCORES

    nc = bacc.Bacc("TRN2", target_bir_lowering=False, debug=False,
                   num_devices=ncores, num_swdge_queues=4)

    xsT = nc.dram_tensor("xsT", [F, NPCP], dt.float32, kind="ExternalInput")
    W1a = nc.dram_tensor("W1a", [F, 2 + H], dt.float32, kind="ExternalInput")
    W2a = nc.dram_tensor("W2a", [H, 2 + H], dt.float32, kind="ExternalInput")
    b1r = nc.dram_tensor("b1r", [P, H], dt.float32, kind="ExternalInput")
    b2r = nc.dram_tensor("b2r", [P, H], dt.float32, kind="ExternalInput")
    iota = nc.dram_tensor("iota", [P, P], dt.bfloat16, kind="ExternalInput")
    fidx = nc.dram_tensor("fidx", [P, CHP * 8], dt.int16, kind="ExternalInput")
    dstl = nc.dram_tensor("dstl", [P, CHV], dt.float32, kind="ExternalInput")
    mhi = nc.dram_tensor("mhi", [16, CHV * P], dt.float8e4, kind="ExternalInput")
    lo8 = nc.dram_tensor("lo8", [P, CHV * 8], dt.bfloat16, kind="ExternalInput")
    out_ext = nc.dram_tensor("out", [NPC, H], dt.float32, kind="ExternalOutput")

    qn_state = [0]

    def qn():
        qn_state[0] = (qn_state[0] + 1) % 4
        return qn_state[0]

    with tile.TileContext(nc) as tc:
        with (
            tc.tile_pool(name="const", bufs=1) as const,
            tc.tile_pool(name="prod", bufs=4) as prod,
            tc.tile_pool(name="tv", bufs=2) as tvpool,
            tc.tile_pool(name="mg", bufs=2) as mgpool,
            tc.tile_pool(name="fxg", bufs=2) as fxpool,
            tc.tile_pool(name="oh", bufs=12) as ohpool,
            tc.tile_pool(name="ee", bufs=4) as eepool,
            tc.tile_pool(name="epi", bufs=4) as epipool,
            tc.tile_pool(name="ps", bufs=3, space="PSUM") as psum,
            tc.tile_pool(name="pse", bufs=2, space="PSUM") as psume,
            tc.tile_pool(name="pst", bufs=2, space="PSUM") as psumt,
            tc.tile_pool(name="ptt", bufs=1, space="PSUM") as psumtt,
            tc.tile_pool(name="dram", bufs=1, space="DRAM") as dram,
        ):
            iota_sb = const.tile([P, P], dt.bfloat16)
            nc.sync.dma_start(out=iota_sb[:], in_=iota[:])
            W1a_sb = const.tile([F, 2 + H], dt.float32)
            nc.sync.dma_start(out=W1a_sb[:], in_=W1a[:])
            W2a_sb = const.tile([H, 2 + H], dt.float32)
            nc.sync.dma_start(out=W2a_sb[:], in_=W2a[:])
            b1r_sb = const.tile([P, H], dt.float32)
            nc.sync.dma_start(out=b1r_sb[:], in_=b1r[:])
            b2r_sb = const.tile([P, H], dt.float32)
            nc.sync.dma_start(out=b2r_sb[:], in_=b2r[:])
            dstl_sb = const.tile([P, CHV], dt.float32)
            nc.sync.dma_start(out=dstl_sb[:], in_=dstl[:])
            er_all = const.tile([P, NB], dt.float16)
            er_all2 = const.tile([P, NB], dt.float16)
            er2d1 = const.tile([16, NB * 8], dt.float16)
            er2d2 = const.tile([16, NB * 8], dt.float16)

            feat1_s = dram.tile([NPCP, TW], dt.bfloat16)
            feat1_f = dram.tile([TROWS, TW], dt.bfloat16, addr_space="Shared")
            feat2_s = dram.tile([NPCP, TW], dt.bfloat16)
            feat2_f = dram.tile([TROWS, TW], dt.bfloat16, addr_space="Shared")

            def build_row(pmm, er_dst, er2d, b, fsb4, k):
                """pmm [P, 2+H] f32 = [el | feat | er] -> bf16 row + er."""
                o = k * TW
                nc.vector.tensor_copy(
                    out=fsb4[:, o + ELC:o + ELC + 2].bitcast(dt.float32),
                    in_=pmm[:, 0:1])
                nc.vector.tensor_copy(out=fsb4[:, o + FEATC:o + FEATC + H],
                                      in_=pmm[:, 1:1 + H])
                nc.vector.tensor_copy(out=er_dst[:, b:b + 1],
                                      in_=pmm[:, 1 + H:2 + H])

            TB = 8

            def emit_er2d(er2d, er_dst):
                # l-major: er2d[k, l*NB + b] = er[8k+l of block b]
                nc.scalar.dma_start(
                    out=er2d[:].rearrange("k (l b) -> k l b", b=NB),
                    in_=er_dst[:])

            def write_rows(feat_s, b0, nb, fsb4):
                nc.sync.dma_start(
                    out=feat_s[b0 * P:(b0 + nb) * P, :]
                        .rearrange("(blk r) w -> r blk w", blk=nb),
                    in_=fsb4[:, :nb * TW].rearrange("p (blk w) -> p blk w",
                                                    w=TW))

            # ---- layer-1 table ----
            for b0 in range(0, NB, TB):
                nb = min(TB, NB - b0)
                xt = prod.tile([F, TB * P], dt.float32, tag="xt")
                nc.sync.dma_start(out=xt[:, :nb * P],
                                  in_=xsT[:, b0 * P:(b0 + nb) * P])
                fsb4 = prod.tile([P, TB * TW], dt.bfloat16, tag="fsb4")
                nc.vector.memset(
                    fsb4[:].rearrange("p (blk w) -> p blk w", w=TW)
                    [:, :, ONEC:ONEC + 1], 1.0)
                for k in range(nb):
                    pmm = psumt.tile([P, 2 + H], dt.float32, tag="pmm")
                    nc.tensor.matmul(out=pmm[:], lhsT=xt[:, k * P:(k + 1) * P],
                                     rhs=W1a_sb[:], start=True, stop=True)
                    build_row(pmm, er_all, er2d1, b0 + k, fsb4, k)
                write_rows(feat1_s, b0, nb, fsb4)

            def allgather(src_t, dst_t):
                if single:
                    nc.sync.dma_start(
                        out=dst_t[0:NPCP, :].rearrange("a b -> (a b)"),
                        in_=src_t[:].rearrange("a b -> (a b)"))
                else:
                    nc.gpsimd.collective_compute(
                        "AllGather", mybir.AluOpType.bypass,
                        replica_groups=[list(range(NCORES))],
                        ins=[src_t[:]], outs=[dst_t[:]],
                    )

            allgather(feat1_s, feat1_f)
            emit_er2d(er2d1, er_all)

            # ---- edge phase ----
            def edge_phase(feat_f, er2d, bias_sb, relu, out_writer):
                for g, gr in enumerate(groups):
                    gp0, nchp = gr["gp0"], gr["nchp"]
                    gv0, nchv = gr["gv0"], gr["nchv"]
                    fxg = fxpool.tile([P, nchp * 8], dt.int16, tag="fxg")
                    nc.sync.dma_start(out=fxg[:],
                                      in_=fidx[:, gp0 * 8:(gp0 + nchp) * 8])
                    mhg = mgpool.tile([16, nchv * P], dt.float8e4, tag="mhg")
                    nc.sync.dma_start(out=mhg[:],
                                      in_=mhi[:, gv0 * P:(gv0 + nchv) * P])
                    log = mgpool.tile([P, nchv * 8], dt.bfloat16, tag="log")
                    nc.sync.dma_start(out=log[:],
                                      in_=lo8[:, gv0 * 8:(gv0 + nchv) * 8])
                    tvg = tvpool.tile([P, nchp * TW], dt.bfloat16, tag="tvg")
                    tv3 = tvg[:].rearrange("p (r e) -> p r e", e=TW)
                    for (s, c0, take) in gr["ops"]:
                        seg_lo = s * SEG
                        seg_hi = min(seg_lo + SEG, TROWS)
                        nc.gpsimd.dma_gather(
                            out_ap=tv3[:, c0 - gp0:c0 - gp0 + take, :],
                            in_ap=feat_f[seg_lo:seg_hi, :],
                            idxs_ap=fxg[:, (c0 - gp0) * 8:
                                        (c0 - gp0 + take) * 8],
                            num_idxs=take * P, num_idxs_reg=take * P,
                            elem_size=TW, queue_num=qn(),
                        )
                    def emit_er(b):
                        # er select stage 1: hi via fp8 [16,128] matmuls ->
                        # [128,8] candidate rows per chunk (emitted one block
                        # ahead so PE's in-order queue can't stall DVE).
                        runs = gr["blocks"][b]
                        Rb = gr["nv"][b]
                        er_ps = psume.tile([P, Rb * 8], dt.float32,
                                           tag="erps")
                        for (s, pcol0, ncols, vcol0, j0, _o) in runs:
                            for rr in range(ncols):
                                gc = vcol0 - gv0 + rr
                                nc.tensor.matmul(
                                    out=er_ps[:, (j0 + rr) * 8:
                                              (j0 + rr + 1) * 8],
                                    lhsT=mhg[:, gc * P:(gc + 1) * P],
                                    rhs=er2d[:].rearrange(
                                        "k (l b2) -> k l b2", b2=NB)
                                        [:, :, b],
                                    start=True, stop=True)
                        return er_ps

                    bl = sorted(gr["blocks"])
                    er_tiles = {bl[0]: emit_er(bl[0])}
                    for bi, b in enumerate(bl):
                        if bi + 1 < len(bl):
                            er_tiles[bl[bi + 1]] = emit_er(bl[bi + 1])
                        runs = gr["blocks"][b]  # (s, pcol0, ncols, vcol0, j0)
                        Rb = gr["nv"][b]
                        er_ps = er_tiles.pop(b)
                        sel8 = eepool.tile([P, Rb * 8], dt.bfloat16,
                                           tag="sel8")
                        for (s, pcol0, ncols, vcol0, j0, _o) in runs:
                            nc.vector.tensor_tensor(
                                out=sel8[:, j0 * 8:(j0 + ncols) * 8],
                                in0=er_ps[:, j0 * 8:(j0 + ncols) * 8],
                                in1=log[:, (vcol0 - gv0) * 8:
                                        (vcol0 - gv0 + ncols) * 8],
                                op=mybir.AluOpType.mult)
                        ers = eepool.tile([P, Rb], dt.float32, tag="ers")
                        nc.vector.tensor_reduce(
                            out=ers[:],
                            in_=sel8[:].rearrange("p (r e) -> p r e", e=8),
                            axis=mybir.AxisListType.X,
                            op=mybir.AluOpType.add)
                        # ee = el + er ; lrelu ; ex
                        ee = eepool.tile([P, Rb], dt.float32, tag="ee")
                        for (s, pcol0, ncols, vcol0, j0, _o) in runs:
                            el = tv3[:, pcol0 - gp0:pcol0 - gp0 + ncols,
                                     ELC:ELC + 2].bitcast(dt.float32)
                            nc.vector.tensor_tensor(
                                out=ee[:, j0:j0 + ncols]
                                    .rearrange("p (r o) -> p r o", o=1),
                                in0=el,
                                in1=ers[:, j0:j0 + ncols]
                                    .rearrange("p (r o) -> p r o", o=1),
                                op=mybir.AluOpType.add)
                        nc.vector.scalar_tensor_tensor(
                            out=ee[:], in0=ee[:], scalar=0.2, in1=ee[:],
                            op0=mybir.AluOpType.mult,
                            op1=mybir.AluOpType.max)
                        ex = eepool.tile([P, Rb], dt.float32, tag="ex")
                        nc.scalar.activation(
                            out=ex[:], in_=ee[:],
                            func=mybir.ActivationFunctionType.Exp)
                        # fused one-hot(+ex) per chunk; rhs = table slice
                        pacc = psum.tile([P, WFW], dt.float32, tag="pacc")
                        done = 0
                        for (s, pcol0, ncols, vcol0, j0, _o) in runs:
                            for rr in range(ncols):
                                oh = ohpool.tile([P, P], dt.bfloat16,
                                                 tag="oh")
                                nc.vector.tensor_scalar(
                                    out=oh[:], in0=iota_sb[:],
                                    scalar1=dstl_sb[:, vcol0 + rr:
                                                    vcol0 + rr + 1],
                                    scalar2=ex[:, j0 + rr:j0 + rr + 1],
                                    op0=mybir.AluOpType.is_equal,
                                    op1=mybir.AluOpType.mult,
                                )
                                nc.tensor.matmul(
                                    out=pacc[:],
                                    lhsT=oh[:],
                                    rhs=tv3[:, pcol0 - gp0 + rr,
                                            ONEC:ONEC + WFW],
                                    start=(done == 0), stop=(done == Rb - 1))
                                done += 1
                        # epilogue
                        den = epipool.tile([P, 1], dt.float32, tag="den")
                        nc.vector.tensor_scalar_add(out=den[:],
                                                    in0=pacc[:, 0:1],
                                                    scalar1=1e-30)
                        rec = epipool.tile([P, 1], dt.float32, tag="rec")
                        nc.vector.reciprocal(out=rec[:], in_=den[:])
                        h = epipool.tile([P, H], dt.float32, tag="h")
                        nc.vector.scalar_tensor_tensor(
                            out=h[:], in0=pacc[:, 1:], scalar=rec[:],
                            in1=bias_sb[:],
                            op0=mybir.AluOpType.mult,
                            op1=mybir.AluOpType.add)
                        if relu:
                            nc.scalar.activation(
                                out=h[:], in_=h[:],
                                func=mybir.ActivationFunctionType.Relu)
                        out_writer(b, h)

            ident = const.tile([P, P], dt.float32)
            from concourse.masks import make_identity
            make_identity(nc, ident[:])

            l1_state = {}

            def l1_writer(b, h):
                pt = psumtt.tile([H, P], dt.float32, tag="pt")
                nc.tensor.transpose(out=pt[:], in_=h[:], identity=ident[:])
                hT = prod.tile([H, P], dt.float32, tag="hT")
                nc.vector.tensor_copy(out=hT[:], in_=pt[:])
                pmm2 = psumt.tile([P, 2 + H], dt.float32, tag="pmm")
                nc.tensor.matmul(out=pmm2[:], lhsT=hT[:], rhs=W2a_sb[:],
                                 start=True, stop=True)
                k = b % TB
                if k == 0:
                    fsb4b = prod.tile([P, TB * TW], dt.bfloat16,
                                      tag="fsb4")
                    nc.vector.memset(
                        fsb4b[:].rearrange("p (blk w) -> p blk w", w=TW)
                        [:, :, ONEC:ONEC + 1], 1.0)
                    l1_state["fsb4"] = fsb4b
                build_row(pmm2, er_all2, er2d2, b, l1_state["fsb4"], k)
                if k == TB - 1 or b == NB - 1:
                    write_rows(feat2_s, b - k, k + 1, l1_state["fsb4"])
                if b == NB - 1:
                    emit_er2d(er2d2, er_all2)

            edge_phase(feat1_f, er2d1, b1r_sb, True, l1_writer)
            allgather(feat2_s, feat2_f)

            def l2_writer(b, h):
                rows = LB if b == NB - 1 else P
                nc.sync.dma_start(out=out_ext[b * P:b * P + rows, :],
                                  in_=h[:rows, :])

            edge_phase(feat2_f, er2d2, b2r_sb, False, l2_writer)

    nc.compile()
    return nc


def _get_program(R_key, single=False):
    key = ("prog", R_key, single)
    if key not in _cache:
        _cache[key] = _build_program(R_key, single=single)
    return _cache[key]


def kernel(x, src, dst, W1, al1, ar1, b1, W2, al2, ar2, b2):
    from concourse.bass_utils import run_bass_kernel_spmd

    in_maps, R_key = _host_prep(x, src, dst, W1, al1, ar1, b1,
                                W2, al2, ar2, b2)
    nc = _get_program(R_key)
    res = run_bass_kernel_spmd(nc, in_maps, list(range(NCORES)))
    out = np.concatenate([res.results[c]["out"] for c in range(NCORES)],
                         axis=0)
    return out.astype(np.float32)


# revision 29
# speedup vs baseline: 1.2391x; 1.0194x over previous
"""GAT (2-layer, single-head) Trainium2 Bass kernel, 8-core SPMD. v2.

Design (vs v1 baseline):
  - dst nodes 1D-sharded (12500/core); edges routed to dst core, bucketed
    by (dst block of 128, src segment of 32768 table rows). Within each
    (group of G blocks, segment) run, buckets pack CONTIGUOUSLY into
    128-slot columns (no per-bucket rounding); a column shared by two
    blocks is processed once per block through its own VIEW column of the
    dstl/MHI/LO8 masks (foreign slots masked out); buckets whose tail gap
    to the next column boundary is < PADT slots are padded instead
    (descriptors are cheaper than an extra view there). One dma_gather op
    (<=1024 idxs, ucode cap) spans buckets within a run.
  - Table row = 128 bf16 (256B, the dma_gather minimum elem):
    [el as f32 (2 slots) | 1.0 | feat(32) | pad]. el kept f32 for softmax
    accuracy; feat bf16.
  - NO per-edge er gather (v1 spent ~50% of its descriptors on it).
    er[dst] is selected on-chip in two levels (dl = 8*hi + lo): per
    chunk a host-shipped fp8 hi-one-hot MHI [16,128] is matmul'd with
    the block's er table redistributed to [16,8] fp16, giving [128,8]
    candidates in PSUM; a host-shipped bf16 lo-one-hot mask [128,8] and
    an X-axis reduce pick the final er per slot. 16B+128B of mask bytes
    per chunk replace 256B of gathered bytes per EDGE.
  - aggregation per chunk: one-hot(+ex) built in ONE fused tensor_scalar
    (is_equal, mult) on bf16 iota (4x DVE mode, ~92ns) with ex and dl as
    per-partition scalars; matmul rhs reads the gathered table slice
    [1|feat] directly; denominator accumulates via the "1" column.
  - Epilogue: out = pacc[:,1:]/pacc[:,0] + bias (+relu), layer-2 table
    built inline; AllGather between layers (excluded from the metric, a
    local copy in the single-core cost program).
"""

import numpy as np
import ml_dtypes

N = 100000
E = 1600000
F = 128
H = 32
NCORES = 8
NPC = N // NCORES          # nodes per core
P = 128
NB = (NPC + P - 1) // P    # dst blocks per core (98; last block 84 rows)
LB = NPC - (NB - 1) * P    # rows in last block
NPCP = NB * P              # padded nodes per core (table rows per core)
TROWS = NCORES * NPCP      # full table rows
TW = 128                   # table row: 128 bf16 = 256B
ELC = 0                    # cols 0-1: el as f32
ONEC = 2                   # col 2: 1.0
FEATC = 3                  # cols 3..34: feat
WFW = 1 + H                # aggregation rhs width: [1 | feat]
SEG = 32768                # src segment size (int16 gather indices)
NSEG = (TROWS + SEG - 1) // SEG
G = 8                      # dst blocks per group tile
NG = (NB + G - 1) // G
CAPC = 8                   # max chunks (1024 indices) per dma_gather op
PADT = 48                  # pad bucket tail to column boundary if gap < PADT

_cache = {}


def _plan(S):
    """Column layout. S: [NB, NSEG] slot counts per bucket (max over cores,
    NOT rounded to 128). Within each (group, seg) run, buckets pack
    contiguously; physical columns = ceil(run_slots/128); a column shared
    by two blocks is processed once per block through its own VIEW column
    of the dstl/MHI/LO8 masks (foreign slots masked to 200/zero).

    groups[g]: gp0/nchp (physical cols), gv0/nchv (view cols),
      ops: (s, pcol0, take<=8) gather ops,
      runbase: {s: run base pcol},
      blocks: {b: [(s, pcol0, ncols, vcol0, j0)]}, nv: {b: Vb}.
    """
    groups = []
    cp = 0
    cv = 0
    for g in range(NG):
        bs = list(range(g * G, min((g + 1) * G, NB)))
        gp0, gv0 = cp, cv
        ops = []
        runbase = {}
        blocks = {b: [] for b in bs}
        jloc = {b: 0 for b in bs}
        for s in range(NSEG):
            run_slots = sum(int(S[b, s]) for b in bs)
            if run_slots == 0:
                continue
            runbase[s] = cp
            op_mark = len(ops)
            o = 0
            for b in bs:
                sb = int(S[b, s])
                if sb == 0:
                    continue
                c_lo = o // 128
                c_hi = (o + sb - 1) // 128
                ncols = c_hi - c_lo + 1
                blocks[b].append((s, cp + c_lo, ncols, cv, jloc[b], o))
                jloc[b] += ncols
                cv += ncols
                o += sb
                gap = (-o) % 128
                if 0 < gap < PADT:
                    o += gap
            cols = -(-o // 128)
            # re-emit ops with the padded column count
            del ops[op_mark:]
            off = 0
            while off < cols:
                take = min(CAPC, cols - off)
                ops.append((s, cp + off, take))
                off += take
            cp += cols
        groups.append({"gp0": gp0, "nchp": cp - gp0,
                       "gv0": gv0, "nchv": cv - gv0,
                       "ops": ops, "runbase": runbase,
                       "blocks": blocks, "nv": dict(jloc)})
    return cp, cv, groups


def _host_prep(x, src, dst, W1, al1, ar1, b1, W2, al2, ar2, b2):
    f32, bf16, i16 = np.float32, ml_dtypes.bfloat16, np.int16
    src = np.asarray(src).astype(np.int64)
    dst = np.asarray(dst).astype(np.int64)

    core = dst // NPC
    r = dst % NPC
    b = r // P
    dl = r % P
    loc = src % NPC
    trow_src = (src // NPC) * NPCP + (loc % P) * NB + (loc // P)
    seg = trow_src // SEG
    bgid = core * NB + b

    key = bgid * NSEG + seg
    counts = np.bincount(key, minlength=NCORES * NB * NSEG)
    counts3 = counts.reshape(NCORES, NB, NSEG)
    S = counts3.max(axis=0)                   # [NB, NSEG] slots per bucket
    CHP, CHV, groups = _plan(S)

    order = np.argsort(key, kind="stable")
    s_trow = trow_src[order]
    s_seg = seg[order]
    s_b = b[order]
    s_bgid = bgid[order]
    s_dl = dl[order]

    seg_start = np.concatenate([[0], np.cumsum(counts)])[:-1]
    rank = np.arange(len(order), dtype=np.int64) - seg_start[key[order]]

    runbase_tab = np.zeros((NB, NSEG), np.int64)  # (g,s) run base pcol
    off_tab = np.zeros((NB, NSEG), np.int64)      # bucket slot offset in run
    vcol_tab = np.zeros((NB, NSEG), np.int64)     # bucket first view col
    pcol_tab = np.zeros((NB, NSEG), np.int64)     # bucket first phys col
    for gr in groups:
        for bb, runs in gr["blocks"].items():
            for (s, pcol0, ncols, vcol0, j0, _o) in runs:
                runbase_tab[bb, s] = gr["runbase"][s]
                pcol_tab[bb, s] = pcol0
                vcol_tab[bb, s] = vcol0
                off_tab[bb, s] = _o

    slot = off_tab[s_b, s_seg] + rank
    pcol = runbase_tab[s_b, s_seg] + slot // P
    p = slot % P
    vcol = vcol_tab[s_b, s_seg] + (pcol - pcol_tab[s_b, s_seg])

    # dstl: [128, CHV] destination-local row per (slot, VIEW col); 200 = pad
    dstl = np.full((NCORES, P, CHV), 200.0, f32)
    flat = (s_bgid // NB) * (P * CHV) + p * CHV + vcol
    dstl.reshape(-1)[flat] = s_dl

    # gather indices: [16 wrap, CHP*8], segment-relative; ops start at
    # 8-column boundaries from each run base
    fidx = np.zeros((NCORES, 16, CHP * 8), i16)
    rb = runbase_tab[s_b, s_seg]
    opbase = rb + ((pcol - rb) // CAPC) * CAPC
    i_op = (pcol - opbase) * P + p
    row16 = i_op % 16
    col16 = i_op // 16
    abscol = opbase * 8 + col16
    fflat = (s_bgid // NB) * (16 * CHP * 8) + row16 * (CHP * 8) + abscol
    fidx.reshape(-1)[fflat] = (s_trow - s_seg * SEG).astype(i16)
    fidx = np.tile(fidx, (1, 8, 1))

    # two-level er-select masks: dl = 8*hi + lo
    #   MHI fp8 [16, CH*128]: MHI[k, c*128+i] = (dstl[i,c]//8 == k)
    #   LO8 bf16 [128, CH*8]: LO8[i, c*8+l] = (dstl[i,c]%8 == l) & valid
    one8 = np.float32(1.0).astype(ml_dtypes.float8_e4m3).view(np.uint8)
    MHI = np.zeros((NCORES, 32, CHV * P), np.uint8)
    LO8 = np.zeros((NCORES, P, CHV * 4), bf16)
    for cc in range(NCORES):
        d = dstl[cc].astype(np.int64)         # [P(slots), CH]
        hi = (d // 4).T                       # [CH, i]; pad 200//4=50
        eqh = hi[:, :, None] == np.arange(32)[None, None, :]
        MHI[cc][np.transpose(eqh, (2, 0, 1)).reshape(32, CHV * P)] = one8
        lo = (d % 4).T
        valid = (d < P).T
        eql = (lo[:, :, None] == np.arange(4)[None, None, :]) \
            & valid[:, :, None]               # [CH, i, 4]
        LO8[cc][np.transpose(eql, (1, 0, 2)).reshape(P, CHV * 4)] = bf16(1.0)
    MHI = MHI.view(ml_dtypes.float8_e4m3)

    def aug(W, al, ar):
        W = np.asarray(W, f32)
        Wa = np.zeros((W.shape[0], 2 + H), f32)
        Wa[:, 0] = W @ np.asarray(al, f32)
        Wa[:, 1:1 + H] = W
        Wa[:, 1 + H] = W @ np.asarray(ar, f32)
        return Wa

    W1a = aug(W1, al1, ar1)
    W2a = aug(W2, al2, ar2)
    b1r = np.tile(np.asarray(b1, f32)[None, :], (P, 1))
    b2r = np.tile(np.asarray(b2, f32)[None, :], (P, 1))
    iota = np.tile(np.arange(P, dtype=bf16)[None, :], (P, 1))

    x = np.asarray(x, f32)
    xsT = np.zeros((NCORES, F, NPCP), f32)
    for cc in range(NCORES):
        xsT[cc, :, :NPC] = x[cc * NPC:(cc + 1) * NPC].T

    in_maps = []
    for cc in range(NCORES):
        in_maps.append({
            "xsT": xsT[cc],
            "W1a": W1a, "W2a": W2a, "b1r": b1r, "b2r": b2r, "iota": iota,
            "fidx": fidx[cc], "dstl": dstl[cc],
            "mhi": MHI[cc], "lo8": LO8[cc],
        })
    return in_maps, tuple(int(v) for v in S.reshape(-1))


def _build_program(R_key, single=False):
    import concourse.bacc as bacc
    import concourse.mybir as mybir
    import concourse.tile as tile

    dt = mybir.dt
    S = np.asarray(R_key, np.int64).reshape(NB, NSEG)
    CHP, CHV, groups = _plan(S)
    ncores = 1 if single else NCORES

    nc = bacc.Bacc("TRN2", target_bir_lowering=False, debug=False,
                   num_devices=ncores, num_swdge_queues=4)

    xsT = nc.dram_tensor("xsT", [F, NPCP], dt.float32, kind="ExternalInput")
    W1a = nc.dram_tensor("W1a", [F, 2 + H], dt.float32, kind="ExternalInput")
    W2a = nc.dram_tensor("W2a", [H, 2 + H], dt.float32, kind="ExternalInput")
    b1r = nc.dram_tensor("b1r", [P, H], dt.float32, kind="ExternalInput")
    b2r = nc.dram_tensor("b2r", [P, H], dt.float32, kind="ExternalInput")
    iota = nc.dram_tensor("iota", [P, P], dt.bfloat16, kind="ExternalInput")
    fidx = nc.dram_tensor("fidx", [P, CHP * 8], dt.int16, kind="ExternalInput")
    dstl = nc.dram_tensor("dstl", [P, CHV], dt.float32, kind="ExternalInput")
    mhi = nc.dram_tensor("mhi", [32, CHV * P], dt.float8e4, kind="ExternalInput")
    lo8 = nc.dram_tensor("lo8", [P, CHV * 4], dt.bfloat16, kind="ExternalInput")
    out_ext = nc.dram_tensor("out", [NPC, H], dt.float32, kind="ExternalOutput")

    qn_state = [0]

    def qn():
        qn_state[0] = (qn_state[0] + 1) % 4
        return qn_state[0]

    with tile.TileContext(nc) as tc:
        with (
            tc.tile_pool(name="const", bufs=1) as const,
            tc.tile_pool(name="prod", bufs=4) as prod,
            tc.tile_pool(name="tv", bufs=2) as tvpool,
            tc.tile_pool(name="mg", bufs=2) as mgpool,
            tc.tile_pool(name="fxg", bufs=2) as fxpool,
            tc.tile_pool(name="oh", bufs=12) as ohpool,
            tc.tile_pool(name="ee", bufs=4) as eepool,
            tc.tile_pool(name="epi", bufs=4) as epipool,
            tc.tile_pool(name="ps", bufs=3, space="PSUM") as psum,
            tc.tile_pool(name="pse", bufs=2, space="PSUM") as psume,
            tc.tile_pool(name="pst", bufs=2, space="PSUM") as psumt,
            tc.tile_pool(name="ptt", bufs=1, space="PSUM") as psumtt,
            tc.tile_pool(name="dram", bufs=1, space="DRAM") as dram,
        ):
            iota_sb = const.tile([P, P], dt.bfloat16)
            nc.sync.dma_start(out=iota_sb[:], in_=iota[:])
            W1a_sb = const.tile([F, 2 + H], dt.float32)
            nc.sync.dma_start(out=W1a_sb[:], in_=W1a[:])
            W2a_sb = const.tile([H, 2 + H], dt.float32)
            nc.sync.dma_start(out=W2a_sb[:], in_=W2a[:])
            b1r_sb = const.tile([P, H], dt.float32)
            nc.sync.dma_start(out=b1r_sb[:], in_=b1r[:])
            b2r_sb = const.tile([P, H], dt.float32)
            nc.sync.dma_start(out=b2r_sb[:], in_=b2r[:])
            dstl_sb = const.tile([P, CHV], dt.float32)
            nc.sync.dma_start(out=dstl_sb[:], in_=dstl[:])
            er_all = const.tile([P, NB], dt.float16)
            er_all2 = const.tile([P, NB], dt.float16)
            er2d1 = const.tile([32, NB * 4], dt.float16)
            er2d2 = const.tile([32, NB * 4], dt.float16)

            feat1_s = dram.tile([NPCP, TW], dt.bfloat16)
            feat1_f = dram.tile([TROWS, TW], dt.bfloat16, addr_space="Shared")
            feat2_s = dram.tile([NPCP, TW], dt.bfloat16)
            feat2_f = dram.tile([TROWS, TW], dt.bfloat16, addr_space="Shared")

            def build_row(pmm, er_dst, er2d, b, fsb4, k):
                """pmm [P, 2+H] f32 = [el | feat | er] -> bf16 row + er."""
                o = k * TW
                nc.vector.tensor_copy(
                    out=fsb4[:, o + ELC:o + ELC + 2].bitcast(dt.float32),
                    in_=pmm[:, 0:1])
                nc.vector.tensor_copy(out=fsb4[:, o + FEATC:o + FEATC + H],
                                      in_=pmm[:, 1:1 + H])
                nc.vector.tensor_copy(out=er_dst[:, b:b + 1],
                                      in_=pmm[:, 1 + H:2 + H])

            TB = 8

            def emit_er2d(er2d, er_dst):
                # l-major: er2d[k, l*NB + b] = er[8k+l of block b]
                nc.scalar.dma_start(
                    out=er2d[:].rearrange("k (l b) -> k l b", b=NB),
                    in_=er_dst[:])

            def write_rows(feat_s, b0, nb, fsb4):
                # permuted layout: node (b, p) lives at row p*NB + b, so a
                # TB-batch is contiguous nb*TW per partition (latmul-1 DMA)
                nc.sync.dma_start(
                    out=feat_s[0:NPCP, :]
                        .rearrange("(r blk) w -> r blk w", blk=NB)
                        [:, b0:b0 + nb, :],
                    in_=fsb4[:, :nb * TW].rearrange("p (blk w) -> p blk w",
                                                    w=TW))

            # ---- layer-1 table ----
            for b0 in range(0, NB, TB):
                nb = min(TB, NB - b0)
                xt = prod.tile([F, TB * P], dt.float32, tag="xt")
                nc.sync.dma_start(out=xt[:, :nb * P],
                                  in_=xsT[:, b0 * P:(b0 + nb) * P])
                fsb4 = prod.tile([P, TB * TW], dt.bfloat16, tag="fsb4")
                nc.vector.memset(
                    fsb4[:].rearrange("p (blk w) -> p blk w", w=TW)
                    [:, :, ONEC:ONEC + 1], 1.0)
                for k in range(nb):
                    pmm = psumt.tile([P, 2 + H], dt.float32, tag="pmm")
                    nc.tensor.matmul(out=pmm[:], lhsT=xt[:, k * P:(k + 1) * P],
                                     rhs=W1a_sb[:], start=True, stop=True)
                    build_row(pmm, er_all, er2d1, b0 + k, fsb4, k)
                write_rows(feat1_s, b0, nb, fsb4)

            def allgather(src_t, dst_t):
                if single:
                    nc.sync.dma_start(
                        out=dst_t[0:NPCP, :].rearrange("a b -> (a b)"),
                        in_=src_t[:].rearrange("a b -> (a b)"))
                else:
                    nc.gpsimd.collective_compute(
                        "AllGather", mybir.AluOpType.bypass,
                        replica_groups=[list(range(NCORES))],
                        ins=[src_t[:]], outs=[dst_t[:]],
                    )

            allgather(feat1_s, feat1_f)
            emit_er2d(er2d1, er_all)

            # ---- edge phase ----
            def edge_phase(feat_f, er2d, bias_sb, relu, out_writer):
                for g, gr in enumerate(groups):
                    gp0, nchp = gr["gp0"], gr["nchp"]
                    gv0, nchv = gr["gv0"], gr["nchv"]
                    fxg = fxpool.tile([P, nchp * 8], dt.int16, tag="fxg")
                    nc.sync.dma_start(out=fxg[:],
                                      in_=fidx[:, gp0 * 8:(gp0 + nchp) * 8])
                    mhg = mgpool.tile([32, nchv * P], dt.float8e4, tag="mhg")
                    nc.sync.dma_start(out=mhg[:],
                                      in_=mhi[:, gv0 * P:(gv0 + nchv) * P])
                    log = mgpool.tile([P, nchv * 4], dt.bfloat16, tag="log")
                    nc.sync.dma_start(out=log[:],
                                      in_=lo8[:, gv0 * 4:(gv0 + nchv) * 4])
                    tvg = tvpool.tile([P, nchp * TW], dt.bfloat16, tag="tvg")
                    tv3 = tvg[:].rearrange("p (r e) -> p r e", e=TW)
                    for (s, c0, take) in gr["ops"]:
                        seg_lo = s * SEG
                        seg_hi = min(seg_lo + SEG, TROWS)
                        nc.gpsimd.dma_gather(
                            out_ap=tv3[:, c0 - gp0:c0 - gp0 + take, :],
                            in_ap=feat_f[seg_lo:seg_hi, :],
                            idxs_ap=fxg[:, (c0 - gp0) * 8:
                                        (c0 - gp0 + take) * 8],
                            num_idxs=take * P, num_idxs_reg=take * P,
                            elem_size=TW, queue_num=qn(),
                        )
                    def emit_er(b):
                        # er select stage 1: hi via fp8 [16,128] matmuls ->
                        # [128,8] candidate rows per chunk (emitted one block
                        # ahead so PE's in-order queue can't stall DVE).
                        runs = gr["blocks"][b]
                        Rb = gr["nv"][b]
                        er_ps = psume.tile([P, Rb * 4], dt.float32,
                                           tag="erps")
                        for (s, pcol0, ncols, vcol0, j0, _o) in runs:
                            for rr in range(ncols):
                                gc = vcol0 - gv0 + rr
                                nc.tensor.matmul(
                                    out=er_ps[:, (j0 + rr) * 4:
                                              (j0 + rr + 1) * 4],
                                    lhsT=mhg[:, gc * P:(gc + 1) * P],
                                    rhs=er2d[:].rearrange(
                                        "k (l b2) -> k l b2", b2=NB)
                                        [:, :, b],
                                    start=True, stop=True)
                        return er_ps

                    bl = sorted(gr["blocks"])
                    er_tiles = {bl[0]: emit_er(bl[0])}
                    for bi, b in enumerate(bl):
                        if bi + 1 < len(bl):
                            er_tiles[bl[bi + 1]] = emit_er(bl[bi + 1])
                        runs = gr["blocks"][b]  # (s, pcol0, ncols, vcol0, j0)
                        Rb = gr["nv"][b]
                        er_ps = er_tiles.pop(b)
                        sel8 = eepool.tile([P, Rb * 4], dt.bfloat16,
                                           tag="sel8")
                        for (s, pcol0, ncols, vcol0, j0, _o) in runs:
                            nc.vector.tensor_tensor(
                                out=sel8[:, j0 * 4:(j0 + ncols) * 4],
                                in0=er_ps[:, j0 * 4:(j0 + ncols) * 4],
                                in1=log[:, (vcol0 - gv0) * 4:
                                        (vcol0 - gv0 + ncols) * 4],
                                op=mybir.AluOpType.mult)
                        ers = eepool.tile([P, Rb], dt.float32, tag="ers")
                        nc.vector.tensor_reduce(
                            out=ers[:],
                            in_=sel8[:].rearrange("p (r e) -> p r e", e=4),
                            axis=mybir.AxisListType.X,
                            op=mybir.AluOpType.add)
                        # ee = el + er ; lrelu ; ex
                        ee = eepool.tile([P, Rb], dt.float32, tag="ee")
                        for (s, pcol0, ncols, vcol0, j0, _o) in runs:
                            el = tv3[:, pcol0 - gp0:pcol0 - gp0 + ncols,
                                     ELC:ELC + 2].bitcast(dt.float32)
                            nc.vector.tensor_tensor(
                                out=ee[:, j0:j0 + ncols]
                                    .rearrange("p (r o) -> p r o", o=1),
                                in0=el,
                                in1=ers[:, j0:j0 + ncols]
                                    .rearrange("p (r o) -> p r o", o=1),
                                op=mybir.AluOpType.add)
                        nc.vector.scalar_tensor_tensor(
                            out=ee[:], in0=ee[:], scalar=0.2, in1=ee[:],
                            op0=mybir.AluOpType.mult,
                            op1=mybir.AluOpType.max)
                        ex = eepool.tile([P, Rb], dt.float32, tag="ex")
                        nc.scalar.activation(
                            out=ex[:], in_=ee[:],
                            func=mybir.ActivationFunctionType.Exp)
                        # fused one-hot(+ex) per chunk; rhs = table slice
                        pacc = psum.tile([P, WFW], dt.float32, tag="pacc")
                        done = 0
                        for (s, pcol0, ncols, vcol0, j0, _o) in runs:
                            for rr in range(ncols):
                                oh = ohpool.tile([P, P], dt.bfloat16,
                                                 tag="oh")
                                nc.vector.tensor_scalar(
                                    out=oh[:], in0=iota_sb[:],
                                    scalar1=dstl_sb[:, vcol0 + rr:
                                                    vcol0 + rr + 1],
                                    scalar2=ex[:, j0 + rr:j0 + rr + 1],
                                    op0=mybir.AluOpType.is_equal,
                                    op1=mybir.AluOpType.mult,
                                )
                                nc.tensor.matmul(
                                    out=pacc[:],
                                    lhsT=oh[:],
                                    rhs=tv3[:, pcol0 - gp0 + rr,
                                            ONEC:ONEC + WFW],
                                    start=(done == 0), stop=(done == Rb - 1))
                                done += 1
                        # epilogue
                        den = epipool.tile([P, 1], dt.float32, tag="den")
                        nc.vector.tensor_scalar_add(out=den[:],
                                                    in0=pacc[:, 0:1],
                                                    scalar1=1e-30)
                        rec = epipool.tile([P, 1], dt.float32, tag="rec")
                        nc.vector.reciprocal(out=rec[:], in_=den[:])
                        h = epipool.tile([P, H], dt.float32, tag="h")
                        nc.vector.scalar_tensor_tensor(
                            out=h[:], in0=pacc[:, 1:], scalar=rec[:],
                            in1=bias_sb[:],
                            op0=mybir.AluOpType.mult,
                            op1=mybir.AluOpType.add)
                        if relu:
                            nc.scalar.activation(
                                out=h[:], in_=h[:],
                                func=mybir.ActivationFunctionType.Relu)
                        out_writer(b, h)

            ident = const.tile([P, P], dt.float32)
            from concourse.masks import make_identity
            make_identity(nc, ident[:])

            l1_state = {}

            def l1_writer(b, h):
                pt = psumtt.tile([H, P], dt.float32, tag="pt")
                nc.tensor.transpose(out=pt[:], in_=h[:], identity=ident[:])
                hT = prod.tile([H, P], dt.float32, tag="hT")
                nc.vector.tensor_copy(out=hT[:], in_=pt[:])
                pmm2 = psumt.tile([P, 2 + H], dt.float32, tag="pmm")
                nc.tensor.matmul(out=pmm2[:], lhsT=hT[:], rhs=W2a_sb[:],
                                 start=True, stop=True)
                k = b % TB
                if k == 0:
                    fsb4b = prod.tile([P, TB * TW], dt.bfloat16,
                                      tag="fsb4")
                    nc.vector.memset(
                        fsb4b[:].rearrange("p (blk w) -> p blk w", w=TW)
                        [:, :, ONEC:ONEC + 1], 1.0)
                    l1_state["fsb4"] = fsb4b
                build_row(pmm2, er_all2, er2d2, b, l1_state["fsb4"], k)
                if k == TB - 1 or b == NB - 1:
                    write_rows(feat2_s, b - k, k + 1, l1_state["fsb4"])
                if b == NB - 1:
                    emit_er2d(er2d2, er_all2)

            edge_phase(feat1_f, er2d1, b1r_sb, True, l1_writer)
            allgather(feat2_s, feat2_f)

            def l2_writer(b, h):
                rows = LB if b == NB - 1 else P
                nc.sync.dma_start(out=out_ext[b * P:b * P + rows, :],
                                  in_=h[:rows, :])

            edge_phase(feat2_f, er2d2, b2r_sb, False, l2_writer)

    nc.compile()
    return nc


def _get_program(R_key, single=False):
    key = ("prog", R_key, single)
    if key not in _cache:
        _cache[key] = _build_program(R_key, single=single)
    return _cache[key]


def kernel(x, src, dst, W1, al1, ar1, b1, W2, al2, ar2, b2):
    from concourse.bass_utils import run_bass_kernel_spmd

    in_maps, R_key = _host_prep(x, src, dst, W1, al1, ar1, b1,
                                W2, al2, ar2, b2)
    nc = _get_program(R_key)
    res = run_bass_kernel_spmd(nc, in_maps, list(range(NCORES)))
    out = np.concatenate([res.results[c]["out"] for c in range(NCORES)],
                         axis=0)
    return out.astype(np.float32)


# revision 34
# speedup vs baseline: 1.2424x; 1.0026x over previous
"""GAT (2-layer, single-head) Trainium2 Bass kernel, 8-core SPMD. v2.

Design (vs v1 baseline):
  - dst nodes 1D-sharded (12500/core); edges routed to dst core, bucketed
    by (dst block of 128, src segment of 32768 table rows). Within each
    (group of G blocks, segment) run, buckets pack CONTIGUOUSLY into
    128-slot columns (no per-bucket rounding); a column shared by two
    blocks is processed once per block through its own VIEW column of the
    dstl/MHI/LO8 masks (foreign slots masked out); buckets whose tail gap
    to the next column boundary is < PADT slots are padded instead
    (descriptors are cheaper than an extra view there). One dma_gather op
    (<=1024 idxs, ucode cap) spans buckets within a run.
  - Table row = 128 bf16 (256B, the dma_gather minimum elem):
    [el as f32 (2 slots) | 1.0 | feat(32) | pad]. el kept f32 for softmax
    accuracy; feat bf16.
  - NO per-edge er gather (v1 spent ~50% of its descriptors on it).
    er[dst] is selected on-chip in two levels (dl = 8*hi + lo): per
    chunk a host-shipped fp8 hi-one-hot MHI [16,128] is matmul'd with
    the block's er table redistributed to [16,8] fp16, giving [128,8]
    candidates in PSUM; a host-shipped bf16 lo-one-hot mask [128,8] and
    an X-axis reduce pick the final er per slot. 16B+128B of mask bytes
    per chunk replace 256B of gathered bytes per EDGE.
  - aggregation per chunk: one-hot(+ex) built in ONE fused tensor_scalar
    (is_equal, mult) on bf16 iota (4x DVE mode, ~92ns) with ex and dl as
    per-partition scalars; matmul rhs reads the gathered table slice
    [1|feat] directly; denominator accumulates via the "1" column.
  - Epilogue: out = pacc[:,1:]/pacc[:,0] + bias (+relu), layer-2 table
    built inline; AllGather between layers (excluded from the metric, a
    local copy in the single-core cost program).
"""

import numpy as np
import ml_dtypes

N = 100000
E = 1600000
F = 128
H = 32
NCORES = 8
NPC = N // NCORES          # nodes per core
P = 128
NB = (NPC + P - 1) // P    # dst blocks per core (98; last block 84 rows)
LB = NPC - (NB - 1) * P    # rows in last block
NPCP = NB * P              # padded nodes per core (table rows per core)
TROWS = NCORES * NPCP      # full table rows
TW = 128                   # table row: 128 bf16 = 256B
ELC = 0                    # cols 0-1: el as f32
ONEC = 2                   # col 2: 1.0
FEATC = 3                  # cols 3..34: feat
WFW = 1 + H                # aggregation rhs width: [1 | feat]
SEG = 32768                # src segment size (int16 gather indices)
NSEG = (TROWS + SEG - 1) // SEG
G = 8                      # dst blocks per group tile
NG = (NB + G - 1) // G
CAPC = 8                   # max chunks (1024 indices) per dma_gather op
PADT = 48                  # pad bucket tail to column boundary if gap < PADT

_cache = {}


def _plan(S):
    """Column layout. S: [NB, NSEG] slot counts per bucket (max over cores,
    NOT rounded to 128). Within each (group, seg) run, buckets pack
    contiguously; physical columns = ceil(run_slots/128); a column shared
    by two blocks is processed once per block through its own VIEW column
    of the dstl/MHI/LO8 masks (foreign slots masked to 200/zero).

    groups[g]: gp0/nchp (physical cols), gv0/nchv (view cols),
      ops: (s, pcol0, take<=8) gather ops,
      runbase: {s: run base pcol},
      blocks: {b: [(s, pcol0, ncols, vcol0, j0)]}, nv: {b: Vb}.
    """
    sizes = [8] * 12 + [2]                    # 12*8 + 2 = 98 = NB
    assert sum(sizes) == NB
    starts = [sum(sizes[:i]) for i in range(len(sizes))]
    groups = []
    cp = 0
    cv = 0
    for g in range(len(sizes)):
        bs = list(range(starts[g], starts[g] + sizes[g]))
        gp0, gv0 = cp, cv
        ops = []
        runbase = {}
        blocks = {b: [] for b in bs}
        jloc = {b: 0 for b in bs}
        for s in range(NSEG):
            run_slots = sum(int(S[b, s]) for b in bs)
            if run_slots == 0:
                continue
            runbase[s] = cp
            op_mark = len(ops)
            o = 0
            for b in bs:
                sb = int(S[b, s])
                if sb == 0:
                    continue
                c_lo = o // 128
                c_hi = (o + sb - 1) // 128
                ncols = c_hi - c_lo + 1
                blocks[b].append((s, cp + c_lo, ncols, cv, jloc[b], o))
                jloc[b] += ncols
                cv += ncols
                o += sb
                gap = (-o) % 128
                if 0 < gap < PADT:
                    o += gap
            cols = -(-o // 128)
            # re-emit ops with the padded column count
            del ops[op_mark:]
            off = 0
            while off < cols:
                take = min(CAPC, cols - off)
                ops.append((s, cp + off, take))
                off += take
            cp += cols
        groups.append({"gp0": gp0, "nchp": cp - gp0,
                       "gv0": gv0, "nchv": cv - gv0,
                       "ops": ops, "runbase": runbase,
                       "blocks": blocks, "nv": dict(jloc)})
    return cp, cv, groups


def _host_prep(x, src, dst, W1, al1, ar1, b1, W2, al2, ar2, b2):
    f32, bf16, i16 = np.float32, ml_dtypes.bfloat16, np.int16
    src = np.asarray(src).astype(np.int64)
    dst = np.asarray(dst).astype(np.int64)

    core = dst // NPC
    r = dst % NPC
    b = r // P
    dl = r % P
    loc = src % NPC
    trow_src = (src // NPC) * NPCP + (loc % P) * NB + (loc // P)
    seg = trow_src // SEG
    bgid = core * NB + b

    key = bgid * NSEG + seg
    counts = np.bincount(key, minlength=NCORES * NB * NSEG)
    counts3 = counts.reshape(NCORES, NB, NSEG)
    S = counts3.max(axis=0)                   # [NB, NSEG] slots per bucket
    CHP, CHV, groups = _plan(S)

    order = np.argsort(key, kind="stable")
    s_trow = trow_src[order]
    s_seg = seg[order]
    s_b = b[order]
    s_bgid = bgid[order]
    s_dl = dl[order]

    seg_start = np.concatenate([[0], np.cumsum(counts)])[:-1]
    rank = np.arange(len(order), dtype=np.int64) - seg_start[key[order]]

    runbase_tab = np.zeros((NB, NSEG), np.int64)  # (g,s) run base pcol
    off_tab = np.zeros((NB, NSEG), np.int64)      # bucket slot offset in run
    vcol_tab = np.zeros((NB, NSEG), np.int64)     # bucket first view col
    pcol_tab = np.zeros((NB, NSEG), np.int64)     # bucket first phys col
    for gr in groups:
        for bb, runs in gr["blocks"].items():
            for (s, pcol0, ncols, vcol0, j0, _o) in runs:
                runbase_tab[bb, s] = gr["runbase"][s]
                pcol_tab[bb, s] = pcol0
                vcol_tab[bb, s] = vcol0
                off_tab[bb, s] = _o

    slot = off_tab[s_b, s_seg] + rank
    pcol = runbase_tab[s_b, s_seg] + slot // P
    p = slot % P
    vcol = vcol_tab[s_b, s_seg] + (pcol - pcol_tab[s_b, s_seg])

    # dstl: [128, CHV] destination-local row per (slot, VIEW col); 200 = pad
    dstl = np.full((NCORES, P, CHV), 200.0, f32)
    flat = (s_bgid // NB) * (P * CHV) + p * CHV + vcol
    dstl.reshape(-1)[flat] = s_dl

    # gather indices: [16 wrap, CHP*8], segment-relative; ops start at
    # 8-column boundaries from each run base
    fidx = np.zeros((NCORES, 16, CHP * 8), i16)
    rb = runbase_tab[s_b, s_seg]
    opbase = rb + ((pcol - rb) // CAPC) * CAPC
    i_op = (pcol - opbase) * P + p
    row16 = i_op % 16
    col16 = i_op // 16
    abscol = opbase * 8 + col16
    fflat = (s_bgid // NB) * (16 * CHP * 8) + row16 * (CHP * 8) + abscol
    fidx.reshape(-1)[fflat] = (s_trow - s_seg * SEG).astype(i16)
    fidx = np.tile(fidx, (1, 8, 1))

    # two-level er-select masks: dl = 8*hi + lo
    #   MHI fp8 [16, CH*128]: MHI[k, c*128+i] = (dstl[i,c]//8 == k)
    #   LO8 bf16 [128, CH*8]: LO8[i, c*8+l] = (dstl[i,c]%8 == l) & valid
    one8 = np.float32(1.0).astype(ml_dtypes.float8_e4m3).view(np.uint8)
    MHI = np.zeros((NCORES, 32, CHV * P), np.uint8)
    LO8 = np.zeros((NCORES, P, CHV * 4), bf16)
    for cc in range(NCORES):
        d = dstl[cc].astype(np.int64)         # [P(slots), CH]
        hi = (d // 4).T                       # [CH, i]; pad 200//4=50
        eqh = hi[:, :, None] == np.arange(32)[None, None, :]
        MHI[cc][np.transpose(eqh, (2, 0, 1)).reshape(32, CHV * P)] = one8
        lo = (d % 4).T
        valid = (d < P).T
        eql = (lo[:, :, None] == np.arange(4)[None, None, :]) \
            & valid[:, :, None]               # [CH, i, 4]
        LO8[cc][np.transpose(eql, (1, 0, 2)).reshape(P, CHV * 4)] = bf16(1.0)
    MHI = MHI.view(ml_dtypes.float8_e4m3)

    def aug(W, al, ar):
        W = np.asarray(W, f32)
        Wa = np.zeros((W.shape[0], 2 + H), f32)
        Wa[:, 0] = W @ np.asarray(al, f32)
        Wa[:, 1:1 + H] = W
        Wa[:, 1 + H] = W @ np.asarray(ar, f32)
        return Wa

    W1a = aug(W1, al1, ar1)
    W2a = aug(W2, al2, ar2)
    b1r = np.tile(np.asarray(b1, f32)[None, :], (P, 1))
    b2r = np.tile(np.asarray(b2, f32)[None, :], (P, 1))
    iota = np.tile(np.arange(P, dtype=bf16)[None, :], (P, 1))

    x = np.asarray(x, f32)
    xsT = np.zeros((NCORES, F, NPCP), f32)
    for cc in range(NCORES):
        xsT[cc, :, :NPC] = x[cc * NPC:(cc + 1) * NPC].T

    in_maps = []
    for cc in range(NCORES):
        in_maps.append({
            "xsT": xsT[cc],
            "W1a": W1a, "W2a": W2a, "b1r": b1r, "b2r": b2r, "iota": iota,
            "fidx": fidx[cc], "dstl": dstl[cc],
            "mhi": MHI[cc], "lo8": LO8[cc],
        })
    return in_maps, tuple(int(v) for v in S.reshape(-1))


def _build_program(R_key, single=False):
    import concourse.bacc as bacc
    import concourse.mybir as mybir
    import concourse.tile as tile

    dt = mybir.dt
    S = np.asarray(R_key, np.int64).reshape(NB, NSEG)
    CHP, CHV, groups = _plan(S)
    ncores = 1 if single else NCORES

    nc = bacc.Bacc("TRN2", target_bir_lowering=False, debug=False,
                   num_devices=ncores, num_swdge_queues=4)

    xsT = nc.dram_tensor("xsT", [F, NPCP], dt.float32, kind="ExternalInput")
    W1a = nc.dram_tensor("W1a", [F, 2 + H], dt.float32, kind="ExternalInput")
    W2a = nc.dram_tensor("W2a", [H, 2 + H], dt.float32, kind="ExternalInput")
    b1r = nc.dram_tensor("b1r", [P, H], dt.float32, kind="ExternalInput")
    b2r = nc.dram_tensor("b2r", [P, H], dt.float32, kind="ExternalInput")
    iota = nc.dram_tensor("iota", [P, P], dt.bfloat16, kind="ExternalInput")
    fidx = nc.dram_tensor("fidx", [P, CHP * 8], dt.int16, kind="ExternalInput")
    dstl = nc.dram_tensor("dstl", [P, CHV], dt.float32, kind="ExternalInput")
    mhi = nc.dram_tensor("mhi", [32, CHV * P], dt.float8e4, kind="ExternalInput")
    lo8 = nc.dram_tensor("lo8", [P, CHV * 4], dt.bfloat16, kind="ExternalInput")
    out_ext = nc.dram_tensor("out", [NPC, H], dt.float32, kind="ExternalOutput")

    qn_state = [0]

    def qn():
        qn_state[0] = (qn_state[0] + 1) % 4
        return qn_state[0]

    with tile.TileContext(nc) as tc:
        with (
            tc.tile_pool(name="const", bufs=1) as const,
            tc.tile_pool(name="prod", bufs=4) as prod,
            tc.tile_pool(name="tv", bufs=2) as tvpool,
            tc.tile_pool(name="mg", bufs=2) as mgpool,
            tc.tile_pool(name="fxg", bufs=2) as fxpool,
            tc.tile_pool(name="oh", bufs=12) as ohpool,
            tc.tile_pool(name="ee", bufs=4) as eepool,
            tc.tile_pool(name="epi", bufs=4) as epipool,
            tc.tile_pool(name="ps", bufs=3, space="PSUM") as psum,
            tc.tile_pool(name="pse", bufs=2, space="PSUM") as psume,
            tc.tile_pool(name="pst", bufs=2, space="PSUM") as psumt,
            tc.tile_pool(name="ptt", bufs=1, space="PSUM") as psumtt,
            tc.tile_pool(name="dram", bufs=1, space="DRAM") as dram,
        ):
            iota_sb = const.tile([P, P], dt.bfloat16)
            nc.sync.dma_start(out=iota_sb[:], in_=iota[:])
            W1a_sb = const.tile([F, 2 + H], dt.float32)
            nc.sync.dma_start(out=W1a_sb[:], in_=W1a[:])
            W2a_sb = const.tile([H, 2 + H], dt.float32)
            nc.sync.dma_start(out=W2a_sb[:], in_=W2a[:])
            b1r_sb = const.tile([P, H], dt.float32)
            nc.sync.dma_start(out=b1r_sb[:], in_=b1r[:])
            b2r_sb = const.tile([P, H], dt.float32)
            nc.sync.dma_start(out=b2r_sb[:], in_=b2r[:])
            dstl_sb = const.tile([P, CHV], dt.float32)
            nc.sync.dma_start(out=dstl_sb[:], in_=dstl[:])
            er_all = const.tile([P, NB], dt.float16)
            er_all2 = const.tile([P, NB], dt.float16)
            er2d1 = const.tile([32, NB * 4], dt.float16)
            er2d2 = const.tile([32, NB * 4], dt.float16)

            feat1_s = dram.tile([NPCP, TW], dt.bfloat16)
            feat1_f = dram.tile([TROWS, TW], dt.bfloat16,
                                **({} if single else {"addr_space": "Shared"}))
            feat2_s = dram.tile([NPCP, TW], dt.bfloat16)
            feat2_f = dram.tile([TROWS, TW], dt.bfloat16,
                                **({} if single else {"addr_space": "Shared"}))

            def build_row(pmm, er_dst, er2d, b, fsb4, k):
                """pmm [P, 2+H] f32 = [el | feat | er] -> bf16 row + er."""
                o = k * TW
                nc.vector.tensor_copy(
                    out=fsb4[:, o + ELC:o + ELC + 2].bitcast(dt.float32),
                    in_=pmm[:, 0:1])
                nc.vector.tensor_copy(out=fsb4[:, o + FEATC:o + FEATC + H],
                                      in_=pmm[:, 1:1 + H])
                nc.vector.tensor_copy(out=er_dst[:, b:b + 1],
                                      in_=pmm[:, 1 + H:2 + H])

            TB = 8

            def emit_er2d(er2d, er_dst):
                # l-major: er2d[k, l*NB + b] = er[8k+l of block b]
                nc.scalar.dma_start(
                    out=er2d[:].rearrange("k (l b) -> k l b", b=NB),
                    in_=er_dst[:])

            def write_rows(feat_s, feat_f, b0, nb, fsb4):
                # permuted layout: node (b, p) lives at row p*NB + b, so a
                # TB-batch is contiguous nb*TW per partition (latmul-1 DMA)
                in3 = fsb4[:, :nb * TW].rearrange("p (blk w) -> p blk w",
                                                  w=TW)
                nc.sync.dma_start(
                    out=feat_s[0:NPCP, :]
                        .rearrange("(r blk) w -> r blk w", blk=NB)
                        [:, b0:b0 + nb, :],
                    in_=in3)
                if single:
                    # timing program: share each batch as it completes (the
                    # 8-core program does this via the AllGather instead)
                    nc.sync.dma_start(
                        out=feat_f[0:NPCP, :]
                            .rearrange("(r blk) w -> r blk w", blk=NB)
                            [:, b0:b0 + nb, :],
                        in_=in3)

            # ---- layer-1 table ----
            for b0 in range(0, NB, TB):
                nb = min(TB, NB - b0)
                xt = prod.tile([F, TB * P], dt.float32, tag="xt")
                nc.sync.dma_start(out=xt[:, :nb * P],
                                  in_=xsT[:, b0 * P:(b0 + nb) * P])
                fsb4 = prod.tile([P, TB * TW], dt.bfloat16, tag="fsb4")
                nc.vector.memset(
                    fsb4[:].rearrange("p (blk w) -> p blk w", w=TW)
                    [:, :, ONEC:ONEC + 1], 1.0)
                for k in range(nb):
                    pmm = psumt.tile([P, 2 + H], dt.float32, tag="pmm")
                    nc.tensor.matmul(out=pmm[:], lhsT=xt[:, k * P:(k + 1) * P],
                                     rhs=W1a_sb[:], start=True, stop=True)
                    build_row(pmm, er_all, er2d1, b0 + k, fsb4, k)
                write_rows(feat1_s, feat1_f, b0, nb, fsb4)

            def allgather(src_t, dst_t):
                if single:
                    pass  # per-batch writes in write_rows already filled it
                else:
                    nc.gpsimd.collective_compute(
                        "AllGather", mybir.AluOpType.bypass,
                        replica_groups=[list(range(NCORES))],
                        ins=[src_t[:]], outs=[dst_t[:]],
                    )

            allgather(feat1_s, feat1_f)
            emit_er2d(er2d1, er_all)

            # ---- edge phase ----
            def edge_phase(feat_f, er2d, bias_sb, relu, out_writer):
                for g, gr in enumerate(groups):
                    gp0, nchp = gr["gp0"], gr["nchp"]
                    gv0, nchv = gr["gv0"], gr["nchv"]
                    fxg = fxpool.tile([P, nchp * 8], dt.int16, tag="fxg")
                    nc.sync.dma_start(out=fxg[:],
                                      in_=fidx[:, gp0 * 8:(gp0 + nchp) * 8])
                    mhg = mgpool.tile([32, nchv * P], dt.float8e4, tag="mhg")
                    nc.sync.dma_start(out=mhg[:],
                                      in_=mhi[:, gv0 * P:(gv0 + nchv) * P])
                    log = mgpool.tile([P, nchv * 4], dt.bfloat16, tag="log")
                    nc.sync.dma_start(out=log[:],
                                      in_=lo8[:, gv0 * 4:(gv0 + nchv) * 4])
                    tvg = tvpool.tile([P, nchp * TW], dt.bfloat16, tag="tvg")
                    tv3 = tvg[:].rearrange("p (r e) -> p r e", e=TW)
                    for (s, c0, take) in gr["ops"]:
                        seg_lo = s * SEG
                        seg_hi = min(seg_lo + SEG, TROWS)
                        nc.gpsimd.dma_gather(
                            out_ap=tv3[:, c0 - gp0:c0 - gp0 + take, :],
                            in_ap=feat_f[seg_lo:seg_hi, :],
                            idxs_ap=fxg[:, (c0 - gp0) * 8:
                                        (c0 - gp0 + take) * 8],
                            num_idxs=take * P, num_idxs_reg=take * P,
                            elem_size=TW, queue_num=qn(),
                        )
                    def emit_er(b):
                        # er select stage 1: hi via fp8 [16,128] matmuls ->
                        # [128,8] candidate rows per chunk (emitted one block
                        # ahead so PE's in-order queue can't stall DVE).
                        runs = gr["blocks"][b]
                        Rb = gr["nv"][b]
                        er_ps = psume.tile([P, Rb * 4], dt.float32,
                                           tag="erps")
                        for (s, pcol0, ncols, vcol0, j0, _o) in runs:
                            for rr in range(ncols):
                                gc = vcol0 - gv0 + rr
                                nc.tensor.matmul(
                                    out=er_ps[:, (j0 + rr) * 4:
                                              (j0 + rr + 1) * 4],
                                    lhsT=mhg[:, gc * P:(gc + 1) * P],
                                    rhs=er2d[:].rearrange(
                                        "k (l b2) -> k l b2", b2=NB)
                                        [:, :, b],
                                    start=True, stop=True)
                        return er_ps

                    bl = sorted(gr["blocks"])
                    er_tiles = {bl[0]: emit_er(bl[0])}
                    for bi, b in enumerate(bl):
                        if bi + 1 < len(bl):
                            er_tiles[bl[bi + 1]] = emit_er(bl[bi + 1])
                        runs = gr["blocks"][b]  # (s, pcol0, ncols, vcol0, j0)
                        Rb = gr["nv"][b]
                        er_ps = er_tiles.pop(b)
                        sel8 = eepool.tile([P, Rb * 4], dt.bfloat16,
                                           tag="sel8")
                        for (s, pcol0, ncols, vcol0, j0, _o) in runs:
                            nc.vector.tensor_tensor(
                                out=sel8[:, j0 * 4:(j0 + ncols) * 4],
                                in0=er_ps[:, j0 * 4:(j0 + ncols) * 4],
                                in1=log[:, (vcol0 - gv0) * 4:
                                        (vcol0 - gv0 + ncols) * 4],
                                op=mybir.AluOpType.mult)
                        ers = eepool.tile([P, Rb], dt.float32, tag="ers")
                        nc.vector.tensor_reduce(
                            out=ers[:],
                            in_=sel8[:].rearrange("p (r e) -> p r e", e=4),
                            axis=mybir.AxisListType.X,
                            op=mybir.AluOpType.add)
                        # ee = el + er ; lrelu ; ex
                        ee = eepool.tile([P, Rb], dt.float32, tag="ee")
                        for (s, pcol0, ncols, vcol0, j0, _o) in runs:
                            el = tv3[:, pcol0 - gp0:pcol0 - gp0 + ncols,
                                     ELC:ELC + 2].bitcast(dt.float32)
                            nc.vector.tensor_tensor(
                                out=ee[:, j0:j0 + ncols]
                                    .rearrange("p (r o) -> p r o", o=1),
                                in0=el,
                                in1=ers[:, j0:j0 + ncols]
                                    .rearrange("p (r o) -> p r o", o=1),
                                op=mybir.AluOpType.add)
                        nc.vector.scalar_tensor_tensor(
                            out=ee[:], in0=ee[:], scalar=0.2, in1=ee[:],
                            op0=mybir.AluOpType.mult,
                            op1=mybir.AluOpType.max)
                        ex = eepool.tile([P, Rb], dt.float32, tag="ex")
                        nc.scalar.activation(
                            out=ex[:], in_=ee[:],
                            func=mybir.ActivationFunctionType.Exp)
                        # fused one-hot(+ex) per chunk; rhs = table slice
                        pacc = psum.tile([P, WFW], dt.float32, tag="pacc")
                        done = 0
                        for (s, pcol0, ncols, vcol0, j0, _o) in runs:
                            for rr in range(ncols):
                                oh = ohpool.tile([P, P], dt.bfloat16,
                                                 tag="oh")
                                nc.vector.tensor_scalar(
                                    out=oh[:], in0=iota_sb[:],
                                    scalar1=dstl_sb[:, vcol0 + rr:
                                                    vcol0 + rr + 1],
                                    scalar2=ex[:, j0 + rr:j0 + rr + 1],
                                    op0=mybir.AluOpType.is_equal,
                                    op1=mybir.AluOpType.mult,
                                )
                                nc.tensor.matmul(
                                    out=pacc[:],
                                    lhsT=oh[:],
                                    rhs=tv3[:, pcol0 - gp0 + rr,
                                            ONEC:ONEC + WFW],
                                    start=(done == 0), stop=(done == Rb - 1))
                                done += 1
                        # epilogue
                        den = epipool.tile([P, 1], dt.float32, tag="den")
                        nc.vector.tensor_scalar_add(out=den[:],
                                                    in0=pacc[:, 0:1],
                                                    scalar1=1e-30)
                        rec = epipool.tile([P, 1], dt.float32, tag="rec")
                        nc.vector.reciprocal(out=rec[:], in_=den[:])
                        h = epipool.tile([P, H], dt.float32, tag="h")
                        nc.vector.scalar_tensor_tensor(
                            out=h[:], in0=pacc[:, 1:], scalar=rec[:],
                            in1=bias_sb[:],
                            op0=mybir.AluOpType.mult,
                            op1=mybir.AluOpType.add)
                        if relu:
                            nc.scalar.activation(
                                out=h[:], in_=h[:],
                                func=mybir.ActivationFunctionType.Relu)
                        out_writer(b, h)

            ident = const.tile([P, P], dt.float32)
            from concourse.masks import make_identity
            make_identity(nc, ident[:])

            l1_state = {}

            def l1_writer(b, h):
                pt = psumtt.tile([H, P], dt.float32, tag="pt")
                nc.tensor.transpose(out=pt[:], in_=h[:], identity=ident[:])
                hT = prod.tile([H, P], dt.float32, tag="hT")
                nc.vector.tensor_copy(out=hT[:], in_=pt[:])
                pmm2 = psumt.tile([P, 2 + H], dt.float32, tag="pmm")
                nc.tensor.matmul(out=pmm2[:], lhsT=hT[:], rhs=W2a_sb[:],
                                 start=True, stop=True)
                k = b % TB
                if k == 0:
                    fsb4b = prod.tile([P, TB * TW], dt.bfloat16,
                                      tag="fsb4")
                    nc.vector.memset(
                        fsb4b[:].rearrange("p (blk w) -> p blk w", w=TW)
                        [:, :, ONEC:ONEC + 1], 1.0)
                    l1_state["fsb4"] = fsb4b
                build_row(pmm2, er_all2, er2d2, b, l1_state["fsb4"], k)
                if k == TB - 1 or b == NB - 1:
                    write_rows(feat2_s, feat2_f, b - k, k + 1, l1_state["fsb4"])
                if b == NB - 1:
                    emit_er2d(er2d2, er_all2)

            edge_phase(feat1_f, er2d1, b1r_sb, True, l1_writer)
            allgather(feat2_s, feat2_f)

            def l2_writer(b, h):
                rows = LB if b == NB - 1 else P
                nc.sync.dma_start(out=out_ext[b * P:b * P + rows, :],
                                  in_=h[:rows, :])

            edge_phase(feat2_f, er2d2, b2r_sb, False, l2_writer)

    nc.compile()
    return nc


def _get_program(R_key, single=False):
    key = ("prog", R_key, single)
    if key not in _cache:
        _cache[key] = _build_program(R_key, single=single)
    return _cache[key]


def kernel(x, src, dst, W1, al1, ar1, b1, W2, al2, ar2, b2):
    from concourse.bass_utils import run_bass_kernel_spmd

    in_maps, R_key = _host_prep(x, src, dst, W1, al1, ar1, b1,
                                W2, al2, ar2, b2)
    nc = _get_program(R_key)
    res = run_bass_kernel_spmd(nc, in_maps, list(range(NCORES)))
    out = np.concatenate([res.results[c]["out"] for c in range(NCORES)],
                         axis=0)
    return out.astype(np.float32)


# revision 37
# speedup vs baseline: 1.2449x; 1.0020x over previous
"""GAT (2-layer, single-head) Trainium2 Bass kernel, 8-core SPMD. v2.

Design (vs v1 baseline):
  - dst nodes 1D-sharded (12500/core); edges routed to dst core, bucketed
    by (dst block of 128, src segment of 32768 table rows). Within each
    (group of G blocks, segment) run, buckets pack CONTIGUOUSLY into
    128-slot columns (no per-bucket rounding); a column shared by two
    blocks is processed once per block through its own VIEW column of the
    dstl/MHI/LO8 masks (foreign slots masked out); buckets whose tail gap
    to the next column boundary is < PADT slots are padded instead
    (descriptors are cheaper than an extra view there). One dma_gather op
    (<=1024 idxs, ucode cap) spans buckets within a run.
  - Table row = 128 bf16 (256B, the dma_gather minimum elem):
    [el as f32 (2 slots) | 1.0 | feat(32) | pad]. el kept f32 for softmax
    accuracy; feat bf16.
  - NO per-edge er gather (v1 spent ~50% of its descriptors on it).
    er[dst] is selected on-chip in two levels (dl = 8*hi + lo): per
    chunk a host-shipped fp8 hi-one-hot MHI [16,128] is matmul'd with
    the block's er table redistributed to [16,8] fp16, giving [128,8]
    candidates in PSUM; a host-shipped bf16 lo-one-hot mask [128,8] and
    an X-axis reduce pick the final er per slot. 16B+128B of mask bytes
    per chunk replace 256B of gathered bytes per EDGE.
  - aggregation per chunk: one-hot(+ex) built in ONE fused tensor_scalar
    (is_equal, mult) on bf16 iota (4x DVE mode, ~92ns) with ex and dl as
    per-partition scalars; matmul rhs reads the gathered table slice
    [1|feat] directly; denominator accumulates via the "1" column.
  - Epilogue: out = pacc[:,1:]/pacc[:,0] + bias (+relu), layer-2 table
    built inline; AllGather between layers (excluded from the metric, a
    local copy in the single-core cost program).
"""

import numpy as np
import ml_dtypes

N = 100000
E = 1600000
F = 128
H = 32
NCORES = 8
NPC = N // NCORES          # nodes per core
P = 128
NB = (NPC + P - 1) // P    # dst blocks per core (98; last block 84 rows)
LB = NPC - (NB - 1) * P    # rows in last block
NPCP = NB * P              # padded nodes per core (table rows per core)
TROWS = NCORES * NPCP      # full table rows
TW = 128                   # table row: 128 bf16 = 256B
ELC = 0                    # cols 0-1: el as f32
ONEC = 2                   # col 2: 1.0
FEATC = 3                  # cols 3..34: feat
WFW = 1 + H                # aggregation rhs width: [1 | feat]
SEG = 32768                # src segment size (int16 gather indices)
NSEG = (TROWS + SEG - 1) // SEG
G = 8                      # dst blocks per group tile
NG = (NB + G - 1) // G
CAPC = 8                   # max chunks (1024 indices) per dma_gather op
PADT = 44                  # pad bucket tail to column boundary if gap < PADT

_cache = {}


def _plan(S):
    """Column layout. S: [NB, NSEG] slot counts per bucket (max over cores,
    NOT rounded to 128). Within each (group, seg) run, buckets pack
    contiguously; physical columns = ceil(run_slots/128); a column shared
    by two blocks is processed once per block through its own VIEW column
    of the dstl/MHI/LO8 masks (foreign slots masked to 200/zero).

    groups[g]: gp0/nchp (physical cols), gv0/nchv (view cols),
      ops: (s, pcol0, take<=8) gather ops,
      runbase: {s: run base pcol},
      blocks: {b: [(s, pcol0, ncols, vcol0, j0)]}, nv: {b: Vb}.
    """
    sizes = [8] * 12 + [2]                    # 12*8 + 2 = 98 = NB
    assert sum(sizes) == NB
    starts = [sum(sizes[:i]) for i in range(len(sizes))]
    groups = []
    cp = 0
    cv = 0
    for g in range(len(sizes)):
        bs = list(range(starts[g], starts[g] + sizes[g]))
        gp0, gv0 = cp, cv
        ops = []
        runbase = {}
        blocks = {b: [] for b in bs}
        jloc = {b: 0 for b in bs}
        for s in range(NSEG):
            run_slots = sum(int(S[b, s]) for b in bs)
            if run_slots == 0:
                continue
            runbase[s] = cp
            op_mark = len(ops)
            o = 0
            for b in bs:
                sb = int(S[b, s])
                if sb == 0:
                    continue
                c_lo = o // 128
                c_hi = (o + sb - 1) // 128
                ncols = c_hi - c_lo + 1
                blocks[b].append((s, cp + c_lo, ncols, cv, jloc[b], o))
                jloc[b] += ncols
                cv += ncols
                o += sb
                gap = (-o) % 128
                if 0 < gap < PADT:
                    o += gap
            cols = -(-o // 128)
            # re-emit ops with the padded column count
            del ops[op_mark:]
            off = 0
            while off < cols:
                take = min(CAPC, cols - off)
                ops.append((s, cp + off, take))
                off += take
            cp += cols
        groups.append({"gp0": gp0, "nchp": cp - gp0,
                       "gv0": gv0, "nchv": cv - gv0,
                       "ops": ops, "runbase": runbase,
                       "blocks": blocks, "nv": dict(jloc)})
    return cp, cv, groups


def _host_prep(x, src, dst, W1, al1, ar1, b1, W2, al2, ar2, b2):
    f32, bf16, i16 = np.float32, ml_dtypes.bfloat16, np.int16
    src = np.asarray(src).astype(np.int64)
    dst = np.asarray(dst).astype(np.int64)

    core = dst // NPC
    r = dst % NPC
    b = r // P
    dl = r % P
    loc = src % NPC
    trow_src = (src // NPC) * NPCP + (loc % P) * NB + (loc // P)
    seg = trow_src // SEG
    bgid = core * NB + b

    key = bgid * NSEG + seg
    counts = np.bincount(key, minlength=NCORES * NB * NSEG)
    counts3 = counts.reshape(NCORES, NB, NSEG)
    S = counts3.max(axis=0)                   # [NB, NSEG] slots per bucket
    CHP, CHV, groups = _plan(S)

    order = np.argsort(key, kind="stable")
    s_trow = trow_src[order]
    s_seg = seg[order]
    s_b = b[order]
    s_bgid = bgid[order]
    s_dl = dl[order]

    seg_start = np.concatenate([[0], np.cumsum(counts)])[:-1]
    rank = np.arange(len(order), dtype=np.int64) - seg_start[key[order]]

    runbase_tab = np.zeros((NB, NSEG), np.int64)  # (g,s) run base pcol
    off_tab = np.zeros((NB, NSEG), np.int64)      # bucket slot offset in run
    vcol_tab = np.zeros((NB, NSEG), np.int64)     # bucket first view col
    pcol_tab = np.zeros((NB, NSEG), np.int64)     # bucket first phys col
    for gr in groups:
        for bb, runs in gr["blocks"].items():
            for (s, pcol0, ncols, vcol0, j0, _o) in runs:
                runbase_tab[bb, s] = gr["runbase"][s]
                pcol_tab[bb, s] = pcol0
                vcol_tab[bb, s] = vcol0
                off_tab[bb, s] = _o

    slot = off_tab[s_b, s_seg] + rank
    pcol = runbase_tab[s_b, s_seg] + slot // P
    p = slot % P
    vcol = vcol_tab[s_b, s_seg] + (pcol - pcol_tab[s_b, s_seg])

    # dstl: [128, CHV] destination-local row per (slot, VIEW col); 200 = pad
    dstl = np.full((NCORES, P, CHV), 200.0, f32)
    flat = (s_bgid // NB) * (P * CHV) + p * CHV + vcol
    dstl.reshape(-1)[flat] = s_dl

    # gather indices: [16 wrap, CHP*8], segment-relative; ops start at
    # 8-column boundaries from each run base
    fidx = np.zeros((NCORES, 16, CHP * 8), i16)
    rb = runbase_tab[s_b, s_seg]
    opbase = rb + ((pcol - rb) // CAPC) * CAPC
    i_op = (pcol - opbase) * P + p
    row16 = i_op % 16
    col16 = i_op // 16
    abscol = opbase * 8 + col16
    fflat = (s_bgid // NB) * (16 * CHP * 8) + row16 * (CHP * 8) + abscol
    fidx.reshape(-1)[fflat] = (s_trow - s_seg * SEG).astype(i16)
    fidx = np.tile(fidx, (1, 8, 1))

    # two-level er-select masks: dl = 8*hi + lo
    #   MHI fp8 [16, CH*128]: MHI[k, c*128+i] = (dstl[i,c]//8 == k)
    #   LO8 bf16 [128, CH*8]: LO8[i, c*8+l] = (dstl[i,c]%8 == l) & valid
    one8 = np.float32(1.0).astype(ml_dtypes.float8_e4m3).view(np.uint8)
    MHI = np.zeros((NCORES, 32, CHV * P), np.uint8)
    LO8 = np.zeros((NCORES, P, CHV * 4), bf16)
    for cc in range(NCORES):
        d = dstl[cc].astype(np.int64)         # [P(slots), CH]
        hi = (d // 4).T                       # [CH, i]; pad 200//4=50
        eqh = hi[:, :, None] == np.arange(32)[None, None, :]
        MHI[cc][np.transpose(eqh, (2, 0, 1)).reshape(32, CHV * P)] = one8
        lo = (d % 4).T
        valid = (d < P).T
        eql = (lo[:, :, None] == np.arange(4)[None, None, :]) \
            & valid[:, :, None]               # [CH, i, 4]
        LO8[cc][np.transpose(eql, (1, 0, 2)).reshape(P, CHV * 4)] = bf16(1.0)
    MHI = MHI.view(ml_dtypes.float8_e4m3)

    def aug(W, al, ar):
        W = np.asarray(W, f32)
        Wa = np.zeros((W.shape[0], 2 + H), f32)
        Wa[:, 0] = W @ np.asarray(al, f32)
        Wa[:, 1:1 + H] = W
        Wa[:, 1 + H] = W @ np.asarray(ar, f32)
        return Wa

    W1a = aug(W1, al1, ar1)
    W2a = aug(W2, al2, ar2)
    b1r = np.tile(np.asarray(b1, f32)[None, :], (P, 1))
    b2r = np.tile(np.asarray(b2, f32)[None, :], (P, 1))
    iota = np.tile(np.arange(P, dtype=bf16)[None, :], (P, 1))

    x = np.asarray(x, f32)
    xsT = np.zeros((NCORES, F, NPCP), f32)
    for cc in range(NCORES):
        xsT[cc, :, :NPC] = x[cc * NPC:(cc + 1) * NPC].T

    in_maps = []
    for cc in range(NCORES):
        in_maps.append({
            "xsT": xsT[cc],
            "W1a": W1a, "W2a": W2a, "b1r": b1r, "b2r": b2r, "iota": iota,
            "fidx": fidx[cc], "dstl": dstl[cc],
            "mhi": MHI[cc], "lo8": LO8[cc],
        })
    return in_maps, tuple(int(v) for v in S.reshape(-1))


def _build_program(R_key, single=False):
    import concourse.bacc as bacc
    import concourse.mybir as mybir
    import concourse.tile as tile

    dt = mybir.dt
    S = np.asarray(R_key, np.int64).reshape(NB, NSEG)
    CHP, CHV, groups = _plan(S)
    ncores = 1 if single else NCORES

    nc = bacc.Bacc("TRN2", target_bir_lowering=False, debug=False,
                   num_devices=ncores, num_swdge_queues=4)

    xsT = nc.dram_tensor("xsT", [F, NPCP], dt.float32, kind="ExternalInput")
    W1a = nc.dram_tensor("W1a", [F, 2 + H], dt.float32, kind="ExternalInput")
    W2a = nc.dram_tensor("W2a", [H, 2 + H], dt.float32, kind="ExternalInput")
    b1r = nc.dram_tensor("b1r", [P, H], dt.float32, kind="ExternalInput")
    b2r = nc.dram_tensor("b2r", [P, H], dt.float32, kind="ExternalInput")
    iota = nc.dram_tensor("iota", [P, P], dt.bfloat16, kind="ExternalInput")
    fidx = nc.dram_tensor("fidx", [P, CHP * 8], dt.int16, kind="ExternalInput")
    dstl = nc.dram_tensor("dstl", [P, CHV], dt.float32, kind="ExternalInput")
    mhi = nc.dram_tensor("mhi", [32, CHV * P], dt.float8e4, kind="ExternalInput")
    lo8 = nc.dram_tensor("lo8", [P, CHV * 4], dt.bfloat16, kind="ExternalInput")
    out_ext = nc.dram_tensor("out", [NPC, H], dt.float32, kind="ExternalOutput")

    qn_state = [0]

    def qn():
        qn_state[0] = (qn_state[0] + 1) % 4
        return qn_state[0]

    with tile.TileContext(nc) as tc:
        with (
            tc.tile_pool(name="const", bufs=1) as const,
            tc.tile_pool(name="prod", bufs=4) as prod,
            tc.tile_pool(name="tv", bufs=2) as tvpool,
            tc.tile_pool(name="mg", bufs=2) as mgpool,
            tc.tile_pool(name="fxg", bufs=2) as fxpool,
            tc.tile_pool(name="oh", bufs=12) as ohpool,
            tc.tile_pool(name="ee", bufs=4) as eepool,
            tc.tile_pool(name="epi", bufs=4) as epipool,
            tc.tile_pool(name="ps", bufs=3, space="PSUM") as psum,
            tc.tile_pool(name="pse", bufs=2, space="PSUM") as psume,
            tc.tile_pool(name="pst", bufs=2, space="PSUM") as psumt,
            tc.tile_pool(name="ptt", bufs=1, space="PSUM") as psumtt,
            tc.tile_pool(name="dram", bufs=1, space="DRAM") as dram,
        ):
            iota_sb = const.tile([P, P], dt.bfloat16)
            nc.sync.dma_start(out=iota_sb[:], in_=iota[:])
            W1a_sb = const.tile([F, 2 + H], dt.float32)
            nc.sync.dma_start(out=W1a_sb[:], in_=W1a[:])
            W2a_sb = const.tile([H, 2 + H], dt.float32)
            nc.sync.dma_start(out=W2a_sb[:], in_=W2a[:])
            b1r_sb = const.tile([P, H], dt.float32)
            nc.sync.dma_start(out=b1r_sb[:], in_=b1r[:])
            b2r_sb = const.tile([P, H], dt.float32)
            nc.sync.dma_start(out=b2r_sb[:], in_=b2r[:])
            dstl_sb = const.tile([P, CHV], dt.float32)
            nc.sync.dma_start(out=dstl_sb[:], in_=dstl[:])
            er_all = const.tile([P, NB], dt.float16)
            er_all2 = const.tile([P, NB], dt.float16)
            er2d1 = const.tile([32, NB * 4], dt.float16)
            er2d2 = const.tile([32, NB * 4], dt.float16)

            feat1_s = dram.tile([NPCP, TW], dt.bfloat16)
            feat1_f = dram.tile([TROWS, TW], dt.bfloat16,
                                **({} if single else {"addr_space": "Shared"}))
            feat2_s = dram.tile([NPCP, TW], dt.bfloat16)
            feat2_f = dram.tile([TROWS, TW], dt.bfloat16,
                                **({} if single else {"addr_space": "Shared"}))

            def build_row(pmm, er_dst, er2d, b, fsb4, k):
                """pmm [P, 2+H] f32 = [el | feat | er] -> bf16 row + er."""
                o = k * TW
                nc.vector.tensor_copy(
                    out=fsb4[:, o + ELC:o + ELC + 2].bitcast(dt.float32),
                    in_=pmm[:, 0:1])
                nc.vector.tensor_copy(out=fsb4[:, o + FEATC:o + FEATC + H],
                                      in_=pmm[:, 1:1 + H])
                nc.vector.tensor_copy(out=er_dst[:, b:b + 1],
                                      in_=pmm[:, 1 + H:2 + H])

            TB = 12

            def emit_er2d(er2d, er_dst):
                # l-major: er2d[k, l*NB + b] = er[8k+l of block b]
                nc.scalar.dma_start(
                    out=er2d[:].rearrange("k (l b) -> k l b", b=NB),
                    in_=er_dst[:])

            def write_rows(feat_s, feat_f, b0, nb, fsb4):
                # permuted layout: node (b, p) lives at row p*NB + b, so a
                # TB-batch is contiguous nb*TW per partition (latmul-1 DMA)
                in3 = fsb4[:, :nb * TW].rearrange("p (blk w) -> p blk w",
                                                  w=TW)
                nc.sync.dma_start(
                    out=feat_s[0:NPCP, :]
                        .rearrange("(r blk) w -> r blk w", blk=NB)
                        [:, b0:b0 + nb, :],
                    in_=in3)
                if single:
                    # timing program: share each batch as it completes (the
                    # 8-core program does this via the AllGather instead)
                    nc.sync.dma_start(
                        out=feat_f[0:NPCP, :]
                            .rearrange("(r blk) w -> r blk w", blk=NB)
                            [:, b0:b0 + nb, :],
                        in_=in3)

            # ---- layer-1 table ----
            for b0 in range(0, NB, TB):
                nb = min(TB, NB - b0)
                xt = prod.tile([F, TB * P], dt.float32, tag="xt")
                nc.sync.dma_start(out=xt[:, :nb * P],
                                  in_=xsT[:, b0 * P:(b0 + nb) * P])
                fsb4 = prod.tile([P, TB * TW], dt.bfloat16, tag="fsb4")
                nc.vector.memset(
                    fsb4[:].rearrange("p (blk w) -> p blk w", w=TW)
                    [:, :, ONEC:ONEC + 1], 1.0)
                for k in range(nb):
                    pmm = psumt.tile([P, 2 + H], dt.float32, tag="pmm")
                    nc.tensor.matmul(out=pmm[:], lhsT=xt[:, k * P:(k + 1) * P],
                                     rhs=W1a_sb[:], start=True, stop=True)
                    build_row(pmm, er_all, er2d1, b0 + k, fsb4, k)
                write_rows(feat1_s, feat1_f, b0, nb, fsb4)

            def allgather(src_t, dst_t):
                if single:
                    pass  # per-batch writes in write_rows already filled it
                else:
                    nc.gpsimd.collective_compute(
                        "AllGather", mybir.AluOpType.bypass,
                        replica_groups=[list(range(NCORES))],
                        ins=[src_t[:]], outs=[dst_t[:]],
                    )

            allgather(feat1_s, feat1_f)
            emit_er2d(er2d1, er_all)

            # ---- edge phase ----
            def edge_phase(feat_f, er2d, bias_sb, relu, out_writer):
                for g, gr in enumerate(groups):
                    gp0, nchp = gr["gp0"], gr["nchp"]
                    gv0, nchv = gr["gv0"], gr["nchv"]
                    fxg = fxpool.tile([P, nchp * 8], dt.int16, tag="fxg")
                    nc.sync.dma_start(out=fxg[:],
                                      in_=fidx[:, gp0 * 8:(gp0 + nchp) * 8])
                    mhg = mgpool.tile([32, nchv * P], dt.float8e4, tag="mhg")
                    nc.sync.dma_start(out=mhg[:],
                                      in_=mhi[:, gv0 * P:(gv0 + nchv) * P])
                    log = mgpool.tile([P, nchv * 4], dt.bfloat16, tag="log")
                    nc.sync.dma_start(out=log[:],
                                      in_=lo8[:, gv0 * 4:(gv0 + nchv) * 4])
                    tvg = tvpool.tile([P, nchp * TW], dt.bfloat16, tag="tvg")
                    tv3 = tvg[:].rearrange("p (r e) -> p r e", e=TW)
                    for (s, c0, take) in gr["ops"]:
                        seg_lo = s * SEG
                        seg_hi = min(seg_lo + SEG, TROWS)
                        nc.gpsimd.dma_gather(
                            out_ap=tv3[:, c0 - gp0:c0 - gp0 + take, :],
                            in_ap=feat_f[seg_lo:seg_hi, :],
                            idxs_ap=fxg[:, (c0 - gp0) * 8:
                                        (c0 - gp0 + take) * 8],
                            num_idxs=take * P, num_idxs_reg=take * P,
                            elem_size=TW, queue_num=qn(),
                        )
                    def emit_er(b):
                        # er select stage 1: hi via fp8 [16,128] matmuls ->
                        # [128,8] candidate rows per chunk (emitted one block
                        # ahead so PE's in-order queue can't stall DVE).
                        runs = gr["blocks"][b]
                        Rb = gr["nv"][b]
                        er_ps = psume.tile([P, Rb * 4], dt.float32,
                                           tag="erps")
                        for (s, pcol0, ncols, vcol0, j0, _o) in runs:
                            for rr in range(ncols):
                                gc = vcol0 - gv0 + rr
                                nc.tensor.matmul(
                                    out=er_ps[:, (j0 + rr) * 4:
                                              (j0 + rr + 1) * 4],
                                    lhsT=mhg[:, gc * P:(gc + 1) * P],
                                    rhs=er2d[:].rearrange(
                                        "k (l b2) -> k l b2", b2=NB)
                                        [:, :, b],
                                    start=True, stop=True)
                        return er_ps

                    bl = sorted(gr["blocks"])
                    er_tiles = {bl[0]: emit_er(bl[0])}
                    for bi, b in enumerate(bl):
                        if bi + 1 < len(bl):
                            er_tiles[bl[bi + 1]] = emit_er(bl[bi + 1])
                        runs = gr["blocks"][b]  # (s, pcol0, ncols, vcol0, j0)
                        Rb = gr["nv"][b]
                        er_ps = er_tiles.pop(b)
                        sel8 = eepool.tile([P, Rb * 4], dt.bfloat16,
                                           tag="sel8")
                        for (s, pcol0, ncols, vcol0, j0, _o) in runs:
                            nc.vector.tensor_tensor(
                                out=sel8[:, j0 * 4:(j0 + ncols) * 4],
                                in0=er_ps[:, j0 * 4:(j0 + ncols) * 4],
                                in1=log[:, (vcol0 - gv0) * 4:
                                        (vcol0 - gv0 + ncols) * 4],
                                op=mybir.AluOpType.mult)
                        ers = eepool.tile([P, Rb], dt.float32, tag="ers")
                        nc.vector.tensor_reduce(
                            out=ers[:],
                            in_=sel8[:].rearrange("p (r e) -> p r e", e=4),
                            axis=mybir.AxisListType.X,
                            op=mybir.AluOpType.add)
                        # ee = el + er ; lrelu ; ex
                        ee = eepool.tile([P, Rb], dt.float32, tag="ee")
                        for (s, pcol0, ncols, vcol0, j0, _o) in runs:
                            el = tv3[:, pcol0 - gp0:pcol0 - gp0 + ncols,
                                     ELC:ELC + 2].bitcast(dt.float32)
                            nc.vector.tensor_tensor(
                                out=ee[:, j0:j0 + ncols]
                                    .rearrange("p (r o) -> p r o", o=1),
                                in0=el,
                                in1=ers[:, j0:j0 + ncols]
                                    .rearrange("p (r o) -> p r o", o=1),
                                op=mybir.AluOpType.add)
                        nc.vector.scalar_tensor_tensor(
                            out=ee[:], in0=ee[:], scalar=0.2, in1=ee[:],
                            op0=mybir.AluOpType.mult,
                            op1=mybir.AluOpType.max)
                        ex = eepool.tile([P, Rb], dt.float32, tag="ex")
                        nc.scalar.activation(
                            out=ex[:], in_=ee[:],
                            func=mybir.ActivationFunctionType.Exp)
                        # fused one-hot(+ex) per chunk; rhs = table slice
                        pacc = psum.tile([P, WFW], dt.float32, tag="pacc")
                        done = 0
                        for (s, pcol0, ncols, vcol0, j0, _o) in runs:
                            for rr in range(ncols):
                                oh = ohpool.tile([P, P], dt.bfloat16,
                                                 tag="oh")
                                nc.vector.tensor_scalar(
                                    out=oh[:], in0=iota_sb[:],
                                    scalar1=dstl_sb[:, vcol0 + rr:
                                                    vcol0 + rr + 1],
                                    scalar2=ex[:, j0 + rr:j0 + rr + 1],
                                    op0=mybir.AluOpType.is_equal,
                                    op1=mybir.AluOpType.mult,
                                )
                                nc.tensor.matmul(
                                    out=pacc[:],
                                    lhsT=oh[:],
                                    rhs=tv3[:, pcol0 - gp0 + rr,
                                            ONEC:ONEC + WFW],
                                    start=(done == 0), stop=(done == Rb - 1))
                                done += 1
                        # epilogue
                        den = epipool.tile([P, 1], dt.float32, tag="den")
                        nc.vector.tensor_scalar_add(out=den[:],
                                                    in0=pacc[:, 0:1],
                                                    scalar1=1e-30)
                        rec = epipool.tile([P, 1], dt.float32, tag="rec")
                        nc.vector.reciprocal(out=rec[:], in_=den[:])
                        h = epipool.tile([P, H], dt.float32, tag="h")
                        nc.vector.scalar_tensor_tensor(
                            out=h[:], in0=pacc[:, 1:], scalar=rec[:],
                            in1=bias_sb[:],
                            op0=mybir.AluOpType.mult,
                            op1=mybir.AluOpType.add)
                        if relu:
                            nc.scalar.activation(
                                out=h[:], in_=h[:],
                                func=mybir.ActivationFunctionType.Relu)
                        out_writer(b, h)

            ident = const.tile([P, P], dt.float32)
            from concourse.masks import make_identity
            make_identity(nc, ident[:])

            l1_state = {}

            def l1_writer(b, h):
                pt = psumtt.tile([H, P], dt.float32, tag="pt")
                nc.tensor.transpose(out=pt[:], in_=h[:], identity=ident[:])
                hT = prod.tile([H, P], dt.float32, tag="hT")
                nc.vector.tensor_copy(out=hT[:], in_=pt[:])
                pmm2 = psumt.tile([P, 2 + H], dt.float32, tag="pmm")
                nc.tensor.matmul(out=pmm2[:], lhsT=hT[:], rhs=W2a_sb[:],
                                 start=True, stop=True)
                k = b % TB
                if k == 0:
                    fsb4b = prod.tile([P, TB * TW], dt.bfloat16,
                                      tag="fsb4")
                    nc.vector.memset(
                        fsb4b[:].rearrange("p (blk w) -> p blk w", w=TW)
                        [:, :, ONEC:ONEC + 1], 1.0)
                    l1_state["fsb4"] = fsb4b
                build_row(pmm2, er_all2, er2d2, b, l1_state["fsb4"], k)
                if k == TB - 1 or b == NB - 1:
                    write_rows(feat2_s, feat2_f, b - k, k + 1, l1_state["fsb4"])
                if b == NB - 1:
                    emit_er2d(er2d2, er_all2)

            edge_phase(feat1_f, er2d1, b1r_sb, True, l1_writer)
            allgather(feat2_s, feat2_f)

            def l2_writer(b, h):
                rows = LB if b == NB - 1 else P
                nc.sync.dma_start(out=out_ext[b * P:b * P + rows, :],
                                  in_=h[:rows, :])

            edge_phase(feat2_f, er2d2, b2r_sb, False, l2_writer)

    nc.compile()
    return nc


def _get_program(R_key, single=False):
    key = ("prog", R_key, single)
    if key not in _cache:
        _cache[key] = _build_program(R_key, single=single)
    return _cache[key]


def kernel(x, src, dst, W1, al1, ar1, b1, W2, al2, ar2, b2):
    from concourse.bass_utils import run_bass_kernel_spmd

    in_maps, R_key = _host_prep(x, src, dst, W1, al1, ar1, b1,
                                W2, al2, ar2, b2)
    nc = _get_program(R_key)
    res = run_bass_kernel_spmd(nc, in_maps, list(range(NCORES)))
    out = np.concatenate([res.results[c]["out"] for c in range(NCORES)],
                         axis=0)
    return out.astype(np.float32)


# revision 42
# speedup vs baseline: 1.2489x; 1.0032x over previous
"""GAT (2-layer, single-head) Trainium2 Bass kernel, 8-core SPMD. v2.

Design (vs v1 baseline):
  - dst nodes 1D-sharded (12500/core); edges routed to dst core, bucketed
    by (dst block of 128, src segment of 32768 table rows). Within each
    (group of G blocks, segment) run, buckets pack CONTIGUOUSLY into
    128-slot columns (no per-bucket rounding); a column shared by two
    blocks is processed once per block through its own VIEW column of the
    dstl/MHI/LO8 masks (foreign slots masked out); buckets whose tail gap
    to the next column boundary is < PADT slots are padded instead
    (descriptors are cheaper than an extra view there). One dma_gather op
    (<=1024 idxs, ucode cap) spans buckets within a run.
  - Table row = 128 bf16 (256B, the dma_gather minimum elem):
    [el as f32 (2 slots) | 1.0 | feat(32) | pad]. el kept f32 for softmax
    accuracy; feat bf16.
  - NO per-edge er gather (v1 spent ~50% of its descriptors on it).
    er[dst] is selected on-chip in two levels (dl = 8*hi + lo): per
    chunk a host-shipped fp8 hi-one-hot MHI [16,128] is matmul'd with
    the block's er table redistributed to [16,8] fp16, giving [128,8]
    candidates in PSUM; a host-shipped bf16 lo-one-hot mask [128,8] and
    an X-axis reduce pick the final er per slot. 16B+128B of mask bytes
    per chunk replace 256B of gathered bytes per EDGE.
  - aggregation per chunk: one-hot(+ex) built in ONE fused tensor_scalar
    (is_equal, mult) on bf16 iota (4x DVE mode, ~92ns) with ex and dl as
    per-partition scalars; matmul rhs reads the gathered table slice
    [1|feat] directly; denominator accumulates via the "1" column.
  - Epilogue: out = pacc[:,1:]/pacc[:,0] + bias (+relu), layer-2 table
    built inline; AllGather between layers (excluded from the metric, a
    local copy in the single-core cost program).
"""

import numpy as np
import ml_dtypes

N = 100000
E = 1600000
F = 128
H = 32
NCORES = 8
NPC = N // NCORES          # nodes per core
P = 128
NB = (NPC + P - 1) // P    # dst blocks per core (98; last block 84 rows)
LB = NPC - (NB - 1) * P    # rows in last block
NPCP = NB * P              # padded nodes per core (table rows per core)
TROWS = NCORES * NPCP      # full table rows
TW = 128                   # table row: 128 bf16 = 256B
ELC = 0                    # cols 0-1: el as f32
ONEC = 2                   # col 2: 1.0
FEATC = 3                  # cols 3..34: feat
WFW = 1 + H                # aggregation rhs width: [1 | feat]
SEG = 32768                # src segment size (int16 gather indices)
NSEG = (TROWS + SEG - 1) // SEG
G = 8                      # dst blocks per group tile
NG = (NB + G - 1) // G
CAPC = 8                   # max chunks (1024 indices) per dma_gather op
PADT = 44                  # pad bucket tail to column boundary if gap < PADT

_cache = {}


def _plan(S):
    """Column layout. S: [NB, NSEG] slot counts per bucket (max over cores,
    NOT rounded to 128). Within each (group, seg) run, buckets pack
    contiguously; physical columns = ceil(run_slots/128); a column shared
    by two blocks is processed once per block through its own VIEW column
    of the dstl/MHI/LO8 masks (foreign slots masked to 200/zero).

    groups[g]: gp0/nchp (physical cols), gv0/nchv (view cols),
      ops: (s, pcol0, take<=8) gather ops,
      runbase: {s: run base pcol},
      blocks: {b: [(s, pcol0, ncols, vcol0, j0)]}, nv: {b: Vb}.
    """
    sizes = [8] * 12 + [2]                    # 12*8 + 2 = 98 = NB
    assert sum(sizes) == NB
    starts = [sum(sizes[:i]) for i in range(len(sizes))]
    groups = []
    cp = 0
    cv = 0
    for g in range(len(sizes)):
        bs = list(range(starts[g], starts[g] + sizes[g]))
        gp0, gv0 = cp, cv
        ops = []
        runbase = {}
        blocks = {b: [] for b in bs}
        jloc = {b: 0 for b in bs}
        for s in range(NSEG):
            run_slots = sum(int(S[b, s]) for b in bs)
            if run_slots == 0:
                continue
            runbase[s] = cp
            op_mark = len(ops)
            o = 0
            for b in bs:
                sb = int(S[b, s])
                if sb == 0:
                    continue
                c_lo = o // 128
                c_hi = (o + sb - 1) // 128
                ncols = c_hi - c_lo + 1
                blocks[b].append((s, cp + c_lo, ncols, cv, jloc[b], o))
                jloc[b] += ncols
                cv += ncols
                o += sb
                gap = (-o) % 128
                if 0 < gap < PADT:
                    o += gap
            cols = -(-o // 128)
            # re-emit ops with the padded column count
            del ops[op_mark:]
            off = 0
            while off < cols:
                take = min(CAPC, cols - off)
                ops.append((s, cp + off, take))
                off += take
            cp += cols
        groups.append({"gp0": gp0, "nchp": cp - gp0,
                       "gv0": gv0, "nchv": cv - gv0,
                       "ops": ops, "runbase": runbase,
                       "blocks": blocks, "nv": dict(jloc)})
    return cp, cv, groups


def _host_prep(x, src, dst, W1, al1, ar1, b1, W2, al2, ar2, b2):
    f32, bf16, i16 = np.float32, ml_dtypes.bfloat16, np.int16
    src = np.asarray(src).astype(np.int64)
    dst = np.asarray(dst).astype(np.int64)

    core = dst // NPC
    r = dst % NPC
    b = r // P
    dl = r % P
    loc = src % NPC
    trow_src = (src // NPC) * NPCP + (loc % P) * NB + (loc // P)
    seg = trow_src // SEG
    bgid = core * NB + b

    key = bgid * NSEG + seg
    counts = np.bincount(key, minlength=NCORES * NB * NSEG)
    counts3 = counts.reshape(NCORES, NB, NSEG)
    S = counts3.max(axis=0)                   # [NB, NSEG] slots per bucket
    CHP, CHV, groups = _plan(S)

    order = np.argsort(key, kind="stable")
    s_trow = trow_src[order]
    s_seg = seg[order]
    s_b = b[order]
    s_bgid = bgid[order]
    s_dl = dl[order]

    seg_start = np.concatenate([[0], np.cumsum(counts)])[:-1]
    rank = np.arange(len(order), dtype=np.int64) - seg_start[key[order]]

    runbase_tab = np.zeros((NB, NSEG), np.int64)  # (g,s) run base pcol
    off_tab = np.zeros((NB, NSEG), np.int64)      # bucket slot offset in run
    vcol_tab = np.zeros((NB, NSEG), np.int64)     # bucket first view col
    pcol_tab = np.zeros((NB, NSEG), np.int64)     # bucket first phys col
    for gr in groups:
        for bb, runs in gr["blocks"].items():
            for (s, pcol0, ncols, vcol0, j0, _o) in runs:
                runbase_tab[bb, s] = gr["runbase"][s]
                pcol_tab[bb, s] = pcol0
                vcol_tab[bb, s] = vcol0
                off_tab[bb, s] = _o

    slot = off_tab[s_b, s_seg] + rank
    pcol = runbase_tab[s_b, s_seg] + slot // P
    p = slot % P
    vcol = vcol_tab[s_b, s_seg] + (pcol - pcol_tab[s_b, s_seg])

    # dstl: [128, CHV] destination-local row per (slot, VIEW col); 200 = pad
    dstl = np.full((NCORES, P, CHV), 200.0, f32)
    flat = (s_bgid // NB) * (P * CHV) + p * CHV + vcol
    dstl.reshape(-1)[flat] = s_dl

    # gather indices: [16 wrap, CHP*8], segment-relative; ops start at
    # 8-column boundaries from each run base
    fidx = np.zeros((NCORES, 16, CHP * 8), i16)
    rb = runbase_tab[s_b, s_seg]
    opbase = rb + ((pcol - rb) // CAPC) * CAPC
    i_op = (pcol - opbase) * P + p
    row16 = i_op % 16
    col16 = i_op // 16
    abscol = opbase * 8 + col16
    fflat = (s_bgid // NB) * (16 * CHP * 8) + row16 * (CHP * 8) + abscol
    fidx.reshape(-1)[fflat] = (s_trow - s_seg * SEG).astype(i16)
    fidx = np.tile(fidx, (1, 8, 1))

    # two-level er-select masks: dl = 8*hi + lo
    #   MHI fp8 [16, CH*128]: MHI[k, c*128+i] = (dstl[i,c]//8 == k)
    #   LO8 bf16 [128, CH*8]: LO8[i, c*8+l] = (dstl[i,c]%8 == l) & valid
    one8 = np.float32(1.0).astype(ml_dtypes.float8_e4m3).view(np.uint8)
    MHI = np.zeros((NCORES, 32, CHV * P), np.uint8)
    LO8 = np.zeros((NCORES, P, CHV * 4), bf16)
    for cc in range(NCORES):
        d = dstl[cc].astype(np.int64)         # [P(slots), CH]
        hi = (d // 4).T                       # [CH, i]; pad 200//4=50
        eqh = hi[:, :, None] == np.arange(32)[None, None, :]
        MHI[cc][np.transpose(eqh, (2, 0, 1)).reshape(32, CHV * P)] = one8
        lo = (d % 4).T
        valid = (d < P).T
        eql = (lo[:, :, None] == np.arange(4)[None, None, :]) \
            & valid[:, :, None]               # [CH, i, 4]
        LO8[cc][np.transpose(eql, (1, 0, 2)).reshape(P, CHV * 4)] = bf16(1.0)
    MHI = MHI.view(ml_dtypes.float8_e4m3)

    def aug(W, al, ar):
        W = np.asarray(W, f32)
        Wa = np.zeros((W.shape[0], 2 + H), f32)
        Wa[:, 0] = W @ np.asarray(al, f32)
        Wa[:, 1:1 + H] = W
        Wa[:, 1 + H] = W @ np.asarray(ar, f32)
        return Wa

    W1a = aug(W1, al1, ar1)
    W2a = aug(W2, al2, ar2)
    b1r = np.tile(np.asarray(b1, f32)[None, :], (P, 1))
    b2r = np.tile(np.asarray(b2, f32)[None, :], (P, 1))
    iota = np.tile(np.arange(P, dtype=bf16)[None, :], (P, 1))

    x = np.asarray(x, f32)
    xsT = np.zeros((NCORES, F, NPCP), f32)
    for cc in range(NCORES):
        xsT[cc, :, :NPC] = x[cc * NPC:(cc + 1) * NPC].T

    in_maps = []
    for cc in range(NCORES):
        in_maps.append({
            "xsT": xsT[cc],
            "W1a": W1a, "W2a": W2a, "b1r": b1r, "b2r": b2r, "iota": iota,
            "fidx": fidx[cc], "dstl": dstl[cc],
            "mhi": MHI[cc], "lo8": LO8[cc],
        })
    return in_maps, tuple(int(v) for v in S.reshape(-1))


def _build_program(R_key, single=False):
    import concourse.bacc as bacc
    import concourse.mybir as mybir
    import concourse.tile as tile

    dt = mybir.dt
    S = np.asarray(R_key, np.int64).reshape(NB, NSEG)
    CHP, CHV, groups = _plan(S)
    ncores = 1 if single else NCORES

    nc = bacc.Bacc("TRN2", target_bir_lowering=False, debug=False,
                   num_devices=ncores, num_swdge_queues=4)

    xsT = nc.dram_tensor("xsT", [F, NPCP], dt.float32, kind="ExternalInput")
    W1a = nc.dram_tensor("W1a", [F, 2 + H], dt.float32, kind="ExternalInput")
    W2a = nc.dram_tensor("W2a", [H, 2 + H], dt.float32, kind="ExternalInput")
    b1r = nc.dram_tensor("b1r", [P, H], dt.float32, kind="ExternalInput")
    b2r = nc.dram_tensor("b2r", [P, H], dt.float32, kind="ExternalInput")
    iota = nc.dram_tensor("iota", [P, P], dt.bfloat16, kind="ExternalInput")
    fidx = nc.dram_tensor("fidx", [P, CHP * 8], dt.int16, kind="ExternalInput")
    dstl = nc.dram_tensor("dstl", [P, CHV], dt.float32, kind="ExternalInput")
    mhi = nc.dram_tensor("mhi", [32, CHV * P], dt.float8e4, kind="ExternalInput")
    lo8 = nc.dram_tensor("lo8", [P, CHV * 4], dt.bfloat16, kind="ExternalInput")
    out_ext = nc.dram_tensor("out", [NPC, H], dt.float32, kind="ExternalOutput")

    qn_state = [0]

    def qn():
        qn_state[0] = (qn_state[0] + 1) % 4
        return qn_state[0]

    with tile.TileContext(nc) as tc:
        with (
            tc.tile_pool(name="const", bufs=1) as const,
            tc.tile_pool(name="prod", bufs=4) as prod,
            tc.tile_pool(name="tv", bufs=2) as tvpool,
            tc.tile_pool(name="mg", bufs=2) as mgpool,
            tc.tile_pool(name="fxg", bufs=2) as fxpool,
            tc.tile_pool(name="oh", bufs=16) as ohpool,
            tc.tile_pool(name="ee", bufs=4) as eepool,
            tc.tile_pool(name="epi", bufs=4) as epipool,
            tc.tile_pool(name="ps", bufs=2, space="PSUM") as psum,
            tc.tile_pool(name="pse", bufs=3, space="PSUM") as psume,
            tc.tile_pool(name="pst", bufs=2, space="PSUM") as psumt,
            tc.tile_pool(name="ptt", bufs=1, space="PSUM") as psumtt,
            tc.tile_pool(name="dram", bufs=1, space="DRAM") as dram,
        ):
            iota_sb = const.tile([P, P], dt.bfloat16)
            nc.sync.dma_start(out=iota_sb[:], in_=iota[:])
            W1a_sb = const.tile([F, 2 + H], dt.float32)
            nc.sync.dma_start(out=W1a_sb[:], in_=W1a[:])
            W2a_sb = const.tile([H, 2 + H], dt.float32)
            nc.sync.dma_start(out=W2a_sb[:], in_=W2a[:])
            b1r_sb = const.tile([P, H], dt.float32)
            nc.sync.dma_start(out=b1r_sb[:], in_=b1r[:])
            b2r_sb = const.tile([P, H], dt.float32)
            nc.sync.dma_start(out=b2r_sb[:], in_=b2r[:])
            dstl_sb = const.tile([P, CHV], dt.float32)
            nc.sync.dma_start(out=dstl_sb[:], in_=dstl[:])
            er_all = const.tile([P, NB], dt.float16)
            er_all2 = const.tile([P, NB], dt.float16)
            er2d1 = const.tile([32, NB * 4], dt.float16)
            er2d2 = const.tile([32, NB * 4], dt.float16)

            feat1_s = dram.tile([NPCP, TW], dt.bfloat16)
            feat1_f = dram.tile([TROWS, TW], dt.bfloat16,
                                **({} if single else {"addr_space": "Shared"}))
            feat2_s = dram.tile([NPCP, TW], dt.bfloat16)
            feat2_f = dram.tile([TROWS, TW], dt.bfloat16,
                                **({} if single else {"addr_space": "Shared"}))

            def build_row(pmm, er_dst, er2d, b, fsb4, k):
                """pmm [P, 2+H] f32 = [el | feat | er] -> bf16 row + er."""
                o = k * TW
                nc.vector.tensor_copy(
                    out=fsb4[:, o + ELC:o + ELC + 2].bitcast(dt.float32),
                    in_=pmm[:, 0:1])
                nc.vector.tensor_copy(out=fsb4[:, o + FEATC:o + FEATC + H],
                                      in_=pmm[:, 1:1 + H])
                nc.vector.tensor_copy(out=er_dst[:, b:b + 1],
                                      in_=pmm[:, 1 + H:2 + H])

            TB = 12

            def emit_er2d(er2d, er_dst):
                # l-major: er2d[k, l*NB + b] = er[8k+l of block b]
                nc.scalar.dma_start(
                    out=er2d[:].rearrange("k (l b) -> k l b", b=NB),
                    in_=er_dst[:])

            def write_rows(feat_s, feat_f, b0, nb, fsb4):
                # permuted layout: node (b, p) lives at row p*NB + b, so a
                # TB-batch is contiguous nb*TW per partition (latmul-1 DMA)
                in3 = fsb4[:, :nb * TW].rearrange("p (blk w) -> p blk w",
                                                  w=TW)
                nc.sync.dma_start(
                    out=feat_s[0:NPCP, :]
                        .rearrange("(r blk) w -> r blk w", blk=NB)
                        [:, b0:b0 + nb, :],
                    in_=in3)
                if single:
                    # timing program: share each batch as it completes (the
                    # 8-core program does this via the AllGather instead)
                    nc.sync.dma_start(
                        out=feat_f[0:NPCP, :]
                            .rearrange("(r blk) w -> r blk w", blk=NB)
                            [:, b0:b0 + nb, :],
                        in_=in3)

            # ---- layer-1 table ----
            for b0 in range(0, NB, TB):
                nb = min(TB, NB - b0)
                xt = prod.tile([F, TB * P], dt.float32, tag="xt")
                nc.sync.dma_start(out=xt[:, :nb * P],
                                  in_=xsT[:, b0 * P:(b0 + nb) * P])
                fsb4 = prod.tile([P, TB * TW], dt.bfloat16, tag="fsb4")
                nc.vector.memset(
                    fsb4[:].rearrange("p (blk w) -> p blk w", w=TW)
                    [:, :, ONEC:ONEC + 1], 1.0)
                for k in range(nb):
                    pmm = psumt.tile([P, 2 + H], dt.float32, tag="pmm")
                    nc.tensor.matmul(out=pmm[:], lhsT=xt[:, k * P:(k + 1) * P],
                                     rhs=W1a_sb[:], start=True, stop=True)
                    build_row(pmm, er_all, er2d1, b0 + k, fsb4, k)
                write_rows(feat1_s, feat1_f, b0, nb, fsb4)

            def allgather(src_t, dst_t):
                if single:
                    pass  # per-batch writes in write_rows already filled it
                else:
                    nc.gpsimd.collective_compute(
                        "AllGather", mybir.AluOpType.bypass,
                        replica_groups=[list(range(NCORES))],
                        ins=[src_t[:]], outs=[dst_t[:]],
                    )

            allgather(feat1_s, feat1_f)
            emit_er2d(er2d1, er_all)

            # ---- edge phase ----
            def edge_phase(feat_f, er2d, bias_sb, relu, out_writer):
                for g, gr in enumerate(groups):
                    gp0, nchp = gr["gp0"], gr["nchp"]
                    gv0, nchv = gr["gv0"], gr["nchv"]
                    fxg = fxpool.tile([P, nchp * 8], dt.int16, tag="fxg")
                    nc.sync.dma_start(out=fxg[:],
                                      in_=fidx[:, gp0 * 8:(gp0 + nchp) * 8])
                    mhg = mgpool.tile([32, nchv * P], dt.float8e4, tag="mhg")
                    nc.sync.dma_start(out=mhg[:],
                                      in_=mhi[:, gv0 * P:(gv0 + nchv) * P])
                    log = mgpool.tile([P, nchv * 4], dt.bfloat16, tag="log")
                    nc.sync.dma_start(out=log[:],
                                      in_=lo8[:, gv0 * 4:(gv0 + nchv) * 4])
                    tvg = tvpool.tile([P, nchp * TW], dt.bfloat16, tag="tvg")
                    tv3 = tvg[:].rearrange("p (r e) -> p r e", e=TW)
                    for (s, c0, take) in gr["ops"]:
                        seg_lo = s * SEG
                        seg_hi = min(seg_lo + SEG, TROWS)
                        nc.gpsimd.dma_gather(
                            out_ap=tv3[:, c0 - gp0:c0 - gp0 + take, :],
                            in_ap=feat_f[seg_lo:seg_hi, :],
                            idxs_ap=fxg[:, (c0 - gp0) * 8:
                                        (c0 - gp0 + take) * 8],
                            num_idxs=take * P, num_idxs_reg=take * P,
                            elem_size=TW, queue_num=qn(),
                        )
                    def emit_er(b):
                        # er select stage 1: hi via fp8 [16,128] matmuls ->
                        # [128,8] candidate rows per chunk (emitted one block
                        # ahead so PE's in-order queue can't stall DVE).
                        runs = gr["blocks"][b]
                        Rb = gr["nv"][b]
                        er_ps = psume.tile([P, Rb * 4], dt.float32,
                                           tag="erps")
                        for (s, pcol0, ncols, vcol0, j0, _o) in runs:
                            for rr in range(ncols):
                                gc = vcol0 - gv0 + rr
                                nc.tensor.matmul(
                                    out=er_ps[:, (j0 + rr) * 4:
                                              (j0 + rr + 1) * 4],
                                    lhsT=mhg[:, gc * P:(gc + 1) * P],
                                    rhs=er2d[:].rearrange(
                                        "k (l b2) -> k l b2", b2=NB)
                                        [:, :, b],
                                    start=True, stop=True)
                        return er_ps

                    bl = sorted(gr["blocks"])
                    er_tiles = {bl[0]: emit_er(bl[0])}
                    for bi, b in enumerate(bl):
                        if bi + 1 < len(bl):
                            er_tiles[bl[bi + 1]] = emit_er(bl[bi + 1])
                        runs = gr["blocks"][b]  # (s, pcol0, ncols, vcol0, j0)
                        Rb = gr["nv"][b]
                        er_ps = er_tiles.pop(b)
                        sel8 = eepool.tile([P, Rb * 4], dt.bfloat16,
                                           tag="sel8")
                        for (s, pcol0, ncols, vcol0, j0, _o) in runs:
                            nc.vector.tensor_tensor(
                                out=sel8[:, j0 * 4:(j0 + ncols) * 4],
                                in0=er_ps[:, j0 * 4:(j0 + ncols) * 4],
                                in1=log[:, (vcol0 - gv0) * 4:
                                        (vcol0 - gv0 + ncols) * 4],
                                op=mybir.AluOpType.mult)
                        ers = eepool.tile([P, Rb], dt.float32, tag="ers")
                        nc.vector.tensor_reduce(
                            out=ers[:],
                            in_=sel8[:].rearrange("p (r e) -> p r e", e=4),
                            axis=mybir.AxisListType.X,
                            op=mybir.AluOpType.add)
                        # ee = el + er ; lrelu ; ex
                        ee = eepool.tile([P, Rb], dt.float32, tag="ee")
                        for (s, pcol0, ncols, vcol0, j0, _o) in runs:
                            el = tv3[:, pcol0 - gp0:pcol0 - gp0 + ncols,
                                     ELC:ELC + 2].bitcast(dt.float32)
                            nc.vector.tensor_tensor(
                                out=ee[:, j0:j0 + ncols]
                                    .rearrange("p (r o) -> p r o", o=1),
                                in0=el,
                                in1=ers[:, j0:j0 + ncols]
                                    .rearrange("p (r o) -> p r o", o=1),
                                op=mybir.AluOpType.add)
                        nc.vector.scalar_tensor_tensor(
                            out=ee[:], in0=ee[:], scalar=0.2, in1=ee[:],
                            op0=mybir.AluOpType.mult,
                            op1=mybir.AluOpType.max)
                        ex = eepool.tile([P, Rb], dt.float32, tag="ex")
                        nc.scalar.activation(
                            out=ex[:], in_=ee[:],
                            func=mybir.ActivationFunctionType.Exp)
                        # fused one-hot(+ex) per chunk; rhs = table slice
                        pacc = psum.tile([P, WFW], dt.float32, tag="pacc")
                        done = 0
                        for (s, pcol0, ncols, vcol0, j0, _o) in runs:
                            for rr in range(ncols):
                                oh = ohpool.tile([P, P], dt.bfloat16,
                                                 tag="oh")
                                nc.vector.tensor_scalar(
                                    out=oh[:], in0=iota_sb[:],
                                    scalar1=dstl_sb[:, vcol0 + rr:
                                                    vcol0 + rr + 1],
                                    scalar2=ex[:, j0 + rr:j0 + rr + 1],
                                    op0=mybir.AluOpType.is_equal,
                                    op1=mybir.AluOpType.mult,
                                )
                                nc.tensor.matmul(
                                    out=pacc[:],
                                    lhsT=oh[:],
                                    rhs=tv3[:, pcol0 - gp0 + rr,
                                            ONEC:ONEC + WFW],
                                    start=(done == 0), stop=(done == Rb - 1))
                                done += 1
                        # epilogue
                        den = epipool.tile([P, 1], dt.float32, tag="den")
                        nc.vector.tensor_scalar_add(out=den[:],
                                                    in0=pacc[:, 0:1],
                                                    scalar1=1e-30)
                        rec = epipool.tile([P, 1], dt.float32, tag="rec")
                        nc.vector.reciprocal(out=rec[:], in_=den[:])
                        h = epipool.tile([P, H], dt.float32, tag="h")
                        nc.vector.scalar_tensor_tensor(
                            out=h[:], in0=pacc[:, 1:], scalar=rec[:],
                            in1=bias_sb[:],
                            op0=mybir.AluOpType.mult,
                            op1=mybir.AluOpType.add)
                        if relu:
                            nc.scalar.activation(
                                out=h[:], in_=h[:],
                                func=mybir.ActivationFunctionType.Relu)
                        out_writer(b, h)

            ident = const.tile([P, P], dt.float32)
            from concourse.masks import make_identity
            make_identity(nc, ident[:])

            l1_state = {}

            def l1_writer(b, h):
                pt = psumtt.tile([H, P], dt.float32, tag="pt")
                nc.tensor.transpose(out=pt[:], in_=h[:], identity=ident[:])
                hT = prod.tile([H, P], dt.float32, tag="hT")
                nc.vector.tensor_copy(out=hT[:], in_=pt[:])
                pmm2 = psumt.tile([P, 2 + H], dt.float32, tag="pmm")
                nc.tensor.matmul(out=pmm2[:], lhsT=hT[:], rhs=W2a_sb[:],
                                 start=True, stop=True)
                k = b % TB
                if k == 0:
                    fsb4b = prod.tile([P, TB * TW], dt.bfloat16,
                                      tag="fsb4")
                    nc.vector.memset(
                        fsb4b[:].rearrange("p (blk w) -> p blk w", w=TW)
                        [:, :, ONEC:ONEC + 1], 1.0)
                    l1_state["fsb4"] = fsb4b
                build_row(pmm2, er_all2, er2d2, b, l1_state["fsb4"], k)
                if k == TB - 1 or b == NB - 1:
                    write_rows(feat2_s, feat2_f, b - k, k + 1, l1_state["fsb4"])
                if b == NB - 1:
                    emit_er2d(er2d2, er_all2)

            edge_phase(feat1_f, er2d1, b1r_sb, True, l1_writer)
            allgather(feat2_s, feat2_f)

            def l2_writer(b, h):
                rows = LB if b == NB - 1 else P
                nc.sync.dma_start(out=out_ext[b * P:b * P + rows, :],
                                  in_=h[:rows, :])

            edge_phase(feat2_f, er2d2, b2r_sb, False, l2_writer)

    nc.compile()
    return nc


def _get_program(R_key, single=False):
    key = ("prog", R_key, single)
    if key not in _cache:
        _cache[key] = _build_program(R_key, single=single)
    return _cache[key]


def kernel(x, src, dst, W1, al1, ar1, b1, W2, al2, ar2, b2):
    from concourse.bass_utils import run_bass_kernel_spmd

    in_maps, R_key = _host_prep(x, src, dst, W1, al1, ar1, b1,
                                W2, al2, ar2, b2)
    nc = _get_program(R_key)
    res = run_bass_kernel_spmd(nc, in_maps, list(range(NCORES)))
    out = np.concatenate([res.results[c]["out"] for c in range(NCORES)],
                         axis=0)
    return out.astype(np.float32)
